# revision 7
# baseline (speedup 1.0000x reference)
"""Trainium2 Bass kernel for nn_Attention (GQA + RoPE + sliding-window mask).

Sharding: tensor-parallel over heads across 8 cores (4 q heads + 1 kv head
per core). The reference's quirky output flatten ((H,S,D)->(H,D,S)->
reshape(S, H*D)) makes the final projection row-shard by head block: core c
produces rows [256c, 256c+256) of the (2048, 4096) result with no collective.

Fast path (pure causal mask, the shape this problem produces):
  * phase 1 (QKV projections) and phase 3 (output projection) run as
    fp8-e4m3 hi/lo pairs in DoubleRow perf mode: X @ W ~= Xh@Wh + Xh@Wl +
    Xl@Wh with Xh = fp8(X), Xl = fp8(X - Xh) - 0.75x the PE time of one
    bf16 pass and more accurate than bf16 (~9-10 effective mantissa bits).
  * attention (phase 2) is interleaved INTO phase 1: block (qs, h) only
    needs s-tiles <= 4qs+3, so blocks weave between projection s-tiles with
    score rows emitted between x-chunk groups - softmax (ACT) latency hides
    under the projection matmuls.
  * no-max softmax (causal logits here are O(10), exp is safe in fp32),
    exp reads score PSUM directly with accum_out row sums; only the
    128-wide triangular diagonal block gets a mask add; diagonal PV
    matmuls are narrowed to the live query columns.
  * P transposed via DMA xbar; wo chunks prefetched/dripped so the big
    loads never head-of-line block the latency-critical transposes.

Fallback (any other mask): the v1 kernel (per-chunk mask add + 2-pass
max/exp softmax), correct for arbitrary additive masks.
"""

import numpy as np
from contextlib import ExitStack

P = 128
D = 128  # head dim
NH = 4   # q heads per core
CORES = 8
NEG_THRESH = -1e8


def build_attention_nc(
    SEQ,
    DIM,
    plan,
    n_uniq,
    p_dt_name="bfloat16",
    wo_dt_name="bfloat16",
    proj_dt_name="bfloat16",
    proj_f32r=True,
    score_f32r=True,
    use_dma_t=True,
):
    """Build the per-core Bass program.

    plan: list over q-tiles i (SEQ//128 entries) of lists of (chunk_idx, uid)
          where uid == -1 means the 512-wide chunk needs no mask add, else the
          index into the maskb tensor. Chunks absent from the list are fully
          masked (skipped).
    """
    import concourse.bass as bass
    import concourse.bacc as bacc
    import concourse.mybir as mybir
    import concourse.tile as tile
    from concourse.masks import make_identity

    f32 = mybir.dt.float32
    f32r = mybir.dt.float32r
    P_DT = getattr(mybir.dt, p_dt_name)
    WO_DT = getattr(mybir.dt, wo_dt_name)
    PJ_DT = getattr(mybir.dt, proj_dt_name)
    pj_f32r = proj_f32r and proj_dt_name == "float32"

    ST = SEQ // P          # 16 s-tiles
    DD = DIM // P          # 32 contraction tiles
    KC = SEQ // 512        # 4 key chunks
    QS = SEQ // 512        # 4 query supers
    EW = NH * D            # 512 q-projection width
    JT = 2 * SEQ // P      # 32 j-tiles for final matmul
    MC = DIM // 512        # 8 output chunks
    ITILES = (NH * 64) // P  # 2 output row tiles
    assert NH == 4 and SEQ % 512 == 0 and DIM % 512 == 0

    def mm_cast(ap, use_r):
        return ap.bitcast(f32r) if use_r else ap

    nc = bacc.Bacc(trn_type="TRN2", debug=False, num_devices=CORES)

    # x pre-tiled on host: xT[p, st, t, si] = x[st*128+si, t*128+p] so each
    # streamed chunk is one DMA with 2KB contiguous per-partition runs
    xT = nc.dram_tensor("xT", [P, ST, DD, P], PJ_DT, kind="ExternalInput").ap()
    wT = nc.dram_tensor("wT", [DIM, EW + 2 * D], PJ_DT, kind="ExternalInput").ap()
    cs = nc.dram_tensor("cs", [SEQ, EW], f32, kind="ExternalInput").ap()
    mb = nc.dram_tensor(
        "maskb", [max(n_uniq, 1), P, 512], f32, kind="ExternalInput"
    ).ap()
    woT = nc.dram_tensor("woT", [2 * SEQ, DIM], WO_DT, kind="ExternalInput").ap()
    out = nc.dram_tensor("out", [NH * 64, DIM], f32, kind="ExternalOutput").ap()

    with tile.TileContext(nc) as tc, ExitStack() as ctx:
        const = ctx.enter_context(tc.tile_pool(name="const", bufs=1))
        idF = const.tile([P, P], f32)
        make_identity(nc, idF)
        idP = const.tile([P, P], P_DT)
        make_identity(nc, idP)
        zeros = const.tile([P, 512], f32)
        nc.vector.memset(zeros, 0.0)

        pers = ctx.enter_context(tc.tile_pool(name="pers", bufs=1))
        QTt = pers.tile([P, NH, ST * P], f32)   # [d, h, s]
        KTt = pers.tile([P, ST * P], f32)       # [d, s]
        Vt = pers.tile([P, ST, D], P_DT)        # [k(part), ktile, d]
        if n_uniq > 0:
            mbt = pers.tile([P, n_uniq, 512], f32)

        # ---------------- phase 1: projections + rope + layout ----------------
        with (
            tc.tile_pool(name="wpool", bufs=1) as wpool,
            tc.tile_pool(name="xpool", bufs=6) as xpool,
            tc.tile_pool(name="cspool", bufs=2) as cspool,
            tc.tile_pool(name="rpool", bufs=2) as rpool,
            tc.tile_pool(name="qps", bufs=2, space="PSUM") as qps,
            tc.tile_pool(name="kvps", bufs=2, space="PSUM") as kvps,
            tc.tile_pool(name="tps", bufs=2, space="PSUM") as tps,
            tc.tile_pool(name="t2ps", bufs=2, space="PSUM") as t2ps,
        ):
            XGW = min(8, DD)
            wTt = wpool.tile([P, DD, EW + 2 * D], PJ_DT)
            wTr = wT.rearrange("(t p) e -> p t e", p=P)

            XG = min(8, DD)  # dd-tiles per streamed x chunk
            NG = DD // XG
            xTr = xT
            # Interleave the weight-chunk loads with s-tile 0's x chunks so
            # the first matmuls start as soon as chunk 0 of each lands.
            st0_x = []
            for g in range(NG):
                xTt = xpool.tile([P, XG, P], PJ_DT, tag="xT")
                nc.sync.dma_start(
                    out=xTt, in_=xTr[:, 0, g * XG : (g + 1) * XG, :]
                )
                st0_x.append(xTt)
                gw = g % (DD // XGW)
                nc.sync.dma_start(
                    out=wTt[:, gw * XGW : (gw + 1) * XGW, :],
                    in_=wTr[:, gw * XGW : (gw + 1) * XGW, :],
                )
            for st in range(ST):
                cst = cspool.tile([P, EW], f32, tag="cs")
                nc.sync.dma_start(out=cst, in_=cs[st * P : (st + 1) * P, :])

                Qp = qps.tile([P, EW], f32, tag="Qp")
                KVp = kvps.tile([P, 2 * D], f32, tag="KVp")
                for g in range(DD // XG):
                    if st == 0:
                        xTt = st0_x[g]
                    else:
                        xTt = xpool.tile([P, XG, P], PJ_DT, tag="xT")
                        nc.sync.dma_start(
                            out=xTt,
                            in_=xTr[:, st, g * XG : (g + 1) * XG, :],
                        )
                    for tt in range(XG):
                        t = g * XG + tt
                        lhsT = mm_cast(xTt[:, tt, :], pj_f32r)
                        nc.tensor.matmul(
                            Qp,
                            lhsT,
                            mm_cast(wTt[:, t, 0:EW], pj_f32r),
                            start=(t == 0),
                            stop=(t == DD - 1),
                        )
                        nc.tensor.matmul(
                            KVp,
                            lhsT,
                            mm_cast(wTt[:, t, EW : EW + 2 * D], pj_f32r),
                            start=(t == 0),
                            stop=(t == DD - 1),
                        )

                # rope via strided even/odd halves (2-level APs only — 3-level
                # APs overflow the fixed ISA instruction encoding).
                # tensor_tensor_reduce instead of tensor_tensor: the plain TT
                # ISA struct has a single sync-wait slot and walrus codegen
                # rejects the PE+DMA double wait Tile emits here; the TTR/ISA
                # struct carries up to 8. accum outputs are dummies.
                def ttr_ew(out, in0, in1, op):
                    nc.vector.tensor_tensor(out=out, in0=in0, in1=in1, op=op)

                A_ = mybir.AluOpType
                HF = EW // 2  # 256: cos table width for q
                rq = rpool.tile([P, EW], f32, tag="rq")
                t1 = rpool.tile([P, HF], f32, tag="t1")
                t2 = rpool.tile([P, HF], f32, tag="t2")
                q_ev, q_od = Qp[:, 0:EW:2], Qp[:, 1:EW:2]
                cosr, sinr = cst[:, 0:HF], cst[:, HF : 2 * HF]
                ttr_ew(t1, q_ev, cosr, A_.mult)
                ttr_ew(t2, q_od, sinr, A_.mult)
                ttr_ew(rq[:, 0:EW:2], t1, t2, A_.subtract)
                ttr_ew(t1, q_ev, sinr, A_.mult)
                ttr_ew(t2, q_od, cosr, A_.mult)
                ttr_ew(rq[:, 1:EW:2], t1, t2, A_.add)

                rk = rpool.tile([P, D], f32, tag="rk")
                k_ev, k_od = KVp[:, 0:D:2], KVp[:, 1:D:2]
                cosk, sink = cst[:, 0 : D // 2], cst[:, HF : HF + D // 2]
                ttr_ew(t1[:, 0 : D // 2], k_ev, cosk, A_.mult)
                ttr_ew(t2[:, 0 : D // 2], k_od, sink, A_.mult)
                ttr_ew(rk[:, 0:D:2], t1[:, 0 : D // 2], t2[:, 0 : D // 2], A_.subtract)
                ttr_ew(t1[:, 0 : D // 2], k_ev, sink, A_.mult)
                ttr_ew(t2[:, 0 : D // 2], k_od, cosk, A_.mult)
                ttr_ew(rk[:, 1:D:2], t1[:, 0 : D // 2], t2[:, 0 : D // 2], A_.add)

                # V -> bf16 [k, d] layout (ACT copy, cast)
                nc.scalar.activation(
                    out=Vt[:, st, :],
                    in_=KVp[:, D : 2 * D],
                    func=mybir.ActivationFunctionType.Copy,
                )

                # transpose rq (per head) and rk into [d, s] layouts
                T1 = tps.tile([P, EW], f32, tag="T1")
                for h in range(NH):
                    nc.tensor.transpose(
                        T1[:, h * P : (h + 1) * P], rq[:, h * P : (h + 1) * P], idF
                    )
                # write as f32r so walrus accepts them as f32r matmul operands
                nc.vector.tensor_copy(
                    out=mm_cast(QTt[:, :, st * P : (st + 1) * P], score_f32r),
                    in_=T1.rearrange("p (h s) -> p h s", h=NH),
                )
                T2 = t2ps.tile([P, P], f32, tag="T2")
                nc.tensor.transpose(T2, rk, idF)
                nc.vector.tensor_copy(
                    out=mm_cast(KTt[:, st * P : (st + 1) * P], score_f32r), in_=T2
                )

        # ---------------- phase 2: attention ----------------
        if n_uniq > 0:
            nc.sync.dma_start(out=mbt, in_=mb.rearrange("u p m -> p u m"))
        apool = ctx.enter_context(tc.tile_pool(name="apool", bufs=1))
        # split by head-pair so phase 3's first row-tile can start once
        # heads 0-1 finish, overlapping the rest of phase 2
        Aall = [
            apool.tile([P, 2 * ST * D], P_DT, name=f"Aall{i}")
            for i in range(NH // 2)
        ]
        with (
            tc.tile_pool(name="ptsb", bufs=2) as ptsb,
            tc.tile_pool(name="spool", bufs=6) as spool,
            tc.tile_pool(name="ppool", bufs=4) as ppool,
            tc.tile_pool(name="stat", bufs=12) as stat,
            tc.tile_pool(name="atsb", bufs=3) as atsb,
            tc.tile_pool(name="sps", bufs=2, space="PSUM") as sps,
            tc.tile_pool(name="ptps", bufs=2, space="PSUM") as ptps,
            tc.tile_pool(name="atps", bufs=1, space="PSUM") as atps,
            tc.tile_pool(name="aps", bufs=1, space="PSUM") as aps,
            tc.tile_pool(name="wopool", bufs=2 if n_uniq <= 4 else 1) as wopool,
            tc.tile_pool(name="osb", bufs=2) as osb,
            tc.tile_pool(name="ops", bufs=2, space="PSUM") as ops,
        ):
            for h in range(NH):
                for qs in range(QS):
                    PTt = ptsb.tile([P, ST, 512], P_DT, tag="PT")
                    kts_used = set()
                    recips = []
                    pt_written = set()
                    for qi in range(4):
                        i = 4 * qs + qi
                        row = plan[i]
                        if not row:
                            recips.append(None)
                            continue
                        pairs = [row[k : k + 2] for k in range(0, len(row), 2)]
                        stats = stat.tile([P, KC], f32, tag="stats")
                        ncols = 0
                        S_tiles = []
                        for pr in pairs:
                            W = 512 * len(pr)
                            S = sps.tile([P, 1024], f32, tag="S")
                            Ssb = spool.tile([P, 1024], f32, tag="Ssb")
                            masked_any = any(uid >= 0 for (_, uid) in pr)
                            for k, (c, uid) in enumerate(pr):
                                sl = S[:, k * 512 : (k + 1) * 512]
                                nc.tensor.matmul(
                                    sl,
                                    mm_cast(
                                        QTt[:, h, i * P : (i + 1) * P], score_f32r
                                    ),
                                    mm_cast(
                                        KTt[:, c * 512 : (c + 1) * 512], score_f32r
                                    ),
                                    start=True,
                                    stop=True,
                                )
                                if uid >= 0:
                                    nc.vector.tensor_add(sl, sl, mbt[:, uid, :])
                                # copy PSUM->SBUF to free the score bank early;
                                # alternate DVE/ACT to balance engine load
                                dst = Ssb[:, k * 512 : (k + 1) * 512]
                                if (i + k) % 2 == 0:
                                    nc.vector.tensor_copy(out=dst, in_=sl)
                                else:
                                    nc.scalar.activation(
                                        out=dst,
                                        in_=sl,
                                        func=mybir.ActivationFunctionType.Copy,
                                    )
                                if masked_any or len(pr) == 1:
                                    nc.vector.tensor_reduce(
                                        out=stats[:, ncols : ncols + 1],
                                        in_=dst,
                                        axis=mybir.AxisListType.X,
                                        op=mybir.AluOpType.max,
                                    )
                                    ncols += 1
                            if not masked_any and len(pr) == 2:
                                # one pair-wide max over both chunks (SBUF 2x)
                                nc.vector.tensor_reduce(
                                    out=stats[:, ncols : ncols + 1],
                                    in_=Ssb,
                                    axis=mybir.AxisListType.X,
                                    op=mybir.AluOpType.max,
                                )
                                ncols += 1
                            S_tiles.append((Ssb, pr))
                        negm = stat.tile([P, 1], f32, tag="negm")
                        nc.vector.tensor_reduce(
                            out=negm,
                            in_=stats[:, 0:ncols],
                            axis=mybir.AxisListType.X,
                            op=mybir.AluOpType.max,
                            negate=True,
                        )
                        sums = stat.tile([P, KC], f32, tag="sums")
                        for k, (Sk, pr) in enumerate(S_tiles):
                            W = 512 * len(pr)
                            Pt = ppool.tile([P, 1024], P_DT, tag="P")
                            nc.scalar.activation(
                                out=Pt[:, 0:W],
                                in_=Sk[:, 0:W],
                                func=mybir.ActivationFunctionType.Exp,
                                bias=negm,
                                accum_out=sums[:, k : k + 1],
                            )
                            # transpose P [q, k] -> PT [k, q]
                            for j, (c, uid) in enumerate(pr):
                                if use_dma_t:
                                    nc.sync.dma_start_transpose(
                                        out=PTt[
                                            :, 4 * c : 4 * c + 4, qi * P : (qi + 1) * P
                                        ],
                                        in_=Pt[:, j * 512 : (j + 1) * 512],
                                    )
                                else:
                                    PTp = ptps.tile([P, 512], P_DT, tag="PTp")
                                    for jj in range(4):
                                        nc.tensor.transpose(
                                            PTp[:, jj * P : (jj + 1) * P],
                                            Pt[:, j * 512 + jj * P : j * 512 + (jj + 1) * P],
                                            idP,
                                        )
                                    nc.vector.tensor_copy(
                                        out=PTt[:, 4 * c : 4 * c + 4, qi * P : (qi + 1) * P],
                                        in_=PTp.rearrange("p (kt q) -> p kt q", kt=4),
                                    )
                                for jj in range(4):
                                    kts_used.add(4 * c + jj)
                                    pt_written.add((4 * c + jj, qi))
                        denom = stat.tile([P, 1], f32, tag="denom")
                        nc.vector.tensor_reduce(
                            out=denom,
                            in_=sums[:, 0 : len(S_tiles)],
                            axis=mybir.AxisListType.X,
                            op=mybir.AluOpType.add,
                        )
                        recip = stat.tile([P, 1], f32, tag="recip")
                        nc.vector.reciprocal(recip, denom)
                        recips.append(recip)

                    # zero-fill PT holes (only for non-causal masks)
                    kts = sorted(kts_used)
                    for kt in kts:
                        for qi in range(4):
                            if (kt, qi) not in pt_written and recips[qi] is not None:
                                nc.vector.memset(
                                    PTt[:, kt, qi * P : (qi + 1) * P], 0.0
                                )
                            elif recips[qi] is None:
                                nc.vector.memset(
                                    PTt[:, kt, qi * P : (qi + 1) * P], 0.0
                                )

                    if not kts:
                        continue
                    # PV: A^T[d, q] accumulated over key tiles
                    At = atps.tile([P, 512], f32, tag="At")
                    for n, kt in enumerate(kts):
                        nc.tensor.matmul(
                            At,
                            Vt[:, kt, :],
                            PTt[:, kt, :],
                            start=(n == 0),
                            stop=(n == len(kts) - 1),
                        )
                    Atsb = atsb.tile([P, 512], P_DT, tag="Atsb")
                    nc.vector.tensor_copy(out=Atsb, in_=At)
                    Ap = aps.tile([P, 512], P_DT, tag="Ap")
                    for qi in range(4):
                        nc.tensor.transpose(
                            Ap[:, qi * P : (qi + 1) * P],
                            Atsb[:, qi * P : (qi + 1) * P],
                            idP,
                        )
                    # Aall layout: [sp, (t*2 + dd)*128 + hb*64 + p] so the final
                    # matmul's stationary slices are contiguous (walrus requires
                    # a single free dim on weight APs)
                    Ah = Aall[h // 2]
                    hb = h % 2
                    for qi in range(4):
                        i = 4 * qs + qi
                        # dview[sp, p, dd] == Ah[:, i*256 + dd*128 + hb*64 + p]
                        dview = Ah[:, i * 2 * P : (i + 1) * 2 * P].rearrange(
                            "a (dd j) -> a dd j", dd=2
                        )[:, :, hb * 64 : hb * 64 + 64].rearrange(
                            "a dd p -> a p dd"
                        )
                        if recips[qi] is None:
                            nc.vector.memset(dview, 0.0)
                            continue
                        nc.scalar.activation(
                            out=dview,
                            in_=Ap[:, qi * P : (qi + 1) * P].rearrange(
                                "a (p two) -> a p two", two=2
                            ),
                            func=mybir.ActivationFunctionType.Copy,
                            scale=recips[qi],
                        )

            # ---------------- phase 3: output projection ----------------
            for mc in range(MC):
                wot = wopool.tile([P, JT, 512], WO_DT, tag="wo")
                nc.sync.dma_start(
                    out=wot,
                    in_=woT[:, mc * 512 : (mc + 1) * 512].rearrange(
                        "(t p) m -> p t m", p=P
                    ),
                )
                for it in range(ITILES):
                    O = ops.tile([P, 512], f32, tag="O")
                    Av = Aall[it]
                    for jt in range(JT):
                        ddj, t = jt // ST, jt % ST
                        lhsT = Av[:, (t * 2 + ddj) * P : (t * 2 + ddj + 1) * P]
                        nc.tensor.matmul(
                            O,
                            lhsT,
                            wot[:, jt, :],
                            start=(jt == 0),
                            stop=(jt == JT - 1),
                        )
                    Ot = osb.tile([P, 512], f32, tag="Ot")
                    nc.scalar.activation(
                        out=Ot, in_=O, func=mybir.ActivationFunctionType.Copy
                    )
                    nc.sync.dma_start(
                        out=out[it * P : (it + 1) * P, mc * 512 : (mc + 1) * 512],
                        in_=Ot,
                    )

    # Bacc.compile() legalizes sync (>=2 waits split into EventSemaphore
    # instructions — this walrus caps every instruction at ONE sync wait)
    nc.compile()
    return nc


def analyze_mask(mask, SEQ):
    """Classify 128x512 mask blocks: skip / free / masked(dedup uid)."""
    ST = SEQ // P
    KC = SEQ // 512
    uniq = {}
    blocks = []
    plan = []
    for i in range(ST):
        row = []
        for c in range(KC):
            blk = mask[i * P : (i + 1) * P, c * 512 : (c + 1) * 512]
            if (blk <= NEG_THRESH).all():
                continue
            if not blk.any():
                row.append((c, -1))
            else:
                key = blk.tobytes()
                if key not in uniq:
                    uniq[key] = len(blocks)
                    blocks.append(np.ascontiguousarray(blk))
                row.append((c, uniq[key]))
        if not row:
            # fully masked query rows: keep all chunks so softmax matches
            # the reference's uniform distribution over -1e9 logits
            for c in range(KC):
                blk = mask[i * P : (i + 1) * P, c * 512 : (c + 1) * 512]
                key = blk.tobytes()
                if key not in uniq:
                    uniq[key] = len(blocks)
                    blocks.append(np.ascontiguousarray(blk))
                row.append((c, uniq[key]))
        plan.append(row)
    return plan, blocks


WS1 = 1024.0  # host pre-scale on wq/wk/wv before fp8 (values ~0.02*N(0,1))
WS3 = 256.0   # host pre-scale on wo before fp8


def build_attention_v7(
    SEQ,
    DIM,
    p_dt_name="bfloat16",
):
    import concourse.bass as bass
    import concourse.bacc as bacc
    import concourse.mybir as mybir
    import concourse.tile as tile
    from concourse.masks import make_identity

    f32 = mybir.dt.float32
    f8 = mybir.dt.float8e4
    P_DT = getattr(mybir.dt, p_dt_name)
    A_ = mybir.AluOpType
    AF = mybir.ActivationFunctionType
    DR = mybir.MatmulPerfMode.DoubleRow

    ST = SEQ // P          # 16 s-tiles
    DD = DIM // P          # 32 contraction tiles
    QS = SEQ // 512        # 4 query supers
    EW = NH * D            # 512 q-projection width
    JT = 2 * SEQ // P      # 32 j-tiles for final matmul
    MC = DIM // 512        # 8 output chunks
    ITILES = (NH * 64) // P  # 2 output row tiles
    assert NH == 4 and SEQ % 512 == 0 and DIM % 512 == 0

    nc = bacc.Bacc(trn_type="TRN2", debug=False, num_devices=CORES)

    xTh = nc.dram_tensor("xTh", [P, ST, DD, P], f8, kind="ExternalInput").ap()
    xTl = nc.dram_tensor("xTl", [P, ST, DD, P], f8, kind="ExternalInput").ap()
    wTh = nc.dram_tensor("wTh", [DIM, EW + 2 * D], f8, kind="ExternalInput").ap()
    wTl = nc.dram_tensor("wTl", [DIM, EW + 2 * D], f8, kind="ExternalInput").ap()
    cs = nc.dram_tensor("cs", [SEQ, EW], P_DT, kind="ExternalInput").ap()
    tri = nc.dram_tensor("tri", [P, P], f32, kind="ExternalInput").ap()
    # wo row blocks permuted host-side: block jt' = 2t+dd <- original dd*16+t
    woh = nc.dram_tensor("woh", [2 * SEQ, DIM], f8, kind="ExternalInput").ap()
    wol = nc.dram_tensor("wol", [2 * SEQ, DIM], f8, kind="ExternalInput").ap()
    out = nc.dram_tensor("out", [NH * 64, DIM], P_DT, kind="ExternalOutput").ap()

    with tile.TileContext(nc) as tc, ExitStack() as ctx:
        const = ctx.enter_context(tc.tile_pool(name="const", bufs=1))
        idP = const.tile([P, P], P_DT)
        make_identity(nc, idP)

        pers = ctx.enter_context(tc.tile_pool(name="pers", bufs=1))
        QTt = pers.tile([P, NH, ST * P], P_DT)   # [d, h, s]
        KTt = pers.tile([P, ST * P], P_DT)       # [d, s]
        Vt = pers.tile([P, ST, D], P_DT)         # [k(part), ktile, d]
        trit = pers.tile([P, P], f32)
        nc.sync.dma_start(out=trit, in_=tri)

        # mc=0 wo chunk is prefetched during the interleaved region (the only
        # chunk SBUF has room for before the phase-1 pools close)
        wopre = ctx.enter_context(tc.tile_pool(name="wopre", bufs=1))
        w0h = wopre.tile([P, 2 * SEQ // P, 512], f8, tag="wph")
        w0l = wopre.tile([P, 2 * SEQ // P, 512], f8, tag="wpl")
        wohr_ = woh.rearrange("(t p) m -> p t m", p=P)
        wolr_ = wol.rearrange("(t p) m -> p t m", p=P)

        def drip_w0(qq):
            q4 = (2 * SEQ // P) // 4
            nc.sync.dma_start(
                out=w0h[:, qq * q4 : (qq + 1) * q4, :],
                in_=wohr_[:, qq * q4 : (qq + 1) * q4, 0:512],
            )
            nc.sync.dma_start(
                out=w0l[:, qq * q4 : (qq + 1) * q4, :],
                in_=wolr_[:, qq * q4 : (qq + 1) * q4, 0:512],
            )

        apool = ctx.enter_context(tc.tile_pool(name="apool", bufs=1))
        Aall_h = [
            apool.tile([P, 2 * ST * D], f8, name=f"Aallh{i}") for i in range(NH // 2)
        ]
        Aall_l = [
            apool.tile([P, 2 * ST * D], f8, name=f"Aalll{i}") for i in range(NH // 2)
        ]
        # attention pools live through phase 1+2 and the tail
        ptsb = ctx.enter_context(tc.tile_pool(name="ptsb", bufs=2))
        ppool = ctx.enter_context(tc.tile_pool(name="ppool", bufs=8))
        stat = ctx.enter_context(tc.tile_pool(name="stat", bufs=12))
        atsb = ctx.enter_context(tc.tile_pool(name="atsb", bufs=3))
        s1ps = ctx.enter_context(tc.tile_pool(name="s1ps", bufs=3, space="PSUM"))
        atps = ctx.enter_context(tc.tile_pool(name="atps", bufs=1, space="PSUM"))
        aps = ctx.enter_context(tc.tile_pool(name="aps", bufs=1, space="PSUM"))

        blocks = [(qs, h) for qs in range(QS) for h in range(NH)]  # j = 4qs+h
        state = {}

        def rowA(j, qi, PTt, recips):
            qs, h = blocks[j]
            i = 4 * qs + qi
            f = i // 4      # fully-allowed 512-chunks
            dsub = i % 4    # full 128-subtiles in the diagonal chunk
            sums = stat.tile([P, 4], f32, tag="sums")
            ncol = 0
            lhsQ = QTt[:, h, i * P : (i + 1) * P]
            for c in range(f):
                S1t = s1ps.tile([P, 512], f32, tag="S1")
                nc.tensor.matmul(
                    S1t,
                    lhsQ,
                    KTt[:, c * 512 : (c + 1) * 512],
                    start=True,
                    stop=True,
                )
                Pt = ppool.tile([P, 512], P_DT, tag="P1")
                nc.scalar.activation(
                    out=Pt,
                    in_=S1t,
                    func=AF.Exp,
                    accum_out=sums[:, ncol : ncol + 1],
                )
                ncol += 1
                nc.sync.dma_start_transpose(
                    out=PTt[:, 4 * c : 4 * c + 4, qi * P : (qi + 1) * P],
                    in_=Pt,
                )
            # diagonal chunk, truncated to (dsub+1)*128 columns
            w = (dsub + 1) * P
            S1t = s1ps.tile([P, 512], f32, tag="S1")
            nc.tensor.matmul(
                S1t[:, 0:w],
                lhsQ,
                KTt[:, f * 512 : f * 512 + w],
                start=True,
                stop=True,
            )
            nc.vector.tensor_add(S1t[:, dsub * P : w], S1t[:, dsub * P : w], trit)
            Pt = ppool.tile([P, 512], P_DT, tag="P1")
            nc.scalar.activation(
                out=Pt[:, 0:w],
                in_=S1t[:, 0:w],
                func=AF.Exp,
                accum_out=sums[:, ncol : ncol + 1],
            )
            ncol += 1
            nc.sync.dma_start_transpose(
                out=PTt[:, 4 * f : 4 * f + dsub + 1, qi * P : (qi + 1) * P],
                in_=Pt[:, 0:w],
            )
            # masked-out subtiles of the diagonal chunk are never read: the
            # PV matmuls for diagonal key-tiles are narrowed to the live
            # query columns instead
            denom = stat.tile([P, 1], f32, tag="denom")
            nc.vector.tensor_reduce(
                out=denom, in_=sums[:, 0:ncol], axis=mybir.AxisListType.X, op=A_.add
            )
            recip = stat.tile([P, 1], f32, tag="recip")
            nc.vector.reciprocal(recip, denom)
            recips.append(recip)

        def open_A(j):
            PTt = ptsb.tile([P, ST, 512], P_DT, tag="PT")
            recips = []
            state[j] = (PTt, recips)
            return PTt, recips

        def stage_B(j):
            qs, h = blocks[j]
            PTt, recips = state.pop(j)
            nkt = 4 * qs + 4
            At = atps.tile([P, 512], f32, tag="At")
            for n in range(nkt):
                # diagonal key-tiles only reach query columns >= off
                off = max(0, n - 4 * qs) * P
                nc.tensor.matmul(
                    At[:, off:512],
                    Vt[:, n, :],
                    PTt[:, n, off:512],
                    start=(n == 0),
                    stop=(n == nkt - 1),
                )
            Atsb = atsb.tile([P, 512], P_DT, tag="Atsb")
            nc.vector.tensor_copy(out=Atsb, in_=At)
            Ap = aps.tile([P, 512], P_DT, tag="Ap")
            for qi in range(4):
                nc.tensor.transpose(
                    Ap[:, qi * P : (qi + 1) * P],
                    Atsb[:, qi * P : (qi + 1) * P],
                    idP,
                )
            hb = h % 2

            def dv(Aarr):
                return Aarr[h // 2][:, i * 2 * P : (i + 1) * 2 * P].rearrange(
                    "a (dd j) -> a dd j", dd=2
                )[:, :, hb * 64 : hb * 64 + 64].rearrange("a dd p -> a p dd")

            for qi in range(4):
                i = 4 * qs + qi
                # normalize on DVE, then split into fp8 hi + residual lo for
                # the DoubleRow output projection
                th = atsb.tile([P, P], f32, tag="th")
                nc.vector.tensor_scalar_mul(
                    th, Ap[:, qi * P : (qi + 1) * P], recips[qi]
                )
                thv = th.rearrange("a (p two) -> a p two", two=2)
                dh, dl = dv(Aall_h), dv(Aall_l)
                nc.vector.tensor_copy(out=dh, in_=thv)
                nc.vector.tensor_tensor(out=dl, in0=thv, in1=dh, op=A_.subtract)

        # ------------- phase 1 with interleaved attention blocks -------------
        with (
            tc.tile_pool(name="wpool", bufs=1) as wpool,
            tc.tile_pool(name="xpool", bufs=8) as xpool,
            tc.tile_pool(name="cspool", bufs=2) as cspool,
            tc.tile_pool(name="rpool", bufs=2) as rpool,
            tc.tile_pool(name="qps", bufs=1, space="PSUM") as qps,
            tc.tile_pool(name="kvsh", bufs=1, space="PSUM") as kvsh,
        ):
            wTth = wpool.tile([P, DD, EW + 2 * D], f8)
            wTtl = wpool.tile([P, DD, EW + 2 * D], f8)
            wTrh = wTh.rearrange("(t p) e -> p t e", p=P)
            wTrl = wTl.rearrange("(t p) e -> p t e", p=P)

            XG = min(8, DD)  # dd-tiles per streamed x chunk
            NG = DD // XG
            # interleave s-tile-0 x chunks with weight loads (weights in 8
            # sub-loads per array so the first matmuls start early)
            st0_x = []
            for g in range(NG):
                xh = xpool.tile([P, XG, P], f8, tag="xh")
                nc.sync.dma_start(out=xh, in_=xTh[:, 0, g * XG : (g + 1) * XG, :])
                xl = xpool.tile([P, XG, P], f8, tag="xl")
                nc.sync.dma_start(out=xl, in_=xTl[:, 0, g * XG : (g + 1) * XG, :])
                st0_x.append((xh, xl))
                for half in range(2):
                    gw = 2 * g + half
                    nc.sync.dma_start(
                        out=wTth[:, gw * 4 : (gw + 1) * 4, :],
                        in_=wTrh[:, gw * 4 : (gw + 1) * 4, :],
                    )
                    nc.sync.dma_start(
                        out=wTtl[:, gw * 4 : (gw + 1) * 4, :],
                        in_=wTrl[:, gw * 4 : (gw + 1) * 4, :],
                    )
            def qkv_terms(Qp, KVp, xh, xl, g, first_flag=True):
                for tp in range(XG // 2):
                    t = g * XG + 2 * tp
                    first = t == 0 and first_flag
                    last = t == DD - 2
                    lh = xh[:, 2 * tp : 2 * tp + 2, :]
                    ll = xl[:, 2 * tp : 2 * tp + 2, :]
                    terms = ((lh, wTth), (lh, wTtl), (ll, wTth))
                    for k, (lhsT, wt) in enumerate(terms):
                        nc.tensor.matmul(
                            Qp,
                            lhsT,
                            wt[:, t : t + 2, 0:EW],
                            start=(first and k == 0),
                            stop=(last and k == 2),
                            perf_mode=DR,
                        )
                    for k, (lhsT, wt) in enumerate(terms):
                        nc.tensor.matmul(
                            KVp,
                            lhsT,
                            wt[:, t : t + 2, EW : EW + 2 * D],
                            start=(first and k == 0),
                            stop=(last and k == 2),
                            perf_mode=DR,
                        )

            # s-tiles 0 and 1 are paired: the weight stream is the DMA
            # bottleneck at kernel start, so each weight granule feeds two
            # s-tiles' matmuls (s1 borrows PSUM from the still-idle
            # attention pools)
            cst0 = cspool.tile([P, EW], P_DT, tag="cs")
            nc.sync.dma_start(out=cst0, in_=cs[0:P, :])
            cst1 = cspool.tile([P, EW], P_DT, tag="cs")
            nc.sync.dma_start(out=cst1, in_=cs[P : 2 * P, :])
            Qp0 = qps.tile([P, EW], f32, tag="Qp")
            KVp0 = kvsh.tile([P, 2 * D], f32, tag="KVp")
            Qp1 = s1ps.tile([P, 512], f32, tag="S1")
            KVt1 = atps.tile([P, 512], f32, tag="At")
            KVp1 = KVt1[:, 0 : 2 * D]
            st1_x = []
            for g in range(NG):
                xh1 = xpool.tile([P, XG, P], f8, tag="xh")
                nc.sync.dma_start(out=xh1, in_=xTh[:, 1, g * XG : (g + 1) * XG, :])
                xl1 = xpool.tile([P, XG, P], f8, tag="xl")
                nc.sync.dma_start(out=xl1, in_=xTl[:, 1, g * XG : (g + 1) * XG, :])
                st1_x.append((xh1, xl1))
                xh0, xl0 = st0_x[g]
                qkv_terms(Qp0, KVp0, xh0, xl0, g)
                qkv_terms(Qp1, KVp1, xh1, xl1, g)

            for st in range(ST):
                j = st - 4  # attention block woven into this s-tile
                if j >= 0:
                    PTt, recips = open_A(j)
                if st == 0:
                    Qp, KVp, cst = Qp0, KVp0, cst0
                elif st == 1:
                    Qp, KVp, cst = Qp1, KVp1, cst1
                else:
                    cst = cspool.tile([P, EW], P_DT, tag="cs")
                    nc.sync.dma_start(out=cst, in_=cs[st * P : (st + 1) * P, :])
                    Qp = qps.tile([P, EW], f32, tag="Qp")
                    KVp = kvsh.tile([P, 2 * D], f32, tag="KVp")
                for g in range(DD // XG if st >= 2 else 0):
                    if st == 0:
                        xh, xl = st0_x[g]
                    else:
                        xh = xpool.tile([P, XG, P], f8, tag="xh")
                        nc.sync.dma_start(
                            out=xh, in_=xTh[:, st, g * XG : (g + 1) * XG, :]
                        )
                        xl = xpool.tile([P, XG, P], f8, tag="xl")
                        nc.sync.dma_start(
                            out=xl, in_=xTl[:, st, g * XG : (g + 1) * XG, :]
                        )
                    for tp in range(XG // 2):
                        t = g * XG + 2 * tp
                        first = t == 0
                        last = t == DD - 2
                        lh = xh[:, 2 * tp : 2 * tp + 2, :]
                        ll = xl[:, 2 * tp : 2 * tp + 2, :]
                        terms = (
                            (lh, wTth),
                            (lh, wTtl),
                            (ll, wTth),
                        )
                        for k, (lhsT, wt) in enumerate(terms):
                            nc.tensor.matmul(
                                Qp,
                                lhsT,
                                wt[:, t : t + 2, 0:EW],
                                start=(first and k == 0),
                                stop=(last and k == 2),
                                perf_mode=DR,
                            )
                        for k, (lhsT, wt) in enumerate(terms):
                            nc.tensor.matmul(
                                KVp,
                                lhsT,
                                wt[:, t : t + 2, EW : EW + 2 * D],
                                start=(first and k == 0),
                                stop=(last and k == 2),
                                perf_mode=DR,
                            )
                    # one attention row between x-chunk groups keeps ACT fed
                    # while PE grinds the projection matmuls
                    if j >= 0:
                        rowA(j, g, PTt, recips)

                # free the Q/KV PSUM banks fast: one copy each, rope reads SBUF
                qsb = rpool.tile([P, EW], f32, tag="qsb")
                nc.vector.tensor_copy(out=qsb, in_=Qp)
                kvsb = rpool.tile([P, 2 * D], f32, tag="kvsb")
                nc.vector.tensor_copy(out=kvsb, in_=KVp)

                def ttr_ew(out_, in0, in1, op):
                    nc.vector.tensor_tensor(out=out_, in0=in0, in1=in1, op=op)

                HF = EW // 2
                rq = rpool.tile([P, EW], P_DT, tag="rq")
                t1 = rpool.tile([P, HF], f32, tag="t1")
                t2 = rpool.tile([P, HF], f32, tag="t2")
                q_ev, q_od = qsb[:, 0:EW:2], qsb[:, 1:EW:2]
                cosr, sinr = cst[:, 0:HF], cst[:, HF : 2 * HF]
                ttr_ew(t1, q_ev, cosr, A_.mult)
                ttr_ew(t2, q_od, sinr, A_.mult)
                ttr_ew(rq[:, 0:EW:2], t1, t2, A_.subtract)
                ttr_ew(t1, q_ev, sinr, A_.mult)
                ttr_ew(t2, q_od, cosr, A_.mult)
                ttr_ew(rq[:, 1:EW:2], t1, t2, A_.add)

                rk = rpool.tile([P, D], P_DT, tag="rk")
                k_ev, k_od = kvsb[:, 0:D:2], kvsb[:, 1:D:2]
                cosk, sink = cst[:, 0 : D // 2], cst[:, HF : HF + D // 2]
                ttr_ew(t1[:, 0 : D // 2], k_ev, cosk, A_.mult)
                ttr_ew(t2[:, 0 : D // 2], k_od, sink, A_.mult)
                ttr_ew(rk[:, 0:D:2], t1[:, 0 : D // 2], t2[:, 0 : D // 2], A_.subtract)
                ttr_ew(t1[:, 0 : D // 2], k_ev, sink, A_.mult)
                ttr_ew(t2[:, 0 : D // 2], k_od, cosk, A_.mult)
                ttr_ew(rk[:, 1:D:2], t1[:, 0 : D // 2], t2[:, 0 : D // 2], A_.add)

                nc.scalar.activation(
                    out=Vt[:, st, :],
                    in_=kvsb[:, D : 2 * D],
                    func=AF.Copy,
                    scale=float(1.0 / WS1),
                )

                # transposes into [d, s] layouts (bf16); q heads + k share one
                # PSUM bank (5*128 bf16 = 1.25KB)
                TT = kvsh.tile([P, (NH + 1) * P], P_DT, tag="TT")
                for h in range(NH):
                    nc.tensor.transpose(
                        TT[:, h * P : (h + 1) * P], rq[:, h * P : (h + 1) * P], idP
                    )
                nc.tensor.transpose(TT[:, NH * P : (NH + 1) * P], rk, idP)
                nc.vector.tensor_copy(
                    out=QTt[:, :, st * P : (st + 1) * P],
                    in_=TT[:, 0 : NH * P].rearrange("p (h s) -> p h s", h=NH),
                )
                nc.vector.tensor_copy(
                    out=KTt[:, st * P : (st + 1) * P],
                    in_=TT[:, NH * P : (NH + 1) * P],
                )

                if j >= 1:
                    stage_B(j - 1)
                if st >= 12:
                    drip_w0(st - 12)

        # ---------------- tail: qs=3 blocks + output projection ----------------
        with (
            tc.tile_pool(name="wopool", bufs=2) as wopool,
            tc.tile_pool(name="osb", bufs=2) as osb,
            tc.tile_pool(name="ops", bufs=2, space="PSUM") as ops,
        ):
            wot_tiles = {}
            wqueue = []
            units = {}
            wohr = woh.rearrange("(t p) m -> p t m", p=P)
            wolr = wol.rearrange("(t p) m -> p t m", p=P)

            def alloc_wot(mc):
                wth = wopool.tile([P, JT, 512], f8, tag="woh")
                wtl = wopool.tile([P, JT, 512], f8, tag="wol")
                wot_tiles[mc] = (wth, wtl)
                # quarter q covers t-pairs [4q, 4q+4); hi then lo
                for qq in range(4):
                    wqueue.append((mc, qq, 0))
                    wqueue.append((mc, qq, 1))

            def drip(n):
                for _ in range(min(n, len(wqueue))):
                    mc, qq, lo = wqueue.pop(0)
                    wt = wot_tiles[mc][lo]
                    src = wolr if lo else wohr
                    nc.sync.dma_start(
                        out=wt[:, qq * (JT // 4) : (qq + 1) * (JT // 4), :],
                        in_=src[
                            :,
                            qq * (JT // 4) : (qq + 1) * (JT // 4),
                            mc * 512 : (mc + 1) * 512,
                        ],
                    )

            def load_wot(mc):
                alloc_wot(mc)
                drip(8)

            def unit_mms(mc, it, tps, start, stop):
                wth, wtl = wot_tiles[mc]
                if (mc, it) in units:
                    O = units[(mc, it)]
                else:
                    O = ops.tile([P, 512], f32, tag="O")
                    units[(mc, it)] = O
                for n, tp in enumerate(tps):
                    lh = Aall_h[it][:, 2 * tp * P : (2 * tp + 2) * P].rearrange(
                        "a (two s) -> a two s", two=2
                    )
                    ll = Aall_l[it][:, 2 * tp * P : (2 * tp + 2) * P].rearrange(
                        "a (two s) -> a two s", two=2
                    )
                    rh = wth[:, 2 * tp : 2 * tp + 2, :]
                    rl = wtl[:, 2 * tp : 2 * tp + 2, :]
                    for k, (lhsT, rhs) in enumerate(((lh, rh), (lh, rl), (ll, rh))):
                        nc.tensor.matmul(
                            O,
                            lhsT,
                            rhs,
                            start=(start and n == 0 and k == 0),
                            stop=(stop and n == len(tps) - 1 and k == 2),
                            perf_mode=DR,
                        )

            def unit_fin(mc, it):
                O = units.pop((mc, it))
                Ot = osb.tile([P, 512], P_DT, tag="Ot")
                nc.scalar.activation(
                    out=Ot, in_=O, func=AF.Copy, scale=float(1.0 / WS3)
                )
                nc.sync.dma_start(
                    out=out[it * P : (it + 1) * P, mc * 512 : (mc + 1) * 512],
                    in_=Ot,
                )

            def unit(mc, it):
                unit_mms(mc, it, range(ST), True, True)
                unit_fin(mc, it)

            wot_tiles[0] = (w0h, w0l)  # prefetched during the interleave
            alloc_wot(1)
            alloc_wot(2)
            # qs=3 attention blocks (need all 16 s-tiles), pipelined; wo
            # chunk loads drip between rows so they never block the
            # latency-critical P transposes on the DMA engines. Phase-3
            # units split: t0-7 accumulation only needs qs<=1 heads (final
            # long before the tail), t8-15 needs the qs=3 heads.
            HALF1, HALF2 = range(0, 8), range(8, 16)
            PTt, recips = open_A(12)
            for qi in range(4):
                rowA(12, qi, PTt, recips)
                drip(2)
            stage_B(11)
            unit_mms(0, 0, HALF1, True, False)
            for j in (13, 14, 15):
                PTt, recips = open_A(j)
                for qi in range(4):
                    rowA(j, qi, PTt, recips)
                    drip(2)
                stage_B(j - 1)
                if j == 13:
                    unit_mms(1, 0, HALF1, True, False)
                elif j == 14:
                    # Aall[0] complete after B(13)
                    unit_mms(0, 0, HALF2, False, True)
                    unit_fin(0, 0)
                    unit_mms(2, 0, HALF1, True, False)
                elif j == 15:
                    unit_mms(1, 0, HALF2, False, True)
                    unit_fin(1, 0)
            stage_B(15)
            drip(len(wqueue))
            unit_mms(2, 0, HALF2, False, True)
            unit_fin(2, 0)
            # preloaded chunks' it=1 work covers the in-flight loads of the
            # later chunks (slot for mc+2 frees as soon as mc's last unit
            # is emitted)
            def load_wot_pre(mc):
                # rotate the wopre slot (mc0's chunk is consumed by now)
                wth = wopre.tile([P, JT, 512], f8, tag="wph")
                wtl = wopre.tile([P, JT, 512], f8, tag="wpl")
                wot_tiles[mc] = (wth, wtl)
                for qq in range(4):
                    wqueue.append((mc, qq, 0))
                    wqueue.append((mc, qq, 1))
                drip(8)

            def unit_split_cols(mc, it):
                # last unit: two column-halves so the closing copy/store
                # overlaps the second half's matmuls
                wth, wtl = wot_tiles[mc]
                for half in range(2):
                    O = ops.tile([P, 512], f32, tag="O")
                    cl, ch = half * 256, (half + 1) * 256
                    for n, tp in enumerate(range(ST)):
                        lh = Aall_h[it][:, 2 * tp * P : (2 * tp + 2) * P].rearrange(
                            "a (two s) -> a two s", two=2
                        )
                        ll = Aall_l[it][:, 2 * tp * P : (2 * tp + 2) * P].rearrange(
                            "a (two s) -> a two s", two=2
                        )
                        rh = wth[:, 2 * tp : 2 * tp + 2, cl:ch]
                        rl = wtl[:, 2 * tp : 2 * tp + 2, cl:ch]
                        for k, (lhsT, rhs) in enumerate(
                            ((lh, rh), (lh, rl), (ll, rh))
                        ):
                            nc.tensor.matmul(
                                O[:, 0:256],
                                lhsT,
                                rhs,
                                start=(n == 0 and k == 0),
                                stop=(n == ST - 1 and k == 2),
                                perf_mode=DR,
                            )
                    Ot = osb.tile([P, 512], P_DT, tag="Ot")
                    nc.scalar.activation(
                        out=Ot[:, 0:256],
                        in_=O[:, 0:256],
                        func=AF.Copy,
                        scale=float(1.0 / WS3),
                    )
                    nc.sync.dma_start(
                        out=out[
                            it * P : (it + 1) * P,
                            mc * 512 + cl : mc * 512 + ch,
                        ],
                        in_=Ot[:, 0:256],
                    )

            for mc in (1, 2, 0, 3, 4, 5, 6, 7):
                if mc >= 3:
                    unit(mc, 0)
                if mc == 7:
                    unit_split_cols(mc, 1)
                else:
                    unit(mc, 1)
                wot_tiles.pop(mc)
                nxt = mc + 2 if mc >= 3 else {1: 3, 2: 4, 0: 5}.get(mc)
                if nxt is not None and nxt < MC and nxt not in wot_tiles:
                    load_wot(nxt)

    nc.compile()
    return nc


def is_pure_causal(mask, SEQ):
    """True iff mask[i,j] == 0 for j<=i and <= NEG_THRESH for j>i."""
    m = np.asarray(mask, np.float32)
    if m.shape != (SEQ, SEQ):
        return False
    j = np.arange(SEQ)
    allowed = j[None, :] <= j[:, None]
    return bool((m[allowed] == 0).all() and (m[~allowed] <= NEG_THRESH).all())


def make_rope_tables(cos_freq, sin_freq, SEQ, scale_quarter):
    cos_t = np.tile(np.asarray(cos_freq, np.float32) * scale_quarter, (1, NH))
    sin_t = np.tile(np.asarray(sin_freq, np.float32) * scale_quarter, (1, NH))
    return np.ascontiguousarray(
        np.concatenate([cos_t, sin_t], axis=1).astype(np.float32)
    )




_BUILD_CACHE = {}


def kernel(
    x,
    cos_freq,
    sin_freq,
    positions,
    mask,
    wq,
    wk,
    wv,
    wo,
    _trace=False,
):
    import sys

    if "/opt/trn_rl_repo" not in sys.path:
        sys.path.insert(0, "/opt/trn_rl_repo")
    from concourse.bass_utils import run_bass_kernel_spmd
    import ml_dtypes

    x = np.asarray(x, np.float32)
    mask = np.asarray(mask, np.float32)
    wq = np.asarray(wq, np.float32)
    wk = np.asarray(wk, np.float32)
    wv = np.asarray(wv, np.float32)
    wo = np.asarray(wo, np.float32)
    SEQ, DIM = x.shape
    assert wq.shape[0] == CORES * NH * D and wk.shape[0] == CORES * D
    assert 2 * SEQ == wq.shape[0], "flatten structure requires H*D == 2*SEQ"

    bf16 = ml_dtypes.bfloat16
    f8 = ml_dtypes.float8_e4m3
    ST_, DD_ = SEQ // P, DIM // P

    if is_pure_causal(mask, SEQ):
        key = (SEQ, DIM, "causal")
        if key not in _BUILD_CACHE:
            _BUILD_CACHE[key] = build_attention_v7(SEQ, DIM)
        nc = _BUILD_CACHE[key]

        def hilo(a):
            hi = np.ascontiguousarray(a).astype(f8)
            lo = np.ascontiguousarray(a - hi.astype(np.float32)).astype(f8)
            return hi, lo

        # fold sqrt(scale) and the 1/WS1 weight pre-scale into rope tables
        scale_quarter = np.float32(D ** -0.25 / WS1)
        cs = make_rope_tables(cos_freq, sin_freq, SEQ, scale_quarter).astype(bf16)
        xT = np.ascontiguousarray(x.reshape(ST_, P, DD_, P).transpose(3, 0, 2, 1))
        xTh, xTl = hilo(xT)
        # wo row-blocks permuted so DoubleRow contraction pairs are adjacent
        JT_ = 2 * SEQ // P
        perm = [(jt % 2) * (JT_ // 2) + jt // 2 for jt in range(JT_)]
        woP = np.ascontiguousarray(
            (wo.T * np.float32(WS3)).reshape(JT_, P, DIM)[perm].reshape(2 * SEQ, DIM)
        )
        woh, wol = hilo(woP)
        tri = np.ascontiguousarray(mask[0:P, 0:P])

        in_maps = []
        for c in range(CORES):
            w_c = np.concatenate(
                [
                    wq[c * NH * D : (c + 1) * NH * D],
                    wk[c * D : (c + 1) * D],
                    wv[c * D : (c + 1) * D],
                ],
                axis=0,
            )
            wTh_, wTl_ = hilo(w_c.T * np.float32(WS1))
            in_maps.append(
                {
                    "xTh": xTh,
                    "xTl": xTl,
                    "wTh": wTh_,
                    "wTl": wTl_,
                    "cs": cs,
                    "tri": tri,
                    "woh": woh,
                    "wol": wol,
                }
            )
        res = run_bass_kernel_spmd(nc, in_maps, list(range(CORES)), trace=_trace)
        outp = np.concatenate(
            [np.asarray(res.results[c]["out"]) for c in range(CORES)], axis=0
        ).astype(np.float32)
        if _trace:
            return outp, res
        return outp

    # ---------------- general-mask fallback (v1 kernel) ----------------
    plan, blocks = analyze_mask(mask, SEQ)
    n_uniq = len(blocks)
    key = (SEQ, DIM, tuple(tuple(r) for r in plan))
    if key not in _BUILD_CACHE:
        _BUILD_CACHE[key] = build_attention_nc(SEQ, DIM, plan, n_uniq)
    nc = _BUILD_CACHE[key]

    scale_quarter = np.float32(D ** -0.25)
    csf = make_rope_tables(cos_freq, sin_freq, SEQ, scale_quarter)
    xT = np.ascontiguousarray(
        x.reshape(ST_, P, DD_, P).transpose(3, 0, 2, 1)
    ).astype(bf16)
    woT = np.ascontiguousarray(wo.T).astype(bf16)
    if n_uniq:
        mbs = np.ascontiguousarray(np.stack(blocks, axis=0))
    else:
        mbs = np.zeros((1, P, 512), np.float32)

    in_maps = []
    for c in range(CORES):
        w_c = np.concatenate(
            [
                wq[c * NH * D : (c + 1) * NH * D],
                wk[c * D : (c + 1) * D],
                wv[c * D : (c + 1) * D],
            ],
            axis=0,
        )
        in_maps.append(
            {
                "xT": xT,
                "wT": np.ascontiguousarray(w_c.T).astype(bf16),
                "cs": csf,
                "maskb": mbs,
                "woT": woT,
            }
        )
    res = run_bass_kernel_spmd(nc, in_maps, list(range(CORES)), trace=_trace)
    outp = np.concatenate(
        [np.asarray(res.results[c]["out"]) for c in range(CORES)], axis=0
    ).astype(np.float32)
    if _trace:
        return outp, res
    return outp


# revision 8
# speedup vs baseline: 1.0153x; 1.0153x over previous
"""Trainium2 Bass kernel for nn_Attention (GQA + RoPE + sliding-window mask).

Sharding: tensor-parallel over heads across 8 cores (4 q heads + 1 kv head
per core). The reference's quirky output flatten ((H,S,D)->(H,D,S)->
reshape(S, H*D)) makes the final projection row-shard by head block: core c
produces rows [256c, 256c+256) of the (2048, 4096) result with no collective.

Fast path (pure causal mask, the shape this problem produces):
  * phase 1 (QKV projections) and phase 3 (output projection) run as
    fp8-e4m3 hi/lo pairs in DoubleRow perf mode: X @ W ~= Xh@Wh + Xh@Wl +
    Xl@Wh with Xh = fp8(X), Xl = fp8(X - Xh) - 0.75x the PE time of one
    bf16 pass and more accurate than bf16 (~9-10 effective mantissa bits).
  * attention (phase 2) is interleaved INTO phase 1: block (qs, h) only
    needs s-tiles <= 4qs+3, so blocks weave between projection s-tiles with
    score rows emitted between x-chunk groups - softmax (ACT) latency hides
    under the projection matmuls.
  * no-max softmax (causal logits here are O(10), exp is safe in fp32),
    exp reads score PSUM directly with accum_out row sums; only the
    128-wide triangular diagonal block gets a mask add; diagonal PV
    matmuls are narrowed to the live query columns.
  * P transposed via DMA xbar; wo chunks prefetched/dripped so the big
    loads never head-of-line block the latency-critical transposes.

Fallback (any other mask): the v1 kernel (per-chunk mask add + 2-pass
max/exp softmax), correct for arbitrary additive masks.
"""

import numpy as np
from contextlib import ExitStack

P = 128
D = 128  # head dim
NH = 4   # q heads per core
CORES = 8
NEG_THRESH = -1e8


def build_attention_nc(
    SEQ,
    DIM,
    plan,
    n_uniq,
    p_dt_name="bfloat16",
    wo_dt_name="bfloat16",
    proj_dt_name="bfloat16",
    proj_f32r=True,
    score_f32r=True,
    use_dma_t=True,
):
    """Build the per-core Bass program.

    plan: list over q-tiles i (SEQ//128 entries) of lists of (chunk_idx, uid)
          where uid == -1 means the 512-wide chunk needs no mask add, else the
          index into the maskb tensor. Chunks absent from the list are fully
          masked (skipped).
    """
    import concourse.bass as bass
    import concourse.bacc as bacc
    import concourse.mybir as mybir
    import concourse.tile as tile
    from concourse.masks import make_identity

    f32 = mybir.dt.float32
    f32r = mybir.dt.float32r
    P_DT = getattr(mybir.dt, p_dt_name)
    WO_DT = getattr(mybir.dt, wo_dt_name)
    PJ_DT = getattr(mybir.dt, proj_dt_name)
    pj_f32r = proj_f32r and proj_dt_name == "float32"

    ST = SEQ // P          # 16 s-tiles
    DD = DIM // P          # 32 contraction tiles
    KC = SEQ // 512        # 4 key chunks
    QS = SEQ // 512        # 4 query supers
    EW = NH * D            # 512 q-projection width
    JT = 2 * SEQ // P      # 32 j-tiles for final matmul
    MC = DIM // 512        # 8 output chunks
    ITILES = (NH * 64) // P  # 2 output row tiles
    assert NH == 4 and SEQ % 512 == 0 and DIM % 512 == 0

    def mm_cast(ap, use_r):
        return ap.bitcast(f32r) if use_r else ap

    nc = bacc.Bacc(trn_type="TRN2", debug=False, num_devices=CORES)

    # x pre-tiled on host: xT[p, st, t, si] = x[st*128+si, t*128+p] so each
    # streamed chunk is one DMA with 2KB contiguous per-partition runs
    xT = nc.dram_tensor("xT", [P, ST, DD, P], PJ_DT, kind="ExternalInput").ap()
    wT = nc.dram_tensor("wT", [DIM, EW + 2 * D], PJ_DT, kind="ExternalInput").ap()
    cs = nc.dram_tensor("cs", [SEQ, EW], f32, kind="ExternalInput").ap()
    mb = nc.dram_tensor(
        "maskb", [max(n_uniq, 1), P, 512], f32, kind="ExternalInput"
    ).ap()
    woT = nc.dram_tensor("woT", [2 * SEQ, DIM], WO_DT, kind="ExternalInput").ap()
    out = nc.dram_tensor("out", [NH * 64, DIM], f32, kind="ExternalOutput").ap()

    with tile.TileContext(nc) as tc, ExitStack() as ctx:
        const = ctx.enter_context(tc.tile_pool(name="const", bufs=1))
        idF = const.tile([P, P], f32)
        make_identity(nc, idF)
        idP = const.tile([P, P], P_DT)
        make_identity(nc, idP)
        zeros = const.tile([P, 512], f32)
        nc.vector.memset(zeros, 0.0)

        pers = ctx.enter_context(tc.tile_pool(name="pers", bufs=1))
        QTt = pers.tile([P, NH, ST * P], f32)   # [d, h, s]
        KTt = pers.tile([P, ST * P], f32)       # [d, s]
        Vt = pers.tile([P, ST, D], P_DT)        # [k(part), ktile, d]
        if n_uniq > 0:
            mbt = pers.tile([P, n_uniq, 512], f32)

        # ---------------- phase 1: projections + rope + layout ----------------
        with (
            tc.tile_pool(name="wpool", bufs=1) as wpool,
            tc.tile_pool(name="xpool", bufs=6) as xpool,
            tc.tile_pool(name="cspool", bufs=2) as cspool,
            tc.tile_pool(name="rpool", bufs=2) as rpool,
            tc.tile_pool(name="qps", bufs=2, space="PSUM") as qps,
            tc.tile_pool(name="kvps", bufs=2, space="PSUM") as kvps,
            tc.tile_pool(name="tps", bufs=2, space="PSUM") as tps,
            tc.tile_pool(name="t2ps", bufs=2, space="PSUM") as t2ps,
        ):
            XGW = min(8, DD)
            wTt = wpool.tile([P, DD, EW + 2 * D], PJ_DT)
            wTr = wT.rearrange("(t p) e -> p t e", p=P)

            XG = min(8, DD)  # dd-tiles per streamed x chunk
            NG = DD // XG
            xTr = xT
            # Interleave the weight-chunk loads with s-tile 0's x chunks so
            # the first matmuls start as soon as chunk 0 of each lands.
            st0_x = []
            for g in range(NG):
                xTt = xpool.tile([P, XG, P], PJ_DT, tag="xT")
                nc.sync.dma_start(
                    out=xTt, in_=xTr[:, 0, g * XG : (g + 1) * XG, :]
                )
                st0_x.append(xTt)
                gw = g % (DD // XGW)
                nc.sync.dma_start(
                    out=wTt[:, gw * XGW : (gw + 1) * XGW, :],
                    in_=wTr[:, gw * XGW : (gw + 1) * XGW, :],
                )
            for st in range(ST):
                cst = cspool.tile([P, EW], f32, tag="cs")
                nc.sync.dma_start(out=cst, in_=cs[st * P : (st + 1) * P, :])

                Qp = qps.tile([P, EW], f32, tag="Qp")
                KVp = kvps.tile([P, 2 * D], f32, tag="KVp")
                for g in range(DD // XG):
                    if st == 0:
                        xTt = st0_x[g]
                    else:
                        xTt = xpool.tile([P, XG, P], PJ_DT, tag="xT")
                        nc.sync.dma_start(
                            out=xTt,
                            in_=xTr[:, st, g * XG : (g + 1) * XG, :],
                        )
                    for tt in range(XG):
                        t = g * XG + tt
                        lhsT = mm_cast(xTt[:, tt, :], pj_f32r)
                        nc.tensor.matmul(
                            Qp,
                            lhsT,
                            mm_cast(wTt[:, t, 0:EW], pj_f32r),
                            start=(t == 0),
                            stop=(t == DD - 1),
                        )
                        nc.tensor.matmul(
                            KVp,
                            lhsT,
                            mm_cast(wTt[:, t, EW : EW + 2 * D], pj_f32r),
                            start=(t == 0),
                            stop=(t == DD - 1),
                        )

                # rope via strided even/odd halves (2-level APs only — 3-level
                # APs overflow the fixed ISA instruction encoding).
                # tensor_tensor_reduce instead of tensor_tensor: the plain TT
                # ISA struct has a single sync-wait slot and walrus codegen
                # rejects the PE+DMA double wait Tile emits here; the TTR/ISA
                # struct carries up to 8. accum outputs are dummies.
                def ttr_ew(out, in0, in1, op):
                    nc.vector.tensor_tensor(out=out, in0=in0, in1=in1, op=op)

                A_ = mybir.AluOpType
                HF = EW // 2  # 256: cos table width for q
                rq = rpool.tile([P, EW], f32, tag="rq")
                t1 = rpool.tile([P, HF], f32, tag="t1")
                t2 = rpool.tile([P, HF], f32, tag="t2")
                q_ev, q_od = Qp[:, 0:EW:2], Qp[:, 1:EW:2]
                cosr, sinr = cst[:, 0:HF], cst[:, HF : 2 * HF]
                ttr_ew(t1, q_ev, cosr, A_.mult)
                ttr_ew(t2, q_od, sinr, A_.mult)
                ttr_ew(rq[:, 0:EW:2], t1, t2, A_.subtract)
                ttr_ew(t1, q_ev, sinr, A_.mult)
                ttr_ew(t2, q_od, cosr, A_.mult)
                ttr_ew(rq[:, 1:EW:2], t1, t2, A_.add)

                rk = rpool.tile([P, D], f32, tag="rk")
                k_ev, k_od = KVp[:, 0:D:2], KVp[:, 1:D:2]
                cosk, sink = cst[:, 0 : D // 2], cst[:, HF : HF + D // 2]
                ttr_ew(t1[:, 0 : D // 2], k_ev, cosk, A_.mult)
                ttr_ew(t2[:, 0 : D // 2], k_od, sink, A_.mult)
                ttr_ew(rk[:, 0:D:2], t1[:, 0 : D // 2], t2[:, 0 : D // 2], A_.subtract)
                ttr_ew(t1[:, 0 : D // 2], k_ev, sink, A_.mult)
                ttr_ew(t2[:, 0 : D // 2], k_od, cosk, A_.mult)
                ttr_ew(rk[:, 1:D:2], t1[:, 0 : D // 2], t2[:, 0 : D // 2], A_.add)

                # V -> bf16 [k, d] layout (ACT copy, cast)
                nc.scalar.activation(
                    out=Vt[:, st, :],
                    in_=KVp[:, D : 2 * D],
                    func=mybir.ActivationFunctionType.Copy,
                )

                # transpose rq (per head) and rk into [d, s] layouts
                T1 = tps.tile([P, EW], f32, tag="T1")
                for h in range(NH):
                    nc.tensor.transpose(
                        T1[:, h * P : (h + 1) * P], rq[:, h * P : (h + 1) * P], idF
                    )
                # write as f32r so walrus accepts them as f32r matmul operands
                nc.vector.tensor_copy(
                    out=mm_cast(QTt[:, :, st * P : (st + 1) * P], score_f32r),
                    in_=T1.rearrange("p (h s) -> p h s", h=NH),
                )
                T2 = t2ps.tile([P, P], f32, tag="T2")
                nc.tensor.transpose(T2, rk, idF)
                nc.vector.tensor_copy(
                    out=mm_cast(KTt[:, st * P : (st + 1) * P], score_f32r), in_=T2
                )

        # ---------------- phase 2: attention ----------------
        if n_uniq > 0:
            nc.sync.dma_start(out=mbt, in_=mb.rearrange("u p m -> p u m"))
        apool = ctx.enter_context(tc.tile_pool(name="apool", bufs=1))
        # split by head-pair so phase 3's first row-tile can start once
        # heads 0-1 finish, overlapping the rest of phase 2
        Aall = [
            apool.tile([P, 2 * ST * D], P_DT, name=f"Aall{i}")
            for i in range(NH // 2)
        ]
        with (
            tc.tile_pool(name="ptsb", bufs=2) as ptsb,
            tc.tile_pool(name="spool", bufs=6) as spool,
            tc.tile_pool(name="ppool", bufs=4) as ppool,
            tc.tile_pool(name="stat", bufs=12) as stat,
            tc.tile_pool(name="atsb", bufs=3) as atsb,
            tc.tile_pool(name="sps", bufs=2, space="PSUM") as sps,
            tc.tile_pool(name="ptps", bufs=2, space="PSUM") as ptps,
            tc.tile_pool(name="atps", bufs=1, space="PSUM") as atps,
            tc.tile_pool(name="aps", bufs=1, space="PSUM") as aps,
            tc.tile_pool(name="wopool", bufs=2 if n_uniq <= 4 else 1) as wopool,
            tc.tile_pool(name="osb", bufs=2) as osb,
            tc.tile_pool(name="ops", bufs=3, space="PSUM") as ops,
        ):
            for h in range(NH):
                for qs in range(QS):
                    PTt = ptsb.tile([P, ST, 512], P_DT, tag="PT")
                    kts_used = set()
                    recips = []
                    pt_written = set()
                    for qi in range(4):
                        i = 4 * qs + qi
                        row = plan[i]
                        if not row:
                            recips.append(None)
                            continue
                        pairs = [row[k : k + 2] for k in range(0, len(row), 2)]
                        stats = stat.tile([P, KC], f32, tag="stats")
                        ncols = 0
                        S_tiles = []
                        for pr in pairs:
                            W = 512 * len(pr)
                            S = sps.tile([P, 1024], f32, tag="S")
                            Ssb = spool.tile([P, 1024], f32, tag="Ssb")
                            masked_any = any(uid >= 0 for (_, uid) in pr)
                            for k, (c, uid) in enumerate(pr):
                                sl = S[:, k * 512 : (k + 1) * 512]
                                nc.tensor.matmul(
                                    sl,
                                    mm_cast(
                                        QTt[:, h, i * P : (i + 1) * P], score_f32r
                                    ),
                                    mm_cast(
                                        KTt[:, c * 512 : (c + 1) * 512], score_f32r
                                    ),
                                    start=True,
                                    stop=True,
                                )
                                if uid >= 0:
                                    nc.vector.tensor_add(sl, sl, mbt[:, uid, :])
                                # copy PSUM->SBUF to free the score bank early;
                                # alternate DVE/ACT to balance engine load
                                dst = Ssb[:, k * 512 : (k + 1) * 512]
                                if (i + k) % 2 == 0:
                                    nc.vector.tensor_copy(out=dst, in_=sl)
                                else:
                                    nc.scalar.activation(
                                        out=dst,
                                        in_=sl,
                                        func=mybir.ActivationFunctionType.Copy,
                                    )
                                if masked_any or len(pr) == 1:
                                    nc.vector.tensor_reduce(
                                        out=stats[:, ncols : ncols + 1],
                                        in_=dst,
                                        axis=mybir.AxisListType.X,
                                        op=mybir.AluOpType.max,
                                    )
                                    ncols += 1
                            if not masked_any and len(pr) == 2:
                                # one pair-wide max over both chunks (SBUF 2x)
                                nc.vector.tensor_reduce(
                                    out=stats[:, ncols : ncols + 1],
                                    in_=Ssb,
                                    axis=mybir.AxisListType.X,
                                    op=mybir.AluOpType.max,
                                )
                                ncols += 1
                            S_tiles.append((Ssb, pr))
                        negm = stat.tile([P, 1], f32, tag="negm")
                        nc.vector.tensor_reduce(
                            out=negm,
                            in_=stats[:, 0:ncols],
                            axis=mybir.AxisListType.X,
                            op=mybir.AluOpType.max,
                            negate=True,
                        )
                        sums = stat.tile([P, KC], f32, tag="sums")
                        for k, (Sk, pr) in enumerate(S_tiles):
                            W = 512 * len(pr)
                            Pt = ppool.tile([P, 1024], P_DT, tag="P")
                            nc.scalar.activation(
                                out=Pt[:, 0:W],
                                in_=Sk[:, 0:W],
                                func=mybir.ActivationFunctionType.Exp,
                                bias=negm,
                                accum_out=sums[:, k : k + 1],
                            )
                            # transpose P [q, k] -> PT [k, q]
                            for j, (c, uid) in enumerate(pr):
                                if use_dma_t:
                                    nc.sync.dma_start_transpose(
                                        out=PTt[
                                            :, 4 * c : 4 * c + 4, qi * P : (qi + 1) * P
                                        ],
                                        in_=Pt[:, j * 512 : (j + 1) * 512],
                                    )
                                else:
                                    PTp = ptps.tile([P, 512], P_DT, tag="PTp")
                                    for jj in range(4):
                                        nc.tensor.transpose(
                                            PTp[:, jj * P : (jj + 1) * P],
                                            Pt[:, j * 512 + jj * P : j * 512 + (jj + 1) * P],
                                            idP,
                                        )
                                    nc.vector.tensor_copy(
                                        out=PTt[:, 4 * c : 4 * c + 4, qi * P : (qi + 1) * P],
                                        in_=PTp.rearrange("p (kt q) -> p kt q", kt=4),
                                    )
                                for jj in range(4):
                                    kts_used.add(4 * c + jj)
                                    pt_written.add((4 * c + jj, qi))
                        denom = stat.tile([P, 1], f32, tag="denom")
                        nc.vector.tensor_reduce(
                            out=denom,
                            in_=sums[:, 0 : len(S_tiles)],
                            axis=mybir.AxisListType.X,
                            op=mybir.AluOpType.add,
                        )
                        recip = stat.tile([P, 1], f32, tag="recip")
                        nc.vector.reciprocal(recip, denom)
                        recips.append(recip)

                    # zero-fill PT holes (only for non-causal masks)
                    kts = sorted(kts_used)
                    for kt in kts:
                        for qi in range(4):
                            if (kt, qi) not in pt_written and recips[qi] is not None:
                                nc.vector.memset(
                                    PTt[:, kt, qi * P : (qi + 1) * P], 0.0
                                )
                            elif recips[qi] is None:
                                nc.vector.memset(
                                    PTt[:, kt, qi * P : (qi + 1) * P], 0.0
                                )

                    if not kts:
                        continue
                    # PV: A^T[d, q] accumulated over key tiles
                    At = atps.tile([P, 512], f32, tag="At")
                    for n, kt in enumerate(kts):
                        nc.tensor.matmul(
                            At,
                            Vt[:, kt, :],
                            PTt[:, kt, :],
                            start=(n == 0),
                            stop=(n == len(kts) - 1),
                        )
                    Atsb = atsb.tile([P, 512], P_DT, tag="Atsb")
                    nc.vector.tensor_copy(out=Atsb, in_=At)
                    Ap = aps.tile([P, 512], P_DT, tag="Ap")
                    for qi in range(4):
                        nc.tensor.transpose(
                            Ap[:, qi * P : (qi + 1) * P],
                            Atsb[:, qi * P : (qi + 1) * P],
                            idP,
                        )
                    # Aall layout: [sp, (t*2 + dd)*128 + hb*64 + p] so the final
                    # matmul's stationary slices are contiguous (walrus requires
                    # a single free dim on weight APs)
                    Ah = Aall[h // 2]
                    hb = h % 2
                    for qi in range(4):
                        i = 4 * qs + qi
                        # dview[sp, p, dd] == Ah[:, i*256 + dd*128 + hb*64 + p]
                        dview = Ah[:, i * 2 * P : (i + 1) * 2 * P].rearrange(
                            "a (dd j) -> a dd j", dd=2
                        )[:, :, hb * 64 : hb * 64 + 64].rearrange(
                            "a dd p -> a p dd"
                        )
                        if recips[qi] is None:
                            nc.vector.memset(dview, 0.0)
                            continue
                        nc.scalar.activation(
                            out=dview,
                            in_=Ap[:, qi * P : (qi + 1) * P].rearrange(
                                "a (p two) -> a p two", two=2
                            ),
                            func=mybir.ActivationFunctionType.Copy,
                            scale=recips[qi],
                        )

            # ---------------- phase 3: output projection ----------------
            for mc in range(MC):
                wot = wopool.tile([P, JT, 512], WO_DT, tag="wo")
                nc.sync.dma_start(
                    out=wot,
                    in_=woT[:, mc * 512 : (mc + 1) * 512].rearrange(
                        "(t p) m -> p t m", p=P
                    ),
                )
                for it in range(ITILES):
                    O = ops.tile([P, 512], f32, tag="O")
                    Av = Aall[it]
                    for jt in range(JT):
                        ddj, t = jt // ST, jt % ST
                        lhsT = Av[:, (t * 2 + ddj) * P : (t * 2 + ddj + 1) * P]
                        nc.tensor.matmul(
                            O,
                            lhsT,
                            wot[:, jt, :],
                            start=(jt == 0),
                            stop=(jt == JT - 1),
                        )
                    Ot = osb.tile([P, 512], f32, tag="Ot")
                    nc.scalar.activation(
                        out=Ot, in_=O, func=mybir.ActivationFunctionType.Copy
                    )
                    nc.sync.dma_start(
                        out=out[it * P : (it + 1) * P, mc * 512 : (mc + 1) * 512],
                        in_=Ot,
                    )

    # Bacc.compile() legalizes sync (>=2 waits split into EventSemaphore
    # instructions — this walrus caps every instruction at ONE sync wait)
    nc.compile()
    return nc


def analyze_mask(mask, SEQ):
    """Classify 128x512 mask blocks: skip / free / masked(dedup uid)."""
    ST = SEQ // P
    KC = SEQ // 512
    uniq = {}
    blocks = []
    plan = []
    for i in range(ST):
        row = []
        for c in range(KC):
            blk = mask[i * P : (i + 1) * P, c * 512 : (c + 1) * 512]
            if (blk <= NEG_THRESH).all():
                continue
            if not blk.any():
                row.append((c, -1))
            else:
                key = blk.tobytes()
                if key not in uniq:
                    uniq[key] = len(blocks)
                    blocks.append(np.ascontiguousarray(blk))
                row.append((c, uniq[key]))
        if not row:
            # fully masked query rows: keep all chunks so softmax matches
            # the reference's uniform distribution over -1e9 logits
            for c in range(KC):
                blk = mask[i * P : (i + 1) * P, c * 512 : (c + 1) * 512]
                key = blk.tobytes()
                if key not in uniq:
                    uniq[key] = len(blocks)
                    blocks.append(np.ascontiguousarray(blk))
                row.append((c, uniq[key]))
        plan.append(row)
    return plan, blocks


WS1 = 1024.0  # host pre-scale on wq/wk/wv before fp8 (values ~0.02*N(0,1))
WS3 = 256.0   # host pre-scale on wo before fp8


def build_attention_v7(
    SEQ,
    DIM,
    p_dt_name="bfloat16",
):
    import concourse.bass as bass
    import concourse.bacc as bacc
    import concourse.mybir as mybir
    import concourse.tile as tile
    from concourse.masks import make_identity

    f32 = mybir.dt.float32
    f8 = mybir.dt.float8e4
    P_DT = getattr(mybir.dt, p_dt_name)
    A_ = mybir.AluOpType
    AF = mybir.ActivationFunctionType
    DR = mybir.MatmulPerfMode.DoubleRow

    ST = SEQ // P          # 16 s-tiles
    DD = DIM // P          # 32 contraction tiles
    QS = SEQ // 512        # 4 query supers
    EW = NH * D            # 512 q-projection width
    JT = 2 * SEQ // P      # 32 j-tiles for final matmul
    MC = DIM // 512        # 8 output chunks
    ITILES = (NH * 64) // P  # 2 output row tiles
    assert NH == 4 and SEQ % 512 == 0 and DIM % 512 == 0

    nc = bacc.Bacc(trn_type="TRN2", debug=False, num_devices=CORES)

    xTh = nc.dram_tensor("xTh", [P, ST, DD, P], f8, kind="ExternalInput").ap()
    xTl = nc.dram_tensor("xTl", [P, ST, DD, P], f8, kind="ExternalInput").ap()
    wTh = nc.dram_tensor("wTh", [DIM, EW + 2 * D], f8, kind="ExternalInput").ap()
    wTl = nc.dram_tensor("wTl", [DIM, EW + 2 * D], f8, kind="ExternalInput").ap()
    cs = nc.dram_tensor("cs", [SEQ, EW], P_DT, kind="ExternalInput").ap()
    tri = nc.dram_tensor("tri", [P, P], f32, kind="ExternalInput").ap()
    # wo row blocks permuted host-side: block jt' = 2t+dd <- original dd*16+t
    woh = nc.dram_tensor("woh", [2 * SEQ, DIM], f8, kind="ExternalInput").ap()
    wol = nc.dram_tensor("wol", [2 * SEQ, DIM], f8, kind="ExternalInput").ap()
    out = nc.dram_tensor("out", [NH * 64, DIM], P_DT, kind="ExternalOutput").ap()

    with tile.TileContext(nc) as tc, ExitStack() as ctx:
        const = ctx.enter_context(tc.tile_pool(name="const", bufs=1))
        idP = const.tile([P, P], P_DT)
        make_identity(nc, idP)

        pers = ctx.enter_context(tc.tile_pool(name="pers", bufs=1))
        QTt = pers.tile([P, NH, ST * P], P_DT)   # [d, h, s]
        KTt = pers.tile([P, ST * P], P_DT)       # [d, s]
        Vt = pers.tile([P, ST, D], P_DT)         # [k(part), ktile, d]
        trit = pers.tile([P, P], f32)
        nc.sync.dma_start(out=trit, in_=tri)

        # mc=0 wo chunk is prefetched during the interleaved region (the only
        # chunk SBUF has room for before the phase-1 pools close)
        wopre = ctx.enter_context(tc.tile_pool(name="wopre", bufs=1))
        w0h = wopre.tile([P, 2 * SEQ // P, 512], f8, tag="wph")
        w0l = wopre.tile([P, 2 * SEQ // P, 512], f8, tag="wpl")
        wohr_ = woh.rearrange("(t p) m -> p t m", p=P)
        wolr_ = wol.rearrange("(t p) m -> p t m", p=P)

        def drip_w0(qq):
            q4 = (2 * SEQ // P) // 4
            nc.sync.dma_start(
                out=w0h[:, qq * q4 : (qq + 1) * q4, :],
                in_=wohr_[:, qq * q4 : (qq + 1) * q4, 0:512],
            )
            nc.sync.dma_start(
                out=w0l[:, qq * q4 : (qq + 1) * q4, :],
                in_=wolr_[:, qq * q4 : (qq + 1) * q4, 0:512],
            )

        apool = ctx.enter_context(tc.tile_pool(name="apool", bufs=1))
        Aall_h = [
            apool.tile([P, 2 * ST * D], f8, name=f"Aallh{i}") for i in range(NH // 2)
        ]
        Aall_l = [
            apool.tile([P, 2 * ST * D], f8, name=f"Aalll{i}") for i in range(NH // 2)
        ]
        # attention pools live through phase 1+2 and the tail
        ptsb = ctx.enter_context(tc.tile_pool(name="ptsb", bufs=2))
        ppool = ctx.enter_context(tc.tile_pool(name="ppool", bufs=8))
        stat = ctx.enter_context(tc.tile_pool(name="stat", bufs=12))
        atsb = ctx.enter_context(tc.tile_pool(name="atsb", bufs=3))
        s1ps = ctx.enter_context(tc.tile_pool(name="s1ps", bufs=3, space="PSUM"))
        atps = ctx.enter_context(tc.tile_pool(name="atps", bufs=1, space="PSUM"))
        aps = ctx.enter_context(tc.tile_pool(name="aps", bufs=1, space="PSUM"))

        blocks = [(qs, h) for qs in range(QS) for h in range(NH)]  # j = 4qs+h
        state = {}

        def rowA(j, qi, PTt, recips):
            qs, h = blocks[j]
            i = 4 * qs + qi
            f = i // 4      # fully-allowed 512-chunks
            dsub = i % 4    # full 128-subtiles in the diagonal chunk
            sums = stat.tile([P, 4], f32, tag="sums")
            ncol = 0
            lhsQ = QTt[:, h, i * P : (i + 1) * P]
            for c in range(f):
                S1t = s1ps.tile([P, 512], f32, tag="S1")
                nc.tensor.matmul(
                    S1t,
                    lhsQ,
                    KTt[:, c * 512 : (c + 1) * 512],
                    start=True,
                    stop=True,
                )
                Pt = ppool.tile([P, 512], P_DT, tag="P1")
                nc.scalar.activation(
                    out=Pt,
                    in_=S1t,
                    func=AF.Exp,
                    accum_out=sums[:, ncol : ncol + 1],
                )
                ncol += 1
                nc.sync.dma_start_transpose(
                    out=PTt[:, 4 * c : 4 * c + 4, qi * P : (qi + 1) * P],
                    in_=Pt,
                )
            # diagonal chunk, truncated to (dsub+1)*128 columns
            w = (dsub + 1) * P
            S1t = s1ps.tile([P, 512], f32, tag="S1")
            nc.tensor.matmul(
                S1t[:, 0:w],
                lhsQ,
                KTt[:, f * 512 : f * 512 + w],
                start=True,
                stop=True,
            )
            nc.vector.tensor_add(S1t[:, dsub * P : w], S1t[:, dsub * P : w], trit)
            Pt = ppool.tile([P, 512], P_DT, tag="P1")
            nc.scalar.activation(
                out=Pt[:, 0:w],
                in_=S1t[:, 0:w],
                func=AF.Exp,
                accum_out=sums[:, ncol : ncol + 1],
            )
            ncol += 1
            nc.sync.dma_start_transpose(
                out=PTt[:, 4 * f : 4 * f + dsub + 1, qi * P : (qi + 1) * P],
                in_=Pt[:, 0:w],
            )
            # masked-out subtiles of the diagonal chunk are never read: the
            # PV matmuls for diagonal key-tiles are narrowed to the live
            # query columns instead
            denom = stat.tile([P, 1], f32, tag="denom")
            nc.vector.tensor_reduce(
                out=denom, in_=sums[:, 0:ncol], axis=mybir.AxisListType.X, op=A_.add
            )
            recip = stat.tile([P, 1], f32, tag="recip")
            nc.vector.reciprocal(recip, denom)
            recips.append(recip)

        def open_A(j):
            PTt = ptsb.tile([P, ST, 512], P_DT, tag="PT")
            recips = []
            state[j] = (PTt, recips)
            return PTt, recips

        def stage_B(j):
            qs, h = blocks[j]
            PTt, recips = state.pop(j)
            nkt = 4 * qs + 4
            At = atps.tile([P, 512], f32, tag="At")
            for n in range(nkt):
                # diagonal key-tiles only reach query columns >= off
                off = max(0, n - 4 * qs) * P
                nc.tensor.matmul(
                    At[:, off:512],
                    Vt[:, n, :],
                    PTt[:, n, off:512],
                    start=(n == 0),
                    stop=(n == nkt - 1),
                )
            Atsb = atsb.tile([P, 512], P_DT, tag="Atsb")
            nc.vector.tensor_copy(out=Atsb, in_=At)
            Ap = aps.tile([P, 512], P_DT, tag="Ap")
            for qi in range(4):
                nc.tensor.transpose(
                    Ap[:, qi * P : (qi + 1) * P],
                    Atsb[:, qi * P : (qi + 1) * P],
                    idP,
                )
            hb = h % 2

            def dv(Aarr):
                return Aarr[h // 2][:, i * 2 * P : (i + 1) * 2 * P].rearrange(
                    "a (dd j) -> a dd j", dd=2
                )[:, :, hb * 64 : hb * 64 + 64].rearrange("a dd p -> a p dd")

            for qi in range(4):
                i = 4 * qs + qi
                # normalize on DVE, then split into fp8 hi + residual lo for
                # the DoubleRow output projection
                th = atsb.tile([P, P], f32, tag="th")
                nc.vector.tensor_scalar_mul(
                    th, Ap[:, qi * P : (qi + 1) * P], recips[qi]
                )
                thv = th.rearrange("a (p two) -> a p two", two=2)
                dh, dl = dv(Aall_h), dv(Aall_l)
                nc.vector.tensor_copy(out=dh, in_=thv)
                nc.vector.tensor_tensor(out=dl, in0=thv, in1=dh, op=A_.subtract)

        # ------------- phase 1 with interleaved attention blocks -------------
        with (
            tc.tile_pool(name="wpool", bufs=1) as wpool,
            tc.tile_pool(name="xpool", bufs=8) as xpool,
            tc.tile_pool(name="cspool", bufs=2) as cspool,
            tc.tile_pool(name="rpool", bufs=2) as rpool,
            tc.tile_pool(name="qps", bufs=1, space="PSUM") as qps,
            tc.tile_pool(name="kvsh", bufs=1, space="PSUM") as kvsh,
        ):
            wTth = wpool.tile([P, DD, EW + 2 * D], f8)
            wTtl = wpool.tile([P, DD, EW + 2 * D], f8)
            wTrh = wTh.rearrange("(t p) e -> p t e", p=P)
            wTrl = wTl.rearrange("(t p) e -> p t e", p=P)

            XG = min(8, DD)  # dd-tiles per streamed x chunk
            NG = DD // XG
            # interleave s-tile-0 x chunks with weight loads (weights in 8
            # sub-loads per array so the first matmuls start early)
            st0_x = []
            for g in range(NG):
                xh = xpool.tile([P, XG, P], f8, tag="xh")
                nc.sync.dma_start(out=xh, in_=xTh[:, 0, g * XG : (g + 1) * XG, :])
                xl = xpool.tile([P, XG, P], f8, tag="xl")
                nc.sync.dma_start(out=xl, in_=xTl[:, 0, g * XG : (g + 1) * XG, :])
                st0_x.append((xh, xl))
                for half in range(2):
                    gw = 2 * g + half
                    nc.sync.dma_start(
                        out=wTth[:, gw * 4 : (gw + 1) * 4, :],
                        in_=wTrh[:, gw * 4 : (gw + 1) * 4, :],
                    )
                    nc.sync.dma_start(
                        out=wTtl[:, gw * 4 : (gw + 1) * 4, :],
                        in_=wTrl[:, gw * 4 : (gw + 1) * 4, :],
                    )
            def qkv_terms(Qp, KVp, xh, xl, g, first_flag=True):
                for tp in range(XG // 2):
                    t = g * XG + 2 * tp
                    first = t == 0 and first_flag
                    last = t == DD - 2
                    lh = xh[:, 2 * tp : 2 * tp + 2, :]
                    ll = xl[:, 2 * tp : 2 * tp + 2, :]
                    terms = ((lh, wTth), (lh, wTtl), (ll, wTth))
                    for k, (lhsT, wt) in enumerate(terms):
                        nc.tensor.matmul(
                            Qp,
                            lhsT,
                            wt[:, t : t + 2, 0:EW],
                            start=(first and k == 0),
                            stop=(last and k == 2),
                            perf_mode=DR,
                        )
                    for k, (lhsT, wt) in enumerate(terms):
                        nc.tensor.matmul(
                            KVp,
                            lhsT,
                            wt[:, t : t + 2, EW : EW + 2 * D],
                            start=(first and k == 0),
                            stop=(last and k == 2),
                            perf_mode=DR,
                        )

            # s-tiles 0 and 1 are paired: the weight stream is the DMA
            # bottleneck at kernel start, so each weight granule feeds two
            # s-tiles' matmuls (s1 borrows PSUM from the still-idle
            # attention pools)
            cst0 = cspool.tile([P, EW], P_DT, tag="cs")
            nc.sync.dma_start(out=cst0, in_=cs[0:P, :])
            cst1 = cspool.tile([P, EW], P_DT, tag="cs")
            nc.sync.dma_start(out=cst1, in_=cs[P : 2 * P, :])
            Qp0 = qps.tile([P, EW], f32, tag="Qp")
            KVp0 = kvsh.tile([P, 2 * D], f32, tag="KVp")
            Qp1 = s1ps.tile([P, 512], f32, tag="S1")
            KVt1 = atps.tile([P, 512], f32, tag="At")
            KVp1 = KVt1[:, 0 : 2 * D]
            st1_x = []
            for g in range(NG):
                xh1 = xpool.tile([P, XG, P], f8, tag="xh")
                nc.sync.dma_start(out=xh1, in_=xTh[:, 1, g * XG : (g + 1) * XG, :])
                xl1 = xpool.tile([P, XG, P], f8, tag="xl")
                nc.sync.dma_start(out=xl1, in_=xTl[:, 1, g * XG : (g + 1) * XG, :])
                st1_x.append((xh1, xl1))
                xh0, xl0 = st0_x[g]
                qkv_terms(Qp0, KVp0, xh0, xl0, g)
                qkv_terms(Qp1, KVp1, xh1, xl1, g)

            for st in range(ST):
                j = st - 4  # attention block woven into this s-tile
                if j >= 0:
                    PTt, recips = open_A(j)
                if st == 0:
                    Qp, KVp, cst = Qp0, KVp0, cst0
                elif st == 1:
                    Qp, KVp, cst = Qp1, KVp1, cst1
                else:
                    cst = cspool.tile([P, EW], P_DT, tag="cs")
                    nc.sync.dma_start(out=cst, in_=cs[st * P : (st + 1) * P, :])
                    Qp = qps.tile([P, EW], f32, tag="Qp")
                    KVp = kvsh.tile([P, 2 * D], f32, tag="KVp")
                for g in range(DD // XG if st >= 2 else 0):
                    if st == 0:
                        xh, xl = st0_x[g]
                    else:
                        xh = xpool.tile([P, XG, P], f8, tag="xh")
                        nc.sync.dma_start(
                            out=xh, in_=xTh[:, st, g * XG : (g + 1) * XG, :]
                        )
                        xl = xpool.tile([P, XG, P], f8, tag="xl")
                        nc.sync.dma_start(
                            out=xl, in_=xTl[:, st, g * XG : (g + 1) * XG, :]
                        )
                    for tp in range(XG // 2):
                        t = g * XG + 2 * tp
                        first = t == 0
                        last = t == DD - 2
                        lh = xh[:, 2 * tp : 2 * tp + 2, :]
                        ll = xl[:, 2 * tp : 2 * tp + 2, :]
                        terms = (
                            (lh, wTth),
                            (lh, wTtl),
                            (ll, wTth),
                        )
                        for k, (lhsT, wt) in enumerate(terms):
                            nc.tensor.matmul(
                                Qp,
                                lhsT,
                                wt[:, t : t + 2, 0:EW],
                                start=(first and k == 0),
                                stop=(last and k == 2),
                                perf_mode=DR,
                            )
                        for k, (lhsT, wt) in enumerate(terms):
                            nc.tensor.matmul(
                                KVp,
                                lhsT,
                                wt[:, t : t + 2, EW : EW + 2 * D],
                                start=(first and k == 0),
                                stop=(last and k == 2),
                                perf_mode=DR,
                            )
                    # one attention row between x-chunk groups keeps ACT fed
                    # while PE grinds the projection matmuls
                    if j >= 0:
                        rowA(j, g, PTt, recips)

                # free the Q/KV PSUM banks fast: one copy each, rope reads SBUF
                qsb = rpool.tile([P, EW], f32, tag="qsb")
                nc.vector.tensor_copy(out=qsb, in_=Qp)
                kvsb = rpool.tile([P, 2 * D], f32, tag="kvsb")
                nc.vector.tensor_copy(out=kvsb, in_=KVp)

                def ttr_ew(out_, in0, in1, op):
                    nc.vector.tensor_tensor(out=out_, in0=in0, in1=in1, op=op)

                HF = EW // 2
                rq = rpool.tile([P, EW], P_DT, tag="rq")
                t1 = rpool.tile([P, HF], f32, tag="t1")
                t2 = rpool.tile([P, HF], f32, tag="t2")
                q_ev, q_od = qsb[:, 0:EW:2], qsb[:, 1:EW:2]
                cosr, sinr = cst[:, 0:HF], cst[:, HF : 2 * HF]
                ttr_ew(t1, q_ev, cosr, A_.mult)
                ttr_ew(t2, q_od, sinr, A_.mult)
                ttr_ew(rq[:, 0:EW:2], t1, t2, A_.subtract)
                ttr_ew(t1, q_ev, sinr, A_.mult)
                ttr_ew(t2, q_od, cosr, A_.mult)
                ttr_ew(rq[:, 1:EW:2], t1, t2, A_.add)

                rk = rpool.tile([P, D], P_DT, tag="rk")
                k_ev, k_od = kvsb[:, 0:D:2], kvsb[:, 1:D:2]
                cosk, sink = cst[:, 0 : D // 2], cst[:, HF : HF + D // 2]
                ttr_ew(t1[:, 0 : D // 2], k_ev, cosk, A_.mult)
                ttr_ew(t2[:, 0 : D // 2], k_od, sink, A_.mult)
                ttr_ew(rk[:, 0:D:2], t1[:, 0 : D // 2], t2[:, 0 : D // 2], A_.subtract)
                ttr_ew(t1[:, 0 : D // 2], k_ev, sink, A_.mult)
                ttr_ew(t2[:, 0 : D // 2], k_od, cosk, A_.mult)
                ttr_ew(rk[:, 1:D:2], t1[:, 0 : D // 2], t2[:, 0 : D // 2], A_.add)

                nc.scalar.activation(
                    out=Vt[:, st, :],
                    in_=kvsb[:, D : 2 * D],
                    func=AF.Copy,
                    scale=float(1.0 / WS1),
                )

                # transposes into [d, s] layouts via the DMA xbar: frees the
                # PE/DVE cycles and the TT PSUM bank
                for h in range(NH):
                    nc.sync.dma_start_transpose(
                        out=QTt[:, h, st * P : (st + 1) * P],
                        in_=rq[:, h * P : (h + 1) * P],
                    )
                nc.sync.dma_start_transpose(
                    out=KTt[:, st * P : (st + 1) * P], in_=rk
                )

                if j >= 1:
                    stage_B(j - 1)
                if st >= 12:
                    drip_w0(st - 12)

        # ---------------- tail: qs=3 blocks + output projection ----------------
        with (
            tc.tile_pool(name="wopool", bufs=2) as wopool,
            tc.tile_pool(name="osb", bufs=2) as osb,
            tc.tile_pool(name="ops", bufs=3, space="PSUM") as ops,
        ):
            wot_tiles = {}
            wqueue = []
            units = {}
            wohr = woh.rearrange("(t p) m -> p t m", p=P)
            wolr = wol.rearrange("(t p) m -> p t m", p=P)

            def alloc_wot(mc):
                wth = wopool.tile([P, JT, 512], f8, tag="woh")
                wtl = wopool.tile([P, JT, 512], f8, tag="wol")
                wot_tiles[mc] = (wth, wtl)
                # quarter q covers t-pairs [4q, 4q+4); hi then lo
                for qq in range(4):
                    wqueue.append((mc, qq, 0))
                    wqueue.append((mc, qq, 1))

            def drip(n):
                for _ in range(min(n, len(wqueue))):
                    mc, qq, lo = wqueue.pop(0)
                    wt = wot_tiles[mc][lo]
                    src = wolr if lo else wohr
                    nc.sync.dma_start(
                        out=wt[:, qq * (JT // 4) : (qq + 1) * (JT // 4), :],
                        in_=src[
                            :,
                            qq * (JT // 4) : (qq + 1) * (JT // 4),
                            mc * 512 : (mc + 1) * 512,
                        ],
                    )

            def load_wot(mc):
                alloc_wot(mc)
                drip(8)

            def unit_mms(mc, it, tps, start, stop):
                wth, wtl = wot_tiles[mc]
                if (mc, it) in units:
                    O = units[(mc, it)]
                else:
                    O = ops.tile([P, 512], f32, tag="O")
                    units[(mc, it)] = O
                for n, tp in enumerate(tps):
                    lh = Aall_h[it][:, 2 * tp * P : (2 * tp + 2) * P].rearrange(
                        "a (two s) -> a two s", two=2
                    )
                    ll = Aall_l[it][:, 2 * tp * P : (2 * tp + 2) * P].rearrange(
                        "a (two s) -> a two s", two=2
                    )
                    rh = wth[:, 2 * tp : 2 * tp + 2, :]
                    rl = wtl[:, 2 * tp : 2 * tp + 2, :]
                    for k, (lhsT, rhs) in enumerate(((lh, rh), (lh, rl), (ll, rh))):
                        nc.tensor.matmul(
                            O,
                            lhsT,
                            rhs,
                            start=(start and n == 0 and k == 0),
                            stop=(stop and n == len(tps) - 1 and k == 2),
                            perf_mode=DR,
                        )

            def unit_fin(mc, it):
                O = units.pop((mc, it))
                Ot = osb.tile([P, 512], P_DT, tag="Ot")
                nc.scalar.activation(
                    out=Ot, in_=O, func=AF.Copy, scale=float(1.0 / WS3)
                )
                nc.sync.dma_start(
                    out=out[it * P : (it + 1) * P, mc * 512 : (mc + 1) * 512],
                    in_=Ot,
                )

            def unit(mc, it):
                unit_mms(mc, it, range(ST), True, True)
                unit_fin(mc, it)

            wot_tiles[0] = (w0h, w0l)  # prefetched during the interleave
            alloc_wot(1)
            alloc_wot(2)
            # qs=3 attention blocks (need all 16 s-tiles), pipelined; wo
            # chunk loads drip between rows so they never block the
            # latency-critical P transposes on the DMA engines. Phase-3
            # units split: t0-7 accumulation only needs qs<=1 heads (final
            # long before the tail), t8-15 needs the qs=3 heads.
            HALF1, HALF2 = range(0, 8), range(8, 16)
            # these two first-half units depend only on qs<=1 heads (done
            # mid-interleave) and the prefetched mc0 chunk: they fill the
            # PE idle at tail start
            unit_mms(0, 0, HALF1, True, False)
            unit_mms(0, 1, HALF1, True, False)
            PTt, recips = open_A(12)
            for qi in range(4):
                rowA(12, qi, PTt, recips)
                drip(2)
            stage_B(11)
            for j in (13, 14, 15):
                PTt, recips = open_A(j)
                for qi in range(4):
                    rowA(j, qi, PTt, recips)
                    drip(2)
                stage_B(j - 1)
                if j == 13:
                    unit_mms(1, 0, HALF1, True, False)
                elif j == 14:
                    # Aall[0] complete after B(13)
                    unit_mms(0, 0, HALF2, False, True)
                    unit_fin(0, 0)
                    unit_mms(2, 0, HALF1, True, False)
                elif j == 15:
                    unit_mms(1, 0, HALF2, False, True)
                    unit_fin(1, 0)
            stage_B(15)
            drip(len(wqueue))
            unit_mms(2, 0, HALF2, False, True)
            unit_fin(2, 0)
            unit_mms(0, 1, HALF2, False, True)
            unit_fin(0, 1)
            # preloaded chunks' it=1 work covers the in-flight loads of the
            # later chunks (slot for mc+2 frees as soon as mc's last unit
            # is emitted)
            def load_wot_pre(mc):
                # rotate the wopre slot (mc0's chunk is consumed by now)
                wth = wopre.tile([P, JT, 512], f8, tag="wph")
                wtl = wopre.tile([P, JT, 512], f8, tag="wpl")
                wot_tiles[mc] = (wth, wtl)
                for qq in range(4):
                    wqueue.append((mc, qq, 0))
                    wqueue.append((mc, qq, 1))
                drip(8)

            def unit_split_cols(mc, it):
                # last unit: two column-halves so the closing copy/store
                # overlaps the second half's matmuls
                wth, wtl = wot_tiles[mc]
                for half in range(2):
                    O = ops.tile([P, 512], f32, tag="O")
                    cl, ch = half * 256, (half + 1) * 256
                    for n, tp in enumerate(range(ST)):
                        lh = Aall_h[it][:, 2 * tp * P : (2 * tp + 2) * P].rearrange(
                            "a (two s) -> a two s", two=2
                        )
                        ll = Aall_l[it][:, 2 * tp * P : (2 * tp + 2) * P].rearrange(
                            "a (two s) -> a two s", two=2
                        )
                        rh = wth[:, 2 * tp : 2 * tp + 2, cl:ch]
                        rl = wtl[:, 2 * tp : 2 * tp + 2, cl:ch]
                        for k, (lhsT, rhs) in enumerate(
                            ((lh, rh), (lh, rl), (ll, rh))
                        ):
                            nc.tensor.matmul(
                                O[:, 0:256],
                                lhsT,
                                rhs,
                                start=(n == 0 and k == 0),
                                stop=(n == ST - 1 and k == 2),
                                perf_mode=DR,
                            )
                    Ot = osb.tile([P, 512], P_DT, tag="Ot")
                    nc.scalar.activation(
                        out=Ot[:, 0:256],
                        in_=O[:, 0:256],
                        func=AF.Copy,
                        scale=float(1.0 / WS3),
                    )
                    nc.sync.dma_start(
                        out=out[
                            it * P : (it + 1) * P,
                            mc * 512 + cl : mc * 512 + ch,
                        ],
                        in_=Ot[:, 0:256],
                    )

            for mc in (1, 2, 3, 4, 5, 6, 7):
                if mc >= 3:
                    unit(mc, 0)
                if mc == 7:
                    unit_split_cols(mc, 1)
                else:
                    unit(mc, 1)
                wot_tiles.pop(mc)
                nxt = mc + 2 if mc >= 3 else {1: 3, 2: 4}.get(mc)
                if nxt is not None and nxt < MC and nxt not in wot_tiles:
                    load_wot(nxt)
                if mc == 2:
                    load_wot(5)

    nc.compile()
    return nc


def is_pure_causal(mask, SEQ):
    """True iff mask[i,j] == 0 for j<=i and <= NEG_THRESH for j>i."""
    m = np.asarray(mask, np.float32)
    if m.shape != (SEQ, SEQ):
        return False
    j = np.arange(SEQ)
    allowed = j[None, :] <= j[:, None]
    return bool((m[allowed] == 0).all() and (m[~allowed] <= NEG_THRESH).all())


def make_rope_tables(cos_freq, sin_freq, SEQ, scale_quarter):
    cos_t = np.tile(np.asarray(cos_freq, np.float32) * scale_quarter, (1, NH))
    sin_t = np.tile(np.asarray(sin_freq, np.float32) * scale_quarter, (1, NH))
    return np.ascontiguousarray(
        np.concatenate([cos_t, sin_t], axis=1).astype(np.float32)
    )




_BUILD_CACHE = {}


def kernel(
    x,
    cos_freq,
    sin_freq,
    positions,
    mask,
    wq,
    wk,
    wv,
    wo,
    _trace=False,
):
    import sys

    if "/opt/trn_rl_repo" not in sys.path:
        sys.path.insert(0, "/opt/trn_rl_repo")
    from concourse.bass_utils import run_bass_kernel_spmd
    import ml_dtypes

    x = np.asarray(x, np.float32)
    mask = np.asarray(mask, np.float32)
    wq = np.asarray(wq, np.float32)
    wk = np.asarray(wk, np.float32)
    wv = np.asarray(wv, np.float32)
    wo = np.asarray(wo, np.float32)
    SEQ, DIM = x.shape
    assert wq.shape[0] == CORES * NH * D and wk.shape[0] == CORES * D
    assert 2 * SEQ == wq.shape[0], "flatten structure requires H*D == 2*SEQ"

    bf16 = ml_dtypes.bfloat16
    f8 = ml_dtypes.float8_e4m3
    ST_, DD_ = SEQ // P, DIM // P

    if is_pure_causal(mask, SEQ):
        key = (SEQ, DIM, "causal")
        if key not in _BUILD_CACHE:
            _BUILD_CACHE[key] = build_attention_v7(SEQ, DIM)
        nc = _BUILD_CACHE[key]

        def hilo(a):
            hi = np.ascontiguousarray(a).astype(f8)
            lo = np.ascontiguousarray(a - hi.astype(np.float32)).astype(f8)
            return hi, lo

        # fold sqrt(scale) and the 1/WS1 weight pre-scale into rope tables
        scale_quarter = np.float32(D ** -0.25 / WS1)
        cs = make_rope_tables(cos_freq, sin_freq, SEQ, scale_quarter).astype(bf16)
        xT = np.ascontiguousarray(x.reshape(ST_, P, DD_, P).transpose(3, 0, 2, 1))
        xTh, xTl = hilo(xT)
        # wo row-blocks permuted so DoubleRow contraction pairs are adjacent
        JT_ = 2 * SEQ // P
        perm = [(jt % 2) * (JT_ // 2) + jt // 2 for jt in range(JT_)]
        woP = np.ascontiguousarray(
            (wo.T * np.float32(WS3)).reshape(JT_, P, DIM)[perm].reshape(2 * SEQ, DIM)
        )
        woh, wol = hilo(woP)
        tri = np.ascontiguousarray(mask[0:P, 0:P])

        in_maps = []
        for c in range(CORES):
            w_c = np.concatenate(
                [
                    wq[c * NH * D : (c + 1) * NH * D],
                    wk[c * D : (c + 1) * D],
                    wv[c * D : (c + 1) * D],
                ],
                axis=0,
            )
            wTh_, wTl_ = hilo(w_c.T * np.float32(WS1))
            in_maps.append(
                {
                    "xTh": xTh,
                    "xTl": xTl,
                    "wTh": wTh_,
                    "wTl": wTl_,
                    "cs": cs,
                    "tri": tri,
                    "woh": woh,
                    "wol": wol,
                }
            )
        res = run_bass_kernel_spmd(nc, in_maps, list(range(CORES)), trace=_trace)
        outp = np.concatenate(
            [np.asarray(res.results[c]["out"]) for c in range(CORES)], axis=0
        ).astype(np.float32)
        if _trace:
            return outp, res
        return outp

    # ---------------- general-mask fallback (v1 kernel) ----------------
    plan, blocks = analyze_mask(mask, SEQ)
    n_uniq = len(blocks)
    key = (SEQ, DIM, tuple(tuple(r) for r in plan))
    if key not in _BUILD_CACHE:
        _BUILD_CACHE[key] = build_attention_nc(SEQ, DIM, plan, n_uniq)
    nc = _BUILD_CACHE[key]

    scale_quarter = np.float32(D ** -0.25)
    csf = make_rope_tables(cos_freq, sin_freq, SEQ, scale_quarter)
    xT = np.ascontiguousarray(
        x.reshape(ST_, P, DD_, P).transpose(3, 0, 2, 1)
    ).astype(bf16)
    woT = np.ascontiguousarray(wo.T).astype(bf16)
    if n_uniq:
        mbs = np.ascontiguousarray(np.stack(blocks, axis=0))
    else:
        mbs = np.zeros((1, P, 512), np.float32)

    in_maps = []
    for c in range(CORES):
        w_c = np.concatenate(
            [
                wq[c * NH * D : (c + 1) * NH * D],
                wk[c * D : (c + 1) * D],
                wv[c * D : (c + 1) * D],
            ],
            axis=0,
        )
        in_maps.append(
            {
                "xT": xT,
                "wT": np.ascontiguousarray(w_c.T).astype(bf16),
                "cs": csf,
                "maskb": mbs,
                "woT": woT,
            }
        )
    res = run_bass_kernel_spmd(nc, in_maps, list(range(CORES)), trace=_trace)
    outp = np.concatenate(
        [np.asarray(res.results[c]["out"]) for c in range(CORES)], axis=0
    ).astype(np.float32)
    if _trace:
        return outp, res
    return outp


# revision 9
# speedup vs baseline: 1.0180x; 1.0026x over previous
"""Trainium2 Bass kernel for nn_Attention (GQA + RoPE + sliding-window mask).

Sharding: tensor-parallel over heads across 8 cores (4 q heads + 1 kv head
per core). The reference's quirky output flatten ((H,S,D)->(H,D,S)->
reshape(S, H*D)) makes the final projection row-shard by head block: core c
produces rows [256c, 256c+256) of the (2048, 4096) result with no collective.

Fast path (pure causal mask, the shape this problem produces):
  * phase 1 (QKV projections) and phase 3 (output projection) run as
    fp8-e4m3 hi/lo pairs in DoubleRow perf mode: X @ W ~= Xh@Wh + Xh@Wl +
    Xl@Wh with Xh = fp8(X), Xl = fp8(X - Xh) - 0.75x the PE time of one
    bf16 pass and more accurate than bf16 (~9-10 effective mantissa bits).
  * attention (phase 2) is interleaved INTO phase 1: block (qs, h) only
    needs s-tiles <= 4qs+3, so blocks weave between projection s-tiles with
    score rows emitted between x-chunk groups - softmax (ACT) latency hides
    under the projection matmuls.
  * no-max softmax (causal logits here are O(10), exp is safe in fp32),
    exp reads score PSUM directly with accum_out row sums; only the
    128-wide triangular diagonal block gets a mask add; diagonal PV
    matmuls are narrowed to the live query columns.
  * P transposed via DMA xbar; wo chunks prefetched/dripped so the big
    loads never head-of-line block the latency-critical transposes.

Fallback (any other mask): the v1 kernel (per-chunk mask add + 2-pass
max/exp softmax), correct for arbitrary additive masks.
"""

import numpy as np
from contextlib import ExitStack

P = 128
D = 128  # head dim
NH = 4   # q heads per core
CORES = 8
NEG_THRESH = -1e8


def build_attention_nc(
    SEQ,
    DIM,
    plan,
    n_uniq,
    p_dt_name="bfloat16",
    wo_dt_name="bfloat16",
    proj_dt_name="bfloat16",
    proj_f32r=True,
    score_f32r=True,
    use_dma_t=True,
):
    """Build the per-core Bass program.

    plan: list over q-tiles i (SEQ//128 entries) of lists of (chunk_idx, uid)
          where uid == -1 means the 512-wide chunk needs no mask add, else the
          index into the maskb tensor. Chunks absent from the list are fully
          masked (skipped).
    """
    import concourse.bass as bass
    import concourse.bacc as bacc
    import concourse.mybir as mybir
    import concourse.tile as tile
    from concourse.masks import make_identity

    f32 = mybir.dt.float32
    f32r = mybir.dt.float32r
    P_DT = getattr(mybir.dt, p_dt_name)
    WO_DT = getattr(mybir.dt, wo_dt_name)
    PJ_DT = getattr(mybir.dt, proj_dt_name)
    pj_f32r = proj_f32r and proj_dt_name == "float32"

    ST = SEQ // P          # 16 s-tiles
    DD = DIM // P          # 32 contraction tiles
    KC = SEQ // 512        # 4 key chunks
    QS = SEQ // 512        # 4 query supers
    EW = NH * D            # 512 q-projection width
    JT = 2 * SEQ // P      # 32 j-tiles for final matmul
    MC = DIM // 512        # 8 output chunks
    ITILES = (NH * 64) // P  # 2 output row tiles
    assert NH == 4 and SEQ % 512 == 0 and DIM % 512 == 0

    def mm_cast(ap, use_r):
        return ap.bitcast(f32r) if use_r else ap

    nc = bacc.Bacc(trn_type="TRN2", debug=False, num_devices=CORES)

    # x pre-tiled on host: xT[p, st, t, si] = x[st*128+si, t*128+p] so each
    # streamed chunk is one DMA with 2KB contiguous per-partition runs
    xT = nc.dram_tensor("xT", [P, ST, DD, P], PJ_DT, kind="ExternalInput").ap()
    wT = nc.dram_tensor("wT", [DIM, EW + 2 * D], PJ_DT, kind="ExternalInput").ap()
    cs = nc.dram_tensor("cs", [SEQ, EW], f32, kind="ExternalInput").ap()
    mb = nc.dram_tensor(
        "maskb", [max(n_uniq, 1), P, 512], f32, kind="ExternalInput"
    ).ap()
    woT = nc.dram_tensor("woT", [2 * SEQ, DIM], WO_DT, kind="ExternalInput").ap()
    out = nc.dram_tensor("out", [NH * 64, DIM], f32, kind="ExternalOutput").ap()

    with tile.TileContext(nc) as tc, ExitStack() as ctx:
        const = ctx.enter_context(tc.tile_pool(name="const", bufs=1))
        idF = const.tile([P, P], f32)
        make_identity(nc, idF)
        idP = const.tile([P, P], P_DT)
        make_identity(nc, idP)
        zeros = const.tile([P, 512], f32)
        nc.vector.memset(zeros, 0.0)

        pers = ctx.enter_context(tc.tile_pool(name="pers", bufs=1))
        QTt = pers.tile([P, NH, ST * P], f32)   # [d, h, s]
        KTt = pers.tile([P, ST * P], f32)       # [d, s]
        Vt = pers.tile([P, ST, D], P_DT)        # [k(part), ktile, d]
        if n_uniq > 0:
            mbt = pers.tile([P, n_uniq, 512], f32)

        # ---------------- phase 1: projections + rope + layout ----------------
        with (
            tc.tile_pool(name="wpool", bufs=1) as wpool,
            tc.tile_pool(name="xpool", bufs=6) as xpool,
            tc.tile_pool(name="cspool", bufs=2) as cspool,
            tc.tile_pool(name="rpool", bufs=2) as rpool,
            tc.tile_pool(name="qps", bufs=2, space="PSUM") as qps,
            tc.tile_pool(name="kvps", bufs=2, space="PSUM") as kvps,
            tc.tile_pool(name="tps", bufs=2, space="PSUM") as tps,
            tc.tile_pool(name="t2ps", bufs=2, space="PSUM") as t2ps,
        ):
            XGW = min(8, DD)
            wTt = wpool.tile([P, DD, EW + 2 * D], PJ_DT)
            wTr = wT.rearrange("(t p) e -> p t e", p=P)

            XG = min(8, DD)  # dd-tiles per streamed x chunk
            NG = DD // XG
            xTr = xT
            # Interleave the weight-chunk loads with s-tile 0's x chunks so
            # the first matmuls start as soon as chunk 0 of each lands.
            st0_x = []
            for g in range(NG):
                xTt = xpool.tile([P, XG, P], PJ_DT, tag="xT")
                nc.sync.dma_start(
                    out=xTt, in_=xTr[:, 0, g * XG : (g + 1) * XG, :]
                )
                st0_x.append(xTt)
                gw = g % (DD // XGW)
                nc.sync.dma_start(
                    out=wTt[:, gw * XGW : (gw + 1) * XGW, :],
                    in_=wTr[:, gw * XGW : (gw + 1) * XGW, :],
                )
            for st in range(ST):
                cst = cspool.tile([P, EW], f32, tag="cs")
                nc.sync.dma_start(out=cst, in_=cs[st * P : (st + 1) * P, :])

                Qp = qps.tile([P, EW], f32, tag="Qp")
                KVp = kvps.tile([P, 2 * D], f32, tag="KVp")
                for g in range(DD // XG):
                    if st == 0:
                        xTt = st0_x[g]
                    else:
                        xTt = xpool.tile([P, XG, P], PJ_DT, tag="xT")
                        nc.sync.dma_start(
                            out=xTt,
                            in_=xTr[:, st, g * XG : (g + 1) * XG, :],
                        )
                    for tt in range(XG):
                        t = g * XG + tt
                        lhsT = mm_cast(xTt[:, tt, :], pj_f32r)
                        nc.tensor.matmul(
                            Qp,
                            lhsT,
                            mm_cast(wTt[:, t, 0:EW], pj_f32r),
                            start=(t == 0),
                            stop=(t == DD - 1),
                        )
                        nc.tensor.matmul(
                            KVp,
                            lhsT,
                            mm_cast(wTt[:, t, EW : EW + 2 * D], pj_f32r),
                            start=(t == 0),
                            stop=(t == DD - 1),
                        )

                # rope via strided even/odd halves (2-level APs only — 3-level
                # APs overflow the fixed ISA instruction encoding).
                # tensor_tensor_reduce instead of tensor_tensor: the plain TT
                # ISA struct has a single sync-wait slot and walrus codegen
                # rejects the PE+DMA double wait Tile emits here; the TTR/ISA
                # struct carries up to 8. accum outputs are dummies.
                def ttr_ew(out, in0, in1, op):
                    nc.vector.tensor_tensor(out=out, in0=in0, in1=in1, op=op)

                A_ = mybir.AluOpType
                HF = EW // 2  # 256: cos table width for q
                rq = rpool.tile([P, EW], f32, tag="rq")
                t1 = rpool.tile([P, HF], f32, tag="t1")
                t2 = rpool.tile([P, HF], f32, tag="t2")
                q_ev, q_od = Qp[:, 0:EW:2], Qp[:, 1:EW:2]
                cosr, sinr = cst[:, 0:HF], cst[:, HF : 2 * HF]
                ttr_ew(t1, q_ev, cosr, A_.mult)
                ttr_ew(t2, q_od, sinr, A_.mult)
                ttr_ew(rq[:, 0:EW:2], t1, t2, A_.subtract)
                ttr_ew(t1, q_ev, sinr, A_.mult)
                ttr_ew(t2, q_od, cosr, A_.mult)
                ttr_ew(rq[:, 1:EW:2], t1, t2, A_.add)

                rk = rpool.tile([P, D], f32, tag="rk")
                k_ev, k_od = KVp[:, 0:D:2], KVp[:, 1:D:2]
                cosk, sink = cst[:, 0 : D // 2], cst[:, HF : HF + D // 2]
                ttr_ew(t1[:, 0 : D // 2], k_ev, cosk, A_.mult)
                ttr_ew(t2[:, 0 : D // 2], k_od, sink, A_.mult)
                ttr_ew(rk[:, 0:D:2], t1[:, 0 : D // 2], t2[:, 0 : D // 2], A_.subtract)
                ttr_ew(t1[:, 0 : D // 2], k_ev, sink, A_.mult)
                ttr_ew(t2[:, 0 : D // 2], k_od, cosk, A_.mult)
                ttr_ew(rk[:, 1:D:2], t1[:, 0 : D // 2], t2[:, 0 : D // 2], A_.add)

                # V -> bf16 [k, d] layout (ACT copy, cast)
                nc.scalar.activation(
                    out=Vt[:, st, :],
                    in_=KVp[:, D : 2 * D],
                    func=mybir.ActivationFunctionType.Copy,
                )

                # transpose rq (per head) and rk into [d, s] layouts
                T1 = tps.tile([P, EW], f32, tag="T1")
                for h in range(NH):
                    nc.tensor.transpose(
                        T1[:, h * P : (h + 1) * P], rq[:, h * P : (h + 1) * P], idF
                    )
                # write as f32r so walrus accepts them as f32r matmul operands
                nc.vector.tensor_copy(
                    out=mm_cast(QTt[:, :, st * P : (st + 1) * P], score_f32r),
                    in_=T1.rearrange("p (h s) -> p h s", h=NH),
                )
                T2 = t2ps.tile([P, P], f32, tag="T2")
                nc.tensor.transpose(T2, rk, idF)
                nc.vector.tensor_copy(
                    out=mm_cast(KTt[:, st * P : (st + 1) * P], score_f32r), in_=T2
                )

        # ---------------- phase 2: attention ----------------
        if n_uniq > 0:
            nc.sync.dma_start(out=mbt, in_=mb.rearrange("u p m -> p u m"))
        apool = ctx.enter_context(tc.tile_pool(name="apool", bufs=1))
        # split by head-pair so phase 3's first row-tile can start once
        # heads 0-1 finish, overlapping the rest of phase 2
        Aall = [
            apool.tile([P, 2 * ST * D], P_DT, name=f"Aall{i}")
            for i in range(NH // 2)
        ]
        with (
            tc.tile_pool(name="ptsb", bufs=2) as ptsb,
            tc.tile_pool(name="spool", bufs=6) as spool,
            tc.tile_pool(name="ppool", bufs=4) as ppool,
            tc.tile_pool(name="stat", bufs=12) as stat,
            tc.tile_pool(name="atsb", bufs=3) as atsb,
            tc.tile_pool(name="sps", bufs=2, space="PSUM") as sps,
            tc.tile_pool(name="ptps", bufs=2, space="PSUM") as ptps,
            tc.tile_pool(name="atps", bufs=1, space="PSUM") as atps,
            tc.tile_pool(name="aps", bufs=1, space="PSUM") as aps,
            tc.tile_pool(name="wopool", bufs=2 if n_uniq <= 4 else 1) as wopool,
            tc.tile_pool(name="osb", bufs=2) as osb,
            tc.tile_pool(name="ops", bufs=3, space="PSUM") as ops,
        ):
            for h in range(NH):
                for qs in range(QS):
                    PTt = ptsb.tile([P, ST, 512], P_DT, tag="PT")
                    kts_used = set()
                    recips = []
                    pt_written = set()
                    for qi in range(4):
                        i = 4 * qs + qi
                        row = plan[i]
                        if not row:
                            recips.append(None)
                            continue
                        pairs = [row[k : k + 2] for k in range(0, len(row), 2)]
                        stats = stat.tile([P, KC], f32, tag="stats")
                        ncols = 0
                        S_tiles = []
                        for pr in pairs:
                            W = 512 * len(pr)
                            S = sps.tile([P, 1024], f32, tag="S")
                            Ssb = spool.tile([P, 1024], f32, tag="Ssb")
                            masked_any = any(uid >= 0 for (_, uid) in pr)
                            for k, (c, uid) in enumerate(pr):
                                sl = S[:, k * 512 : (k + 1) * 512]
                                nc.tensor.matmul(
                                    sl,
                                    mm_cast(
                                        QTt[:, h, i * P : (i + 1) * P], score_f32r
                                    ),
                                    mm_cast(
                                        KTt[:, c * 512 : (c + 1) * 512], score_f32r
                                    ),
                                    start=True,
                                    stop=True,
                                )
                                if uid >= 0:
                                    nc.vector.tensor_add(sl, sl, mbt[:, uid, :])
                                # copy PSUM->SBUF to free the score bank early;
                                # alternate DVE/ACT to balance engine load
                                dst = Ssb[:, k * 512 : (k + 1) * 512]
                                if (i + k) % 2 == 0:
                                    nc.vector.tensor_copy(out=dst, in_=sl)
                                else:
                                    nc.scalar.activation(
                                        out=dst,
                                        in_=sl,
                                        func=mybir.ActivationFunctionType.Copy,
                                    )
                                if masked_any or len(pr) == 1:
                                    nc.vector.tensor_reduce(
                                        out=stats[:, ncols : ncols + 1],
                                        in_=dst,
                                        axis=mybir.AxisListType.X,
                                        op=mybir.AluOpType.max,
                                    )
                                    ncols += 1
                            if not masked_any and len(pr) == 2:
                                # one pair-wide max over both chunks (SBUF 2x)
                                nc.vector.tensor_reduce(
                                    out=stats[:, ncols : ncols + 1],
                                    in_=Ssb,
                                    axis=mybir.AxisListType.X,
                                    op=mybir.AluOpType.max,
                                )
                                ncols += 1
                            S_tiles.append((Ssb, pr))
                        negm = stat.tile([P, 1], f32, tag="negm")
                        nc.vector.tensor_reduce(
                            out=negm,
                            in_=stats[:, 0:ncols],
                            axis=mybir.AxisListType.X,
                            op=mybir.AluOpType.max,
                            negate=True,
                        )
                        sums = stat.tile([P, KC], f32, tag="sums")
                        for k, (Sk, pr) in enumerate(S_tiles):
                            W = 512 * len(pr)
                            Pt = ppool.tile([P, 1024], P_DT, tag="P")
                            nc.scalar.activation(
                                out=Pt[:, 0:W],
                                in_=Sk[:, 0:W],
                                func=mybir.ActivationFunctionType.Exp,
                                bias=negm,
                                accum_out=sums[:, k : k + 1],
                            )
                            # transpose P [q, k] -> PT [k, q]
                            for j, (c, uid) in enumerate(pr):
                                if use_dma_t:
                                    nc.sync.dma_start_transpose(
                                        out=PTt[
                                            :, 4 * c : 4 * c + 4, qi * P : (qi + 1) * P
                                        ],
                                        in_=Pt[:, j * 512 : (j + 1) * 512],
                                    )
                                else:
                                    PTp = ptps.tile([P, 512], P_DT, tag="PTp")
                                    for jj in range(4):
                                        nc.tensor.transpose(
                                            PTp[:, jj * P : (jj + 1) * P],
                                            Pt[:, j * 512 + jj * P : j * 512 + (jj + 1) * P],
                                            idP,
                                        )
                                    nc.vector.tensor_copy(
                                        out=PTt[:, 4 * c : 4 * c + 4, qi * P : (qi + 1) * P],
                                        in_=PTp.rearrange("p (kt q) -> p kt q", kt=4),
                                    )
                                for jj in range(4):
                                    kts_used.add(4 * c + jj)
                                    pt_written.add((4 * c + jj, qi))
                        denom = stat.tile([P, 1], f32, tag="denom")
                        nc.vector.tensor_reduce(
                            out=denom,
                            in_=sums[:, 0 : len(S_tiles)],
                            axis=mybir.AxisListType.X,
                            op=mybir.AluOpType.add,
                        )
                        recip = stat.tile([P, 1], f32, tag="recip")
                        nc.vector.reciprocal(recip, denom)
                        recips.append(recip)

                    # zero-fill PT holes (only for non-causal masks)
                    kts = sorted(kts_used)
                    for kt in kts:
                        for qi in range(4):
                            if (kt, qi) not in pt_written and recips[qi] is not None:
                                nc.vector.memset(
                                    PTt[:, kt, qi * P : (qi + 1) * P], 0.0
                                )
                            elif recips[qi] is None:
                                nc.vector.memset(
                                    PTt[:, kt, qi * P : (qi + 1) * P], 0.0
                                )

                    if not kts:
                        continue
                    # PV: A^T[d, q] accumulated over key tiles
                    At = atps.tile([P, 512], f32, tag="At")
                    for n, kt in enumerate(kts):
                        nc.tensor.matmul(
                            At,
                            Vt[:, kt, :],
                            PTt[:, kt, :],
                            start=(n == 0),
                            stop=(n == len(kts) - 1),
                        )
                    Atsb = atsb.tile([P, 512], P_DT, tag="Atsb")
                    nc.vector.tensor_copy(out=Atsb, in_=At)
                    Ap = aps.tile([P, 512], P_DT, tag="Ap")
                    for qi in range(4):
                        nc.tensor.transpose(
                            Ap[:, qi * P : (qi + 1) * P],
                            Atsb[:, qi * P : (qi + 1) * P],
                            idP,
                        )
                    # Aall layout: [sp, (t*2 + dd)*128 + hb*64 + p] so the final
                    # matmul's stationary slices are contiguous (walrus requires
                    # a single free dim on weight APs)
                    Ah = Aall[h // 2]
                    hb = h % 2
                    for qi in range(4):
                        i = 4 * qs + qi
                        # dview[sp, p, dd] == Ah[:, i*256 + dd*128 + hb*64 + p]
                        dview = Ah[:, i * 2 * P : (i + 1) * 2 * P].rearrange(
                            "a (dd j) -> a dd j", dd=2
                        )[:, :, hb * 64 : hb * 64 + 64].rearrange(
                            "a dd p -> a p dd"
                        )
                        if recips[qi] is None:
                            nc.vector.memset(dview, 0.0)
                            continue
                        nc.scalar.activation(
                            out=dview,
                            in_=Ap[:, qi * P : (qi + 1) * P].rearrange(
                                "a (p two) -> a p two", two=2
                            ),
                            func=mybir.ActivationFunctionType.Copy,
                            scale=recips[qi],
                        )

            # ---------------- phase 3: output projection ----------------
            for mc in range(MC):
                wot = wopool.tile([P, JT, 512], WO_DT, tag="wo")
                nc.sync.dma_start(
                    out=wot,
                    in_=woT[:, mc * 512 : (mc + 1) * 512].rearrange(
                        "(t p) m -> p t m", p=P
                    ),
                )
                for it in range(ITILES):
                    O = ops.tile([P, 512], f32, tag="O")
                    Av = Aall[it]
                    for jt in range(JT):
                        ddj, t = jt // ST, jt % ST
                        lhsT = Av[:, (t * 2 + ddj) * P : (t * 2 + ddj + 1) * P]
                        nc.tensor.matmul(
                            O,
                            lhsT,
                            wot[:, jt, :],
                            start=(jt == 0),
                            stop=(jt == JT - 1),
                        )
                    Ot = osb.tile([P, 512], f32, tag="Ot")
                    nc.scalar.activation(
                        out=Ot, in_=O, func=mybir.ActivationFunctionType.Copy
                    )
                    nc.sync.dma_start(
                        out=out[it * P : (it + 1) * P, mc * 512 : (mc + 1) * 512],
                        in_=Ot,
                    )

    # Bacc.compile() legalizes sync (>=2 waits split into EventSemaphore
    # instructions — this walrus caps every instruction at ONE sync wait)
    nc.compile()
    return nc


def analyze_mask(mask, SEQ):
    """Classify 128x512 mask blocks: skip / free / masked(dedup uid)."""
    ST = SEQ // P
    KC = SEQ // 512
    uniq = {}
    blocks = []
    plan = []
    for i in range(ST):
        row = []
        for c in range(KC):
            blk = mask[i * P : (i + 1) * P, c * 512 : (c + 1) * 512]
            if (blk <= NEG_THRESH).all():
                continue
            if not blk.any():
                row.append((c, -1))
            else:
                key = blk.tobytes()
                if key not in uniq:
                    uniq[key] = len(blocks)
                    blocks.append(np.ascontiguousarray(blk))
                row.append((c, uniq[key]))
        if not row:
            # fully masked query rows: keep all chunks so softmax matches
            # the reference's uniform distribution over -1e9 logits
            for c in range(KC):
                blk = mask[i * P : (i + 1) * P, c * 512 : (c + 1) * 512]
                key = blk.tobytes()
                if key not in uniq:
                    uniq[key] = len(blocks)
                    blocks.append(np.ascontiguousarray(blk))
                row.append((c, uniq[key]))
        plan.append(row)
    return plan, blocks


WS1 = 1024.0  # host pre-scale on wq/wk/wv before fp8 (values ~0.02*N(0,1))
WS3 = 256.0   # host pre-scale on wo before fp8


def build_attention_v7(
    SEQ,
    DIM,
    p_dt_name="bfloat16",
):
    import concourse.bass as bass
    import concourse.bacc as bacc
    import concourse.mybir as mybir
    import concourse.tile as tile
    from concourse.masks import make_identity

    f32 = mybir.dt.float32
    f8 = mybir.dt.float8e4
    P_DT = getattr(mybir.dt, p_dt_name)
    A_ = mybir.AluOpType
    AF = mybir.ActivationFunctionType
    DR = mybir.MatmulPerfMode.DoubleRow

    ST = SEQ // P          # 16 s-tiles
    DD = DIM // P          # 32 contraction tiles
    QS = SEQ // 512        # 4 query supers
    EW = NH * D            # 512 q-projection width
    JT = 2 * SEQ // P      # 32 j-tiles for final matmul
    MC = DIM // 512        # 8 output chunks
    ITILES = (NH * 64) // P  # 2 output row tiles
    assert NH == 4 and SEQ % 512 == 0 and DIM % 512 == 0

    nc = bacc.Bacc(trn_type="TRN2", debug=False, num_devices=CORES)

    # x hi/lo packed per dd-tile so each DMA moves 2KB contiguous runs
    xTc = nc.dram_tensor("xTc", [P, ST, DD, 2, P], f8, kind="ExternalInput").ap()
    wTh = nc.dram_tensor("wTh", [DIM, EW + 2 * D], f8, kind="ExternalInput").ap()
    wTl = nc.dram_tensor("wTl", [DIM, EW + 2 * D], f8, kind="ExternalInput").ap()
    cs = nc.dram_tensor("cs", [SEQ, EW], P_DT, kind="ExternalInput").ap()
    tri = nc.dram_tensor("tri", [P, P], f32, kind="ExternalInput").ap()
    # wo row blocks permuted host-side: block jt' = 2t+dd <- original dd*16+t
    woh = nc.dram_tensor("woh", [2 * SEQ, DIM], f8, kind="ExternalInput").ap()
    wol = nc.dram_tensor("wol", [2 * SEQ, DIM], f8, kind="ExternalInput").ap()
    out = nc.dram_tensor("out", [NH * 64, DIM], P_DT, kind="ExternalOutput").ap()

    with tile.TileContext(nc) as tc, ExitStack() as ctx:
        const = ctx.enter_context(tc.tile_pool(name="const", bufs=1))
        idP = const.tile([P, P], P_DT)
        make_identity(nc, idP)

        pers = ctx.enter_context(tc.tile_pool(name="pers", bufs=1))
        QTt = pers.tile([P, NH, ST * P], P_DT)   # [d, h, s]
        KTt = pers.tile([P, ST * P], P_DT)       # [d, s]
        Vt = pers.tile([P, ST, D], P_DT)         # [k(part), ktile, d]
        trit = pers.tile([P, P], f32)
        nc.sync.dma_start(out=trit, in_=tri)

        # mc=0 wo chunk is prefetched during the interleaved region (the only
        # chunk SBUF has room for before the phase-1 pools close)
        wopre = ctx.enter_context(tc.tile_pool(name="wopre", bufs=1))
        w0h = wopre.tile([P, 2 * SEQ // P, 512], f8, tag="wph")
        w0l = wopre.tile([P, 2 * SEQ // P, 512], f8, tag="wpl")
        wohr_ = woh.rearrange("(t p) m -> p t m", p=P)
        wolr_ = wol.rearrange("(t p) m -> p t m", p=P)

        def drip_w0(qq):
            q4 = (2 * SEQ // P) // 4
            nc.sync.dma_start(
                out=w0h[:, qq * q4 : (qq + 1) * q4, :],
                in_=wohr_[:, qq * q4 : (qq + 1) * q4, 0:512],
            )
            nc.sync.dma_start(
                out=w0l[:, qq * q4 : (qq + 1) * q4, :],
                in_=wolr_[:, qq * q4 : (qq + 1) * q4, 0:512],
            )

        apool = ctx.enter_context(tc.tile_pool(name="apool", bufs=1))
        Aall_h = [
            apool.tile([P, 2 * ST * D], f8, name=f"Aallh{i}") for i in range(NH // 2)
        ]
        Aall_l = [
            apool.tile([P, 2 * ST * D], f8, name=f"Aalll{i}") for i in range(NH // 2)
        ]
        # attention pools live through phase 1+2 and the tail
        ptsb = ctx.enter_context(tc.tile_pool(name="ptsb", bufs=2))
        ppool = ctx.enter_context(tc.tile_pool(name="ppool", bufs=8))
        stat = ctx.enter_context(tc.tile_pool(name="stat", bufs=12))
        atsb = ctx.enter_context(tc.tile_pool(name="atsb", bufs=3))
        s1ps = ctx.enter_context(tc.tile_pool(name="s1ps", bufs=3, space="PSUM"))
        atps = ctx.enter_context(tc.tile_pool(name="atps", bufs=1, space="PSUM"))
        aps = ctx.enter_context(tc.tile_pool(name="aps", bufs=1, space="PSUM"))

        blocks = [(qs, h) for qs in range(QS) for h in range(NH)]  # j = 4qs+h
        state = {}

        def rowA(j, qi, PTt, recips):
            qs, h = blocks[j]
            i = 4 * qs + qi
            f = i // 4      # fully-allowed 512-chunks
            dsub = i % 4    # full 128-subtiles in the diagonal chunk
            sums = stat.tile([P, 4], f32, tag="sums")
            ncol = 0
            lhsQ = QTt[:, h, i * P : (i + 1) * P]
            for c in range(f):
                S1t = s1ps.tile([P, 512], f32, tag="S1")
                nc.tensor.matmul(
                    S1t,
                    lhsQ,
                    KTt[:, c * 512 : (c + 1) * 512],
                    start=True,
                    stop=True,
                )
                Pt = ppool.tile([P, 512], P_DT, tag="P1")
                nc.scalar.activation(
                    out=Pt,
                    in_=S1t,
                    func=AF.Exp,
                    accum_out=sums[:, ncol : ncol + 1],
                )
                ncol += 1
                nc.sync.dma_start_transpose(
                    out=PTt[:, 4 * c : 4 * c + 4, qi * P : (qi + 1) * P],
                    in_=Pt,
                )
            # diagonal chunk, truncated to (dsub+1)*128 columns
            w = (dsub + 1) * P
            S1t = s1ps.tile([P, 512], f32, tag="S1")
            nc.tensor.matmul(
                S1t[:, 0:w],
                lhsQ,
                KTt[:, f * 512 : f * 512 + w],
                start=True,
                stop=True,
            )
            nc.vector.tensor_add(S1t[:, dsub * P : w], S1t[:, dsub * P : w], trit)
            Pt = ppool.tile([P, 512], P_DT, tag="P1")
            nc.scalar.activation(
                out=Pt[:, 0:w],
                in_=S1t[:, 0:w],
                func=AF.Exp,
                accum_out=sums[:, ncol : ncol + 1],
            )
            ncol += 1
            nc.sync.dma_start_transpose(
                out=PTt[:, 4 * f : 4 * f + dsub + 1, qi * P : (qi + 1) * P],
                in_=Pt[:, 0:w],
            )
            # masked-out subtiles of the diagonal chunk are never read: the
            # PV matmuls for diagonal key-tiles are narrowed to the live
            # query columns instead
            denom = stat.tile([P, 1], f32, tag="denom")
            nc.vector.tensor_reduce(
                out=denom, in_=sums[:, 0:ncol], axis=mybir.AxisListType.X, op=A_.add
            )
            recip = stat.tile([P, 1], f32, tag="recip")
            nc.vector.reciprocal(recip, denom)
            recips.append(recip)

        def open_A(j):
            PTt = ptsb.tile([P, ST, 512], P_DT, tag="PT")
            recips = []
            state[j] = (PTt, recips)
            return PTt, recips

        def stage_B(j):
            qs, h = blocks[j]
            PTt, recips = state.pop(j)
            nkt = 4 * qs + 4
            At = atps.tile([P, 512], f32, tag="At")
            for n in range(nkt):
                # diagonal key-tiles only reach query columns >= off
                off = max(0, n - 4 * qs) * P
                nc.tensor.matmul(
                    At[:, off:512],
                    Vt[:, n, :],
                    PTt[:, n, off:512],
                    start=(n == 0),
                    stop=(n == nkt - 1),
                )
            Atsb = atsb.tile([P, 512], P_DT, tag="Atsb")
            nc.vector.tensor_copy(out=Atsb, in_=At)
            Ap = aps.tile([P, 512], P_DT, tag="Ap")
            for qi in range(4):
                nc.tensor.transpose(
                    Ap[:, qi * P : (qi + 1) * P],
                    Atsb[:, qi * P : (qi + 1) * P],
                    idP,
                )
            hb = h % 2

            def dv(Aarr):
                return Aarr[h // 2][:, i * 2 * P : (i + 1) * 2 * P].rearrange(
                    "a (dd j) -> a dd j", dd=2
                )[:, :, hb * 64 : hb * 64 + 64].rearrange("a dd p -> a p dd")

            for qi in range(4):
                i = 4 * qs + qi
                # normalize on DVE, then split into fp8 hi + residual lo for
                # the DoubleRow output projection
                th = atsb.tile([P, P], f32, tag="th")
                nc.vector.tensor_scalar_mul(
                    th, Ap[:, qi * P : (qi + 1) * P], recips[qi]
                )
                thv = th.rearrange("a (p two) -> a p two", two=2)
                dh, dl = dv(Aall_h), dv(Aall_l)
                nc.vector.tensor_copy(out=dh, in_=thv)
                nc.vector.tensor_tensor(out=dl, in0=thv, in1=dh, op=A_.subtract)

        # ------------- phase 1 with interleaved attention blocks -------------
        with (
            tc.tile_pool(name="wpool", bufs=1) as wpool,
            tc.tile_pool(name="xpool", bufs=8) as xpool,
            tc.tile_pool(name="cspool", bufs=2) as cspool,
            tc.tile_pool(name="rpool", bufs=2) as rpool,
            tc.tile_pool(name="qps", bufs=1, space="PSUM") as qps,
            tc.tile_pool(name="kvsh", bufs=1, space="PSUM") as kvsh,
        ):
            wTth = wpool.tile([P, DD, EW + 2 * D], f8)
            wTtl = wpool.tile([P, DD, EW + 2 * D], f8)
            wTrh = wTh.rearrange("(t p) e -> p t e", p=P)
            wTrl = wTl.rearrange("(t p) e -> p t e", p=P)

            XG = min(8, DD)  # dd-tiles per streamed x chunk
            NG = DD // XG
            # interleave s-tile-0 x chunks with weight loads (weights in 8
            # sub-loads per array so the first matmuls start early)
            st0_x = []
            for g in range(NG):
                xc = xpool.tile([P, XG, 2, P], f8, tag="xc")
                nc.sync.dma_start(out=xc, in_=xTc[:, 0, g * XG : (g + 1) * XG, :, :])
                st0_x.append(xc)
                for half in range(2):
                    gw = 2 * g + half
                    nc.sync.dma_start(
                        out=wTth[:, gw * 4 : (gw + 1) * 4, :],
                        in_=wTrh[:, gw * 4 : (gw + 1) * 4, :],
                    )
                    nc.sync.dma_start(
                        out=wTtl[:, gw * 4 : (gw + 1) * 4, :],
                        in_=wTrl[:, gw * 4 : (gw + 1) * 4, :],
                    )
            def qkv_terms(Qp, KVp, xc, g, first_flag=True):
                for tp in range(XG // 2):
                    t = g * XG + 2 * tp
                    first = t == 0 and first_flag
                    last = t == DD - 2
                    lh = xc[:, 2 * tp : 2 * tp + 2, 0, :]
                    ll = xc[:, 2 * tp : 2 * tp + 2, 1, :]
                    terms = ((lh, wTth), (lh, wTtl), (ll, wTth))
                    for k, (lhsT, wt) in enumerate(terms):
                        nc.tensor.matmul(
                            Qp,
                            lhsT,
                            wt[:, t : t + 2, 0:EW],
                            start=(first and k == 0),
                            stop=(last and k == 2),
                            perf_mode=DR,
                        )
                    for k, (lhsT, wt) in enumerate(terms):
                        nc.tensor.matmul(
                            KVp,
                            lhsT,
                            wt[:, t : t + 2, EW : EW + 2 * D],
                            start=(first and k == 0),
                            stop=(last and k == 2),
                            perf_mode=DR,
                        )

            # s-tiles 0 and 1 are paired: the weight stream is the DMA
            # bottleneck at kernel start, so each weight granule feeds two
            # s-tiles' matmuls (s1 borrows PSUM from the still-idle
            # attention pools)
            cst0 = cspool.tile([P, EW], P_DT, tag="cs")
            nc.sync.dma_start(out=cst0, in_=cs[0:P, :])
            cst1 = cspool.tile([P, EW], P_DT, tag="cs")
            nc.sync.dma_start(out=cst1, in_=cs[P : 2 * P, :])
            Qp0 = qps.tile([P, EW], f32, tag="Qp")
            KVp0 = kvsh.tile([P, 2 * D], f32, tag="KVp")
            Qp1 = s1ps.tile([P, 512], f32, tag="S1")
            KVt1 = atps.tile([P, 512], f32, tag="At")
            KVp1 = KVt1[:, 0 : 2 * D]
            st1_x = []
            for g in range(NG):
                xc1 = xpool.tile([P, XG, 2, P], f8, tag="xc")
                nc.sync.dma_start(out=xc1, in_=xTc[:, 1, g * XG : (g + 1) * XG, :, :])
                st1_x.append(xc1)
                qkv_terms(Qp0, KVp0, st0_x[g], g)
                qkv_terms(Qp1, KVp1, xc1, g)

            for st in range(ST):
                j = st - 4  # attention block woven into this s-tile
                if j >= 0:
                    PTt, recips = open_A(j)
                if st == 0:
                    Qp, KVp, cst = Qp0, KVp0, cst0
                elif st == 1:
                    Qp, KVp, cst = Qp1, KVp1, cst1
                else:
                    cst = cspool.tile([P, EW], P_DT, tag="cs")
                    nc.sync.dma_start(out=cst, in_=cs[st * P : (st + 1) * P, :])
                    Qp = qps.tile([P, EW], f32, tag="Qp")
                    KVp = kvsh.tile([P, 2 * D], f32, tag="KVp")
                for g in range(DD // XG if st >= 2 else 0):
                    xc = xpool.tile([P, XG, 2, P], f8, tag="xc")
                    nc.sync.dma_start(
                        out=xc, in_=xTc[:, st, g * XG : (g + 1) * XG, :, :]
                    )
                    qkv_terms(Qp, KVp, xc, g)
                    # one attention row between x-chunk groups keeps ACT fed
                    # while PE grinds the projection matmuls
                    if j >= 0:
                        rowA(j, g, PTt, recips)

                # free the Q/KV PSUM banks fast: one copy each, rope reads SBUF
                qsb = rpool.tile([P, EW], f32, tag="qsb")
                nc.vector.tensor_copy(out=qsb, in_=Qp)
                kvsb = rpool.tile([P, 2 * D], f32, tag="kvsb")
                nc.vector.tensor_copy(out=kvsb, in_=KVp)

                def ttr_ew(out_, in0, in1, op):
                    nc.vector.tensor_tensor(out=out_, in0=in0, in1=in1, op=op)

                HF = EW // 2
                rq = rpool.tile([P, EW], P_DT, tag="rq")
                t1 = rpool.tile([P, HF], f32, tag="t1")
                t2 = rpool.tile([P, HF], f32, tag="t2")
                q_ev, q_od = qsb[:, 0:EW:2], qsb[:, 1:EW:2]
                cosr, sinr = cst[:, 0:HF], cst[:, HF : 2 * HF]
                ttr_ew(t1, q_ev, cosr, A_.mult)
                ttr_ew(t2, q_od, sinr, A_.mult)
                ttr_ew(rq[:, 0:EW:2], t1, t2, A_.subtract)
                ttr_ew(t1, q_ev, sinr, A_.mult)
                ttr_ew(t2, q_od, cosr, A_.mult)
                ttr_ew(rq[:, 1:EW:2], t1, t2, A_.add)

                rk = rpool.tile([P, D], P_DT, tag="rk")
                k_ev, k_od = kvsb[:, 0:D:2], kvsb[:, 1:D:2]
                cosk, sink = cst[:, 0 : D // 2], cst[:, HF : HF + D // 2]
                ttr_ew(t1[:, 0 : D // 2], k_ev, cosk, A_.mult)
                ttr_ew(t2[:, 0 : D // 2], k_od, sink, A_.mult)
                ttr_ew(rk[:, 0:D:2], t1[:, 0 : D // 2], t2[:, 0 : D // 2], A_.subtract)
                ttr_ew(t1[:, 0 : D // 2], k_ev, sink, A_.mult)
                ttr_ew(t2[:, 0 : D // 2], k_od, cosk, A_.mult)
                ttr_ew(rk[:, 1:D:2], t1[:, 0 : D // 2], t2[:, 0 : D // 2], A_.add)

                nc.scalar.activation(
                    out=Vt[:, st, :],
                    in_=kvsb[:, D : 2 * D],
                    func=AF.Copy,
                    scale=float(1.0 / WS1),
                )

                # transposes into [d, s] layouts via the DMA xbar: frees the
                # PE/DVE cycles and the TT PSUM bank
                for h in range(NH):
                    nc.sync.dma_start_transpose(
                        out=QTt[:, h, st * P : (st + 1) * P],
                        in_=rq[:, h * P : (h + 1) * P],
                    )
                nc.sync.dma_start_transpose(
                    out=KTt[:, st * P : (st + 1) * P], in_=rk
                )

                if j >= 1:
                    stage_B(j - 1)
                if st >= 12:
                    drip_w0(st - 12)

        # ---------------- tail: qs=3 blocks + output projection ----------------
        with (
            tc.tile_pool(name="wopool", bufs=2) as wopool,
            tc.tile_pool(name="osb", bufs=2) as osb,
            tc.tile_pool(name="ops", bufs=3, space="PSUM") as ops,
        ):
            wot_tiles = {}
            wqueue = []
            units = {}
            wohr = woh.rearrange("(t p) m -> p t m", p=P)
            wolr = wol.rearrange("(t p) m -> p t m", p=P)

            def alloc_wot(mc):
                wth = wopool.tile([P, JT, 512], f8, tag="woh")
                wtl = wopool.tile([P, JT, 512], f8, tag="wol")
                wot_tiles[mc] = (wth, wtl)
                # quarter q covers t-pairs [4q, 4q+4); hi then lo
                for qq in range(4):
                    wqueue.append((mc, qq, 0))
                    wqueue.append((mc, qq, 1))

            def drip(n):
                for _ in range(min(n, len(wqueue))):
                    mc, qq, lo = wqueue.pop(0)
                    wt = wot_tiles[mc][lo]
                    src = wolr if lo else wohr
                    nc.sync.dma_start(
                        out=wt[:, qq * (JT // 4) : (qq + 1) * (JT // 4), :],
                        in_=src[
                            :,
                            qq * (JT // 4) : (qq + 1) * (JT // 4),
                            mc * 512 : (mc + 1) * 512,
                        ],
                    )

            def load_wot(mc):
                alloc_wot(mc)
                drip(8)

            def unit_mms(mc, it, tps, start, stop):
                wth, wtl = wot_tiles[mc]
                if (mc, it) in units:
                    O = units[(mc, it)]
                else:
                    O = ops.tile([P, 512], f32, tag="O")
                    units[(mc, it)] = O
                for n, tp in enumerate(tps):
                    lh = Aall_h[it][:, 2 * tp * P : (2 * tp + 2) * P].rearrange(
                        "a (two s) -> a two s", two=2
                    )
                    ll = Aall_l[it][:, 2 * tp * P : (2 * tp + 2) * P].rearrange(
                        "a (two s) -> a two s", two=2
                    )
                    rh = wth[:, 2 * tp : 2 * tp + 2, :]
                    rl = wtl[:, 2 * tp : 2 * tp + 2, :]
                    for k, (lhsT, rhs) in enumerate(((lh, rh), (lh, rl), (ll, rh))):
                        nc.tensor.matmul(
                            O,
                            lhsT,
                            rhs,
                            start=(start and n == 0 and k == 0),
                            stop=(stop and n == len(tps) - 1 and k == 2),
                            perf_mode=DR,
                        )

            def unit_fin(mc, it):
                O = units.pop((mc, it))
                Ot = osb.tile([P, 512], P_DT, tag="Ot")
                nc.scalar.activation(
                    out=Ot, in_=O, func=AF.Copy, scale=float(1.0 / WS3)
                )
                nc.sync.dma_start(
                    out=out[it * P : (it + 1) * P, mc * 512 : (mc + 1) * 512],
                    in_=Ot,
                )

            def unit(mc, it):
                unit_mms(mc, it, range(ST), True, True)
                unit_fin(mc, it)

            wot_tiles[0] = (w0h, w0l)  # prefetched during the interleave
            alloc_wot(1)
            alloc_wot(2)
            # qs=3 attention blocks (need all 16 s-tiles), pipelined; wo
            # chunk loads drip between rows so they never block the
            # latency-critical P transposes on the DMA engines. Phase-3
            # units split: t0-7 accumulation only needs qs<=1 heads (final
            # long before the tail), t8-15 needs the qs=3 heads.
            HALF1, HALF2 = range(0, 8), range(8, 16)
            # these two first-half units depend only on qs<=1 heads (done
            # mid-interleave) and the prefetched mc0 chunk: they fill the
            # PE idle at tail start
            unit_mms(0, 0, HALF1, True, False)
            unit_mms(0, 1, HALF1, True, False)
            PTt, recips = open_A(12)
            for qi in range(4):
                rowA(12, qi, PTt, recips)
                drip(2)
            stage_B(11)
            for j in (13, 14, 15):
                PTt, recips = open_A(j)
                for qi in range(4):
                    rowA(j, qi, PTt, recips)
                    drip(2)
                stage_B(j - 1)
                if j == 13:
                    unit_mms(1, 0, HALF1, True, False)
                elif j == 14:
                    # Aall[0] complete after B(13)
                    unit_mms(0, 0, HALF2, False, True)
                    unit_fin(0, 0)
                    unit_mms(2, 0, HALF1, True, False)
                elif j == 15:
                    unit_mms(1, 0, HALF2, False, True)
                    unit_fin(1, 0)
            stage_B(15)
            drip(len(wqueue))
            unit_mms(2, 0, HALF2, False, True)
            unit_fin(2, 0)
            unit_mms(0, 1, HALF2, False, True)
            unit_fin(0, 1)
            # preloaded chunks' it=1 work covers the in-flight loads of the
            # later chunks (slot for mc+2 frees as soon as mc's last unit
            # is emitted)
            def load_wot_pre(mc):
                # rotate the wopre slot (mc0's chunk is consumed by now)
                wth = wopre.tile([P, JT, 512], f8, tag="wph")
                wtl = wopre.tile([P, JT, 512], f8, tag="wpl")
                wot_tiles[mc] = (wth, wtl)
                for qq in range(4):
                    wqueue.append((mc, qq, 0))
                    wqueue.append((mc, qq, 1))
                drip(8)

            def unit_split_cols(mc, it):
                # last unit: two column-halves so the closing copy/store
                # overlaps the second half's matmuls
                wth, wtl = wot_tiles[mc]
                for half in range(2):
                    O = ops.tile([P, 512], f32, tag="O")
                    cl, ch = half * 256, (half + 1) * 256
                    for n, tp in enumerate(range(ST)):
                        lh = Aall_h[it][:, 2 * tp * P : (2 * tp + 2) * P].rearrange(
                            "a (two s) -> a two s", two=2
                        )
                        ll = Aall_l[it][:, 2 * tp * P : (2 * tp + 2) * P].rearrange(
                            "a (two s) -> a two s", two=2
                        )
                        rh = wth[:, 2 * tp : 2 * tp + 2, cl:ch]
                        rl = wtl[:, 2 * tp : 2 * tp + 2, cl:ch]
                        for k, (lhsT, rhs) in enumerate(
                            ((lh, rh), (lh, rl), (ll, rh))
                        ):
                            nc.tensor.matmul(
                                O[:, 0:256],
                                lhsT,
                                rhs,
                                start=(n == 0 and k == 0),
                                stop=(n == ST - 1 and k == 2),
                                perf_mode=DR,
                            )
                    Ot = osb.tile([P, 512], P_DT, tag="Ot")
                    nc.scalar.activation(
                        out=Ot[:, 0:256],
                        in_=O[:, 0:256],
                        func=AF.Copy,
                        scale=float(1.0 / WS3),
                    )
                    nc.sync.dma_start(
                        out=out[
                            it * P : (it + 1) * P,
                            mc * 512 + cl : mc * 512 + ch,
                        ],
                        in_=Ot[:, 0:256],
                    )

            for mc in (1, 2, 3, 4, 5, 6, 7):
                if mc >= 3:
                    unit(mc, 0)
                if mc == 7:
                    unit_split_cols(mc, 1)
                else:
                    unit(mc, 1)
                wot_tiles.pop(mc)
                nxt = mc + 2 if mc >= 3 else {1: 3, 2: 4}.get(mc)
                if nxt is not None and nxt < MC and nxt not in wot_tiles:
                    load_wot(nxt)
                if mc == 2:
                    load_wot(5)

    nc.compile()
    return nc


def is_pure_causal(mask, SEQ):
    """True iff mask[i,j] == 0 for j<=i and <= NEG_THRESH for j>i."""
    m = np.asarray(mask, np.float32)
    if m.shape != (SEQ, SEQ):
        return False
    j = np.arange(SEQ)
    allowed = j[None, :] <= j[:, None]
    return bool((m[allowed] == 0).all() and (m[~allowed] <= NEG_THRESH).all())


def make_rope_tables(cos_freq, sin_freq, SEQ, scale_quarter):
    cos_t = np.tile(np.asarray(cos_freq, np.float32) * scale_quarter, (1, NH))
    sin_t = np.tile(np.asarray(sin_freq, np.float32) * scale_quarter, (1, NH))
    return np.ascontiguousarray(
        np.concatenate([cos_t, sin_t], axis=1).astype(np.float32)
    )




_BUILD_CACHE = {}


def kernel(
    x,
    cos_freq,
    sin_freq,
    positions,
    mask,
    wq,
    wk,
    wv,
    wo,
    _trace=False,
):
    import sys

    if "/opt/trn_rl_repo" not in sys.path:
        sys.path.insert(0, "/opt/trn_rl_repo")
    from concourse.bass_utils import run_bass_kernel_spmd
    import ml_dtypes

    x = np.asarray(x, np.float32)
    mask = np.asarray(mask, np.float32)
    wq = np.asarray(wq, np.float32)
    wk = np.asarray(wk, np.float32)
    wv = np.asarray(wv, np.float32)
    wo = np.asarray(wo, np.float32)
    SEQ, DIM = x.shape
    assert wq.shape[0] == CORES * NH * D and wk.shape[0] == CORES * D
    assert 2 * SEQ == wq.shape[0], "flatten structure requires H*D == 2*SEQ"

    bf16 = ml_dtypes.bfloat16
    f8 = ml_dtypes.float8_e4m3
    ST_, DD_ = SEQ // P, DIM // P

    if is_pure_causal(mask, SEQ):
        key = (SEQ, DIM, "causal")
        if key not in _BUILD_CACHE:
            _BUILD_CACHE[key] = build_attention_v7(SEQ, DIM)
        nc = _BUILD_CACHE[key]

        def hilo(a):
            hi = np.ascontiguousarray(a).astype(f8)
            lo = np.ascontiguousarray(a - hi.astype(np.float32)).astype(f8)
            return hi, lo

        # fold sqrt(scale) and the 1/WS1 weight pre-scale into rope tables
        scale_quarter = np.float32(D ** -0.25 / WS1)
        cs = make_rope_tables(cos_freq, sin_freq, SEQ, scale_quarter).astype(bf16)
        xT = np.ascontiguousarray(x.reshape(ST_, P, DD_, P).transpose(3, 0, 2, 1))
        xh_, xl_ = hilo(xT)
        xTc = np.ascontiguousarray(np.stack([xh_, xl_], axis=3))
        # wo row-blocks permuted so DoubleRow contraction pairs are adjacent
        JT_ = 2 * SEQ // P
        perm = [(jt % 2) * (JT_ // 2) + jt // 2 for jt in range(JT_)]
        woP = np.ascontiguousarray(
            (wo.T * np.float32(WS3)).reshape(JT_, P, DIM)[perm].reshape(2 * SEQ, DIM)
        )
        woh, wol = hilo(woP)
        tri = np.ascontiguousarray(mask[0:P, 0:P])

        in_maps = []
        for c in range(CORES):
            w_c = np.concatenate(
                [
                    wq[c * NH * D : (c + 1) * NH * D],
                    wk[c * D : (c + 1) * D],
                    wv[c * D : (c + 1) * D],
                ],
                axis=0,
            )
            wTh_, wTl_ = hilo(w_c.T * np.float32(WS1))
            in_maps.append(
                {
                    "xTc": xTc,
                    "wTh": wTh_,
                    "wTl": wTl_,
                    "cs": cs,
                    "tri": tri,
                    "woh": woh,
                    "wol": wol,
                }
            )
        res = run_bass_kernel_spmd(nc, in_maps, list(range(CORES)), trace=_trace)
        outp = np.concatenate(
            [np.asarray(res.results[c]["out"]) for c in range(CORES)], axis=0
        ).astype(np.float32)
        if _trace:
            return outp, res
        return outp

    # ---------------- general-mask fallback (v1 kernel) ----------------
    plan, blocks = analyze_mask(mask, SEQ)
    n_uniq = len(blocks)
    key = (SEQ, DIM, tuple(tuple(r) for r in plan))
    if key not in _BUILD_CACHE:
        _BUILD_CACHE[key] = build_attention_nc(SEQ, DIM, plan, n_uniq)
    nc = _BUILD_CACHE[key]

    scale_quarter = np.float32(D ** -0.25)
    csf = make_rope_tables(cos_freq, sin_freq, SEQ, scale_quarter)
    xT = np.ascontiguousarray(
        x.reshape(ST_, P, DD_, P).transpose(3, 0, 2, 1)
    ).astype(bf16)
    woT = np.ascontiguousarray(wo.T).astype(bf16)
    if n_uniq:
        mbs = np.ascontiguousarray(np.stack(blocks, axis=0))
    else:
        mbs = np.zeros((1, P, 512), np.float32)

    in_maps = []
    for c in range(CORES):
        w_c = np.concatenate(
            [
                wq[c * NH * D : (c + 1) * NH * D],
                wk[c * D : (c + 1) * D],
                wv[c * D : (c + 1) * D],
            ],
            axis=0,
        )
        in_maps.append(
            {
                "xT": xT,
                "wT": np.ascontiguousarray(w_c.T).astype(bf16),
                "cs": csf,
                "maskb": mbs,
                "woT": woT,
            }
        )
    res = run_bass_kernel_spmd(nc, in_maps, list(range(CORES)), trace=_trace)
    outp = np.concatenate(
        [np.asarray(res.results[c]["out"]) for c in range(CORES)], axis=0
    ).astype(np.float32)
    if _trace:
        return outp, res
    return outp


# revision 10
# speedup vs baseline: 1.0197x; 1.0017x over previous
"""Trainium2 Bass kernel for nn_Attention (GQA + RoPE + sliding-window mask).

Sharding: tensor-parallel over heads across 8 cores (4 q heads + 1 kv head
per core). The reference's quirky output flatten ((H,S,D)->(H,D,S)->
reshape(S, H*D)) makes the final projection row-shard by head block: core c
produces rows [256c, 256c+256) of the (2048, 4096) result with no collective.

Fast path (pure causal mask, the shape this problem produces):
  * phase 1 (QKV projections) and phase 3 (output projection) run as
    fp8-e4m3 hi/lo pairs in DoubleRow perf mode: X @ W ~= Xh@Wh + Xh@Wl +
    Xl@Wh with Xh = fp8(X), Xl = fp8(X - Xh) - 0.75x the PE time of one
    bf16 pass and more accurate than bf16 (~9-10 effective mantissa bits).
  * attention (phase 2) is interleaved INTO phase 1: block (qs, h) only
    needs s-tiles <= 4qs+3, so blocks weave between projection s-tiles with
    score rows emitted between x-chunk groups - softmax (ACT) latency hides
    under the projection matmuls.
  * no-max softmax (causal logits here are O(10), exp is safe in fp32),
    exp reads score PSUM directly with accum_out row sums; only the
    128-wide triangular diagonal block gets a mask add; diagonal PV
    matmuls are narrowed to the live query columns.
  * P transposed via DMA xbar; wo chunks prefetched/dripped so the big
    loads never head-of-line block the latency-critical transposes.

Fallback (any other mask): the v1 kernel (per-chunk mask add + 2-pass
max/exp softmax), correct for arbitrary additive masks.
"""

import numpy as np
from contextlib import ExitStack

P = 128
D = 128  # head dim
NH = 4   # q heads per core
CORES = 8
NEG_THRESH = -1e8


def build_attention_nc(
    SEQ,
    DIM,
    plan,
    n_uniq,
    p_dt_name="bfloat16",
    wo_dt_name="bfloat16",
    proj_dt_name="bfloat16",
    proj_f32r=True,
    score_f32r=True,
    use_dma_t=True,
):
    """Build the per-core Bass program.

    plan: list over q-tiles i (SEQ//128 entries) of lists of (chunk_idx, uid)
          where uid == -1 means the 512-wide chunk needs no mask add, else the
          index into the maskb tensor. Chunks absent from the list are fully
          masked (skipped).
    """
    import concourse.bass as bass
    import concourse.bacc as bacc
    import concourse.mybir as mybir
    import concourse.tile as tile
    from concourse.masks import make_identity

    f32 = mybir.dt.float32
    f32r = mybir.dt.float32r
    P_DT = getattr(mybir.dt, p_dt_name)
    WO_DT = getattr(mybir.dt, wo_dt_name)
    PJ_DT = getattr(mybir.dt, proj_dt_name)
    pj_f32r = proj_f32r and proj_dt_name == "float32"

    ST = SEQ // P          # 16 s-tiles
    DD = DIM // P          # 32 contraction tiles
    KC = SEQ // 512        # 4 key chunks
    QS = SEQ // 512        # 4 query supers
    EW = NH * D            # 512 q-projection width
    JT = 2 * SEQ // P      # 32 j-tiles for final matmul
    MC = DIM // 512        # 8 output chunks
    ITILES = (NH * 64) // P  # 2 output row tiles
    assert NH == 4 and SEQ % 512 == 0 and DIM % 512 == 0

    def mm_cast(ap, use_r):
        return ap.bitcast(f32r) if use_r else ap

    nc = bacc.Bacc(trn_type="TRN2", debug=False, num_devices=CORES)

    # x pre-tiled on host: xT[p, st, t, si] = x[st*128+si, t*128+p] so each
    # streamed chunk is one DMA with 2KB contiguous per-partition runs
    xT = nc.dram_tensor("xT", [P, ST, DD, P], PJ_DT, kind="ExternalInput").ap()
    wT = nc.dram_tensor("wT", [DIM, EW + 2 * D], PJ_DT, kind="ExternalInput").ap()
    cs = nc.dram_tensor("cs", [SEQ, EW], f32, kind="ExternalInput").ap()
    mb = nc.dram_tensor(
        "maskb", [max(n_uniq, 1), P, 512], f32, kind="ExternalInput"
    ).ap()
    woT = nc.dram_tensor("woT", [2 * SEQ, DIM], WO_DT, kind="ExternalInput").ap()
    out = nc.dram_tensor("out", [NH * 64, DIM], f32, kind="ExternalOutput").ap()

    with tile.TileContext(nc) as tc, ExitStack() as ctx:
        const = ctx.enter_context(tc.tile_pool(name="const", bufs=1))
        idF = const.tile([P, P], f32)
        make_identity(nc, idF)
        idP = const.tile([P, P], P_DT)
        make_identity(nc, idP)
        zeros = const.tile([P, 512], f32)
        nc.vector.memset(zeros, 0.0)

        pers = ctx.enter_context(tc.tile_pool(name="pers", bufs=1))
        QTt = pers.tile([P, NH, ST * P], f32)   # [d, h, s]
        KTt = pers.tile([P, ST * P], f32)       # [d, s]
        Vt = pers.tile([P, ST, D], P_DT)        # [k(part), ktile, d]
        if n_uniq > 0:
            mbt = pers.tile([P, n_uniq, 512], f32)

        # ---------------- phase 1: projections + rope + layout ----------------
        with (
            tc.tile_pool(name="wpool", bufs=1) as wpool,
            tc.tile_pool(name="xpool", bufs=6) as xpool,
            tc.tile_pool(name="cspool", bufs=2) as cspool,
            tc.tile_pool(name="rpool", bufs=2) as rpool,
            tc.tile_pool(name="qps", bufs=2, space="PSUM") as qps,
            tc.tile_pool(name="kvps", bufs=2, space="PSUM") as kvps,
            tc.tile_pool(name="tps", bufs=2, space="PSUM") as tps,
            tc.tile_pool(name="t2ps", bufs=2, space="PSUM") as t2ps,
        ):
            XGW = min(8, DD)
            wTt = wpool.tile([P, DD, EW + 2 * D], PJ_DT)
            wTr = wT.rearrange("(t p) e -> p t e", p=P)

            XG = min(8, DD)  # dd-tiles per streamed x chunk
            NG = DD // XG
            xTr = xT
            # Interleave the weight-chunk loads with s-tile 0's x chunks so
            # the first matmuls start as soon as chunk 0 of each lands.
            st0_x = []
            for g in range(NG):
                xTt = xpool.tile([P, XG, P], PJ_DT, tag="xT")
                nc.sync.dma_start(
                    out=xTt, in_=xTr[:, 0, g * XG : (g + 1) * XG, :]
                )
                st0_x.append(xTt)
                gw = g % (DD // XGW)
                nc.sync.dma_start(
                    out=wTt[:, gw * XGW : (gw + 1) * XGW, :],
                    in_=wTr[:, gw * XGW : (gw + 1) * XGW, :],
                )
            for st in range(ST):
                cst = cspool.tile([P, EW], f32, tag="cs")
                nc.sync.dma_start(out=cst, in_=cs[st * P : (st + 1) * P, :])

                Qp = qps.tile([P, EW], f32, tag="Qp")
                KVp = kvps.tile([P, 2 * D], f32, tag="KVp")
                for g in range(DD // XG):
                    if st == 0:
                        xTt = st0_x[g]
                    else:
                        xTt = xpool.tile([P, XG, P], PJ_DT, tag="xT")
                        nc.sync.dma_start(
                            out=xTt,
                            in_=xTr[:, st, g * XG : (g + 1) * XG, :],
                        )
                    for tt in range(XG):
                        t = g * XG + tt
                        lhsT = mm_cast(xTt[:, tt, :], pj_f32r)
                        nc.tensor.matmul(
                            Qp,
                            lhsT,
                            mm_cast(wTt[:, t, 0:EW], pj_f32r),
                            start=(t == 0),
                            stop=(t == DD - 1),
                        )
                        nc.tensor.matmul(
                            KVp,
                            lhsT,
                            mm_cast(wTt[:, t, EW : EW + 2 * D], pj_f32r),
                            start=(t == 0),
                            stop=(t == DD - 1),
                        )

                # rope via strided even/odd halves (2-level APs only — 3-level
                # APs overflow the fixed ISA instruction encoding).
                # tensor_tensor_reduce instead of tensor_tensor: the plain TT
                # ISA struct has a single sync-wait slot and walrus codegen
                # rejects the PE+DMA double wait Tile emits here; the TTR/ISA
                # struct carries up to 8. accum outputs are dummies.
                def ttr_ew(out, in0, in1, op):
                    nc.vector.tensor_tensor(out=out, in0=in0, in1=in1, op=op)

                A_ = mybir.AluOpType
                HF = EW // 2  # 256: cos table width for q
                rq = rpool.tile([P, EW], f32, tag="rq")
                t1 = rpool.tile([P, HF], f32, tag="t1")
                t2 = rpool.tile([P, HF], f32, tag="t2")
                q_ev, q_od = Qp[:, 0:EW:2], Qp[:, 1:EW:2]
                cosr, sinr = cst[:, 0:HF], cst[:, HF : 2 * HF]
                ttr_ew(t1, q_ev, cosr, A_.mult)
                ttr_ew(t2, q_od, sinr, A_.mult)
                ttr_ew(rq[:, 0:EW:2], t1, t2, A_.subtract)
                ttr_ew(t1, q_ev, sinr, A_.mult)
                ttr_ew(t2, q_od, cosr, A_.mult)
                ttr_ew(rq[:, 1:EW:2], t1, t2, A_.add)

                rk = rpool.tile([P, D], f32, tag="rk")
                k_ev, k_od = KVp[:, 0:D:2], KVp[:, 1:D:2]
                cosk, sink = cst[:, 0 : D // 2], cst[:, HF : HF + D // 2]
                ttr_ew(t1[:, 0 : D // 2], k_ev, cosk, A_.mult)
                ttr_ew(t2[:, 0 : D // 2], k_od, sink, A_.mult)
                ttr_ew(rk[:, 0:D:2], t1[:, 0 : D // 2], t2[:, 0 : D // 2], A_.subtract)
                ttr_ew(t1[:, 0 : D // 2], k_ev, sink, A_.mult)
                ttr_ew(t2[:, 0 : D // 2], k_od, cosk, A_.mult)
                ttr_ew(rk[:, 1:D:2], t1[:, 0 : D // 2], t2[:, 0 : D // 2], A_.add)

                # V -> bf16 [k, d] layout (ACT copy, cast)
                nc.scalar.activation(
                    out=Vt[:, st, :],
                    in_=KVp[:, D : 2 * D],
                    func=mybir.ActivationFunctionType.Copy,
                )

                # transpose rq (per head) and rk into [d, s] layouts
                T1 = tps.tile([P, EW], f32, tag="T1")
                for h in range(NH):
                    nc.tensor.transpose(
                        T1[:, h * P : (h + 1) * P], rq[:, h * P : (h + 1) * P], idF
                    )
                # write as f32r so walrus accepts them as f32r matmul operands
                nc.vector.tensor_copy(
                    out=mm_cast(QTt[:, :, st * P : (st + 1) * P], score_f32r),
                    in_=T1.rearrange("p (h s) -> p h s", h=NH),
                )
                T2 = t2ps.tile([P, P], f32, tag="T2")
                nc.tensor.transpose(T2, rk, idF)
                nc.vector.tensor_copy(
                    out=mm_cast(KTt[:, st * P : (st + 1) * P], score_f32r), in_=T2
                )

        # ---------------- phase 2: attention ----------------
        if n_uniq > 0:
            nc.sync.dma_start(out=mbt, in_=mb.rearrange("u p m -> p u m"))
        apool = ctx.enter_context(tc.tile_pool(name="apool", bufs=1))
        # split by head-pair so phase 3's first row-tile can start once
        # heads 0-1 finish, overlapping the rest of phase 2
        Aall = [
            apool.tile([P, 2 * ST * D], P_DT, name=f"Aall{i}")
            for i in range(NH // 2)
        ]
        with (
            tc.tile_pool(name="ptsb", bufs=2) as ptsb,
            tc.tile_pool(name="spool", bufs=6) as spool,
            tc.tile_pool(name="ppool", bufs=4) as ppool,
            tc.tile_pool(name="stat", bufs=12) as stat,
            tc.tile_pool(name="atsb", bufs=3) as atsb,
            tc.tile_pool(name="sps", bufs=2, space="PSUM") as sps,
            tc.tile_pool(name="ptps", bufs=2, space="PSUM") as ptps,
            tc.tile_pool(name="atps", bufs=1, space="PSUM") as atps,
            tc.tile_pool(name="aps", bufs=1, space="PSUM") as aps,
            tc.tile_pool(name="wopool", bufs=2 if n_uniq <= 4 else 1) as wopool,
            tc.tile_pool(name="osb", bufs=2) as osb,
            tc.tile_pool(name="ops", bufs=3, space="PSUM") as ops,
        ):
            for h in range(NH):
                for qs in range(QS):
                    PTt = ptsb.tile([P, ST, 512], P_DT, tag="PT")
                    kts_used = set()
                    recips = []
                    pt_written = set()
                    for qi in range(4):
                        i = 4 * qs + qi
                        row = plan[i]
                        if not row:
                            recips.append(None)
                            continue
                        pairs = [row[k : k + 2] for k in range(0, len(row), 2)]
                        stats = stat.tile([P, KC], f32, tag="stats")
                        ncols = 0
                        S_tiles = []
                        for pr in pairs:
                            W = 512 * len(pr)
                            S = sps.tile([P, 1024], f32, tag="S")
                            Ssb = spool.tile([P, 1024], f32, tag="Ssb")
                            masked_any = any(uid >= 0 for (_, uid) in pr)
                            for k, (c, uid) in enumerate(pr):
                                sl = S[:, k * 512 : (k + 1) * 512]
                                nc.tensor.matmul(
                                    sl,
                                    mm_cast(
                                        QTt[:, h, i * P : (i + 1) * P], score_f32r
                                    ),
                                    mm_cast(
                                        KTt[:, c * 512 : (c + 1) * 512], score_f32r
                                    ),
                                    start=True,
                                    stop=True,
                                )
                                if uid >= 0:
                                    nc.vector.tensor_add(sl, sl, mbt[:, uid, :])
                                # copy PSUM->SBUF to free the score bank early;
                                # alternate DVE/ACT to balance engine load
                                dst = Ssb[:, k * 512 : (k + 1) * 512]
                                if (i + k) % 2 == 0:
                                    nc.vector.tensor_copy(out=dst, in_=sl)
                                else:
                                    nc.scalar.activation(
                                        out=dst,
                                        in_=sl,
                                        func=mybir.ActivationFunctionType.Copy,
                                    )
                                if masked_any or len(pr) == 1:
                                    nc.vector.tensor_reduce(
                                        out=stats[:, ncols : ncols + 1],
                                        in_=dst,
                                        axis=mybir.AxisListType.X,
                                        op=mybir.AluOpType.max,
                                    )
                                    ncols += 1
                            if not masked_any and len(pr) == 2:
                                # one pair-wide max over both chunks (SBUF 2x)
                                nc.vector.tensor_reduce(
                                    out=stats[:, ncols : ncols + 1],
                                    in_=Ssb,
                                    axis=mybir.AxisListType.X,
                                    op=mybir.AluOpType.max,
                                )
                                ncols += 1
                            S_tiles.append((Ssb, pr))
                        negm = stat.tile([P, 1], f32, tag="negm")
                        nc.vector.tensor_reduce(
                            out=negm,
                            in_=stats[:, 0:ncols],
                            axis=mybir.AxisListType.X,
                            op=mybir.AluOpType.max,
                            negate=True,
                        )
                        sums = stat.tile([P, KC], f32, tag="sums")
                        for k, (Sk, pr) in enumerate(S_tiles):
                            W = 512 * len(pr)
                            Pt = ppool.tile([P, 1024], P_DT, tag="P")
                            nc.scalar.activation(
                                out=Pt[:, 0:W],
                                in_=Sk[:, 0:W],
                                func=mybir.ActivationFunctionType.Exp,
                                bias=negm,
                                accum_out=sums[:, k : k + 1],
                            )
                            # transpose P [q, k] -> PT [k, q]
                            for j, (c, uid) in enumerate(pr):
                                if use_dma_t:
                                    nc.sync.dma_start_transpose(
                                        out=PTt[
                                            :, 4 * c : 4 * c + 4, qi * P : (qi + 1) * P
                                        ],
                                        in_=Pt[:, j * 512 : (j + 1) * 512],
                                    )
                                else:
                                    PTp = ptps.tile([P, 512], P_DT, tag="PTp")
                                    for jj in range(4):
                                        nc.tensor.transpose(
                                            PTp[:, jj * P : (jj + 1) * P],
                                            Pt[:, j * 512 + jj * P : j * 512 + (jj + 1) * P],
                                            idP,
                                        )
                                    nc.vector.tensor_copy(
                                        out=PTt[:, 4 * c : 4 * c + 4, qi * P : (qi + 1) * P],
                                        in_=PTp.rearrange("p (kt q) -> p kt q", kt=4),
                                    )
                                for jj in range(4):
                                    kts_used.add(4 * c + jj)
                                    pt_written.add((4 * c + jj, qi))
                        denom = stat.tile([P, 1], f32, tag="denom")
                        nc.vector.tensor_reduce(
                            out=denom,
                            in_=sums[:, 0 : len(S_tiles)],
                            axis=mybir.AxisListType.X,
                            op=mybir.AluOpType.add,
                        )
                        recip = stat.tile([P, 1], f32, tag="recip")
                        nc.vector.reciprocal(recip, denom)
                        recips.append(recip)

                    # zero-fill PT holes (only for non-causal masks)
                    kts = sorted(kts_used)
                    for kt in kts:
                        for qi in range(4):
                            if (kt, qi) not in pt_written and recips[qi] is not None:
                                nc.vector.memset(
                                    PTt[:, kt, qi * P : (qi + 1) * P], 0.0
                                )
                            elif recips[qi] is None:
                                nc.vector.memset(
                                    PTt[:, kt, qi * P : (qi + 1) * P], 0.0
                                )

                    if not kts:
                        continue
                    # PV: A^T[d, q] accumulated over key tiles
                    At = atps.tile([P, 512], f32, tag="At")
                    for n, kt in enumerate(kts):
                        nc.tensor.matmul(
                            At,
                            Vt[:, kt, :],
                            PTt[:, kt, :],
                            start=(n == 0),
                            stop=(n == len(kts) - 1),
                        )
                    Atsb = atsb.tile([P, 512], P_DT, tag="Atsb")
                    nc.vector.tensor_copy(out=Atsb, in_=At)
                    Ap = aps.tile([P, 512], P_DT, tag="Ap")
                    for qi in range(4):
                        nc.tensor.transpose(
                            Ap[:, qi * P : (qi + 1) * P],
                            Atsb[:, qi * P : (qi + 1) * P],
                            idP,
                        )
                    # Aall layout: [sp, (t*2 + dd)*128 + hb*64 + p] so the final
                    # matmul's stationary slices are contiguous (walrus requires
                    # a single free dim on weight APs)
                    Ah = Aall[h // 2]
                    hb = h % 2
                    for qi in range(4):
                        i = 4 * qs + qi
                        # dview[sp, p, dd] == Ah[:, i*256 + dd*128 + hb*64 + p]
                        dview = Ah[:, i * 2 * P : (i + 1) * 2 * P].rearrange(
                            "a (dd j) -> a dd j", dd=2
                        )[:, :, hb * 64 : hb * 64 + 64].rearrange(
                            "a dd p -> a p dd"
                        )
                        if recips[qi] is None:
                            nc.vector.memset(dview, 0.0)
                            continue
                        nc.scalar.activation(
                            out=dview,
                            in_=Ap[:, qi * P : (qi + 1) * P].rearrange(
                                "a (p two) -> a p two", two=2
                            ),
                            func=mybir.ActivationFunctionType.Copy,
                            scale=recips[qi],
                        )

            # ---------------- phase 3: output projection ----------------
            for mc in range(MC):
                wot = wopool.tile([P, JT, 512], WO_DT, tag="wo")
                nc.sync.dma_start(
                    out=wot,
                    in_=woT[:, mc * 512 : (mc + 1) * 512].rearrange(
                        "(t p) m -> p t m", p=P
                    ),
                )
                for it in range(ITILES):
                    O = ops.tile([P, 512], f32, tag="O")
                    Av = Aall[it]
                    for jt in range(JT):
                        ddj, t = jt // ST, jt % ST
                        lhsT = Av[:, (t * 2 + ddj) * P : (t * 2 + ddj + 1) * P]
                        nc.tensor.matmul(
                            O,
                            lhsT,
                            wot[:, jt, :],
                            start=(jt == 0),
                            stop=(jt == JT - 1),
                        )
                    Ot = osb.tile([P, 512], f32, tag="Ot")
                    nc.scalar.activation(
                        out=Ot, in_=O, func=mybir.ActivationFunctionType.Copy
                    )
                    nc.sync.dma_start(
                        out=out[it * P : (it + 1) * P, mc * 512 : (mc + 1) * 512],
                        in_=Ot,
                    )

    # Bacc.compile() legalizes sync (>=2 waits split into EventSemaphore
    # instructions — this walrus caps every instruction at ONE sync wait)
    nc.compile()
    return nc


def analyze_mask(mask, SEQ):
    """Classify 128x512 mask blocks: skip / free / masked(dedup uid)."""
    ST = SEQ // P
    KC = SEQ // 512
    uniq = {}
    blocks = []
    plan = []
    for i in range(ST):
        row = []
        for c in range(KC):
            blk = mask[i * P : (i + 1) * P, c * 512 : (c + 1) * 512]
            if (blk <= NEG_THRESH).all():
                continue
            if not blk.any():
                row.append((c, -1))
            else:
                key = blk.tobytes()
                if key not in uniq:
                    uniq[key] = len(blocks)
                    blocks.append(np.ascontiguousarray(blk))
                row.append((c, uniq[key]))
        if not row:
            # fully masked query rows: keep all chunks so softmax matches
            # the reference's uniform distribution over -1e9 logits
            for c in range(KC):
                blk = mask[i * P : (i + 1) * P, c * 512 : (c + 1) * 512]
                key = blk.tobytes()
                if key not in uniq:
                    uniq[key] = len(blocks)
                    blocks.append(np.ascontiguousarray(blk))
                row.append((c, uniq[key]))
        plan.append(row)
    return plan, blocks


WS1 = 1024.0  # host pre-scale on wq/wk/wv before fp8 (values ~0.02*N(0,1))
WS3 = 256.0   # host pre-scale on wo before fp8


def build_attention_v7(
    SEQ,
    DIM,
    p_dt_name="bfloat16",
):
    import concourse.bass as bass
    import concourse.bacc as bacc
    import concourse.mybir as mybir
    import concourse.tile as tile
    from concourse.masks import make_identity

    f32 = mybir.dt.float32
    f8 = mybir.dt.float8e4
    P_DT = getattr(mybir.dt, p_dt_name)
    A_ = mybir.AluOpType
    AF = mybir.ActivationFunctionType
    DR = mybir.MatmulPerfMode.DoubleRow

    ST = SEQ // P          # 16 s-tiles
    DD = DIM // P          # 32 contraction tiles
    QS = SEQ // 512        # 4 query supers
    EW = NH * D            # 512 q-projection width
    JT = 2 * SEQ // P      # 32 j-tiles for final matmul
    MC = DIM // 512        # 8 output chunks
    ITILES = (NH * 64) // P  # 2 output row tiles
    assert NH == 4 and SEQ % 512 == 0 and DIM % 512 == 0

    nc = bacc.Bacc(trn_type="TRN2", debug=False, num_devices=CORES)

    # x hi/lo packed per dd-tile so each DMA moves 2KB contiguous runs
    xTc = nc.dram_tensor("xTc", [P, ST, DD, 2, P], f8, kind="ExternalInput").ap()
    wTh = nc.dram_tensor("wTh", [DIM, EW + 2 * D], f8, kind="ExternalInput").ap()
    wTl = nc.dram_tensor("wTl", [DIM, EW + 2 * D], f8, kind="ExternalInput").ap()
    cs = nc.dram_tensor("cs", [SEQ, EW], P_DT, kind="ExternalInput").ap()
    tri = nc.dram_tensor("tri", [P, P], f32, kind="ExternalInput").ap()
    # wo row blocks permuted host-side: block jt' = 2t+dd <- original dd*16+t
    woh = nc.dram_tensor("woh", [2 * SEQ, DIM], f8, kind="ExternalInput").ap()
    wol = nc.dram_tensor("wol", [2 * SEQ, DIM], f8, kind="ExternalInput").ap()
    out = nc.dram_tensor("out", [NH * 64, DIM], P_DT, kind="ExternalOutput").ap()

    with tile.TileContext(nc) as tc, ExitStack() as ctx:
        const = ctx.enter_context(tc.tile_pool(name="const", bufs=1))
        idP = const.tile([P, P], P_DT)
        make_identity(nc, idP)

        pers = ctx.enter_context(tc.tile_pool(name="pers", bufs=1))
        QTt = pers.tile([P, NH, ST * P], P_DT)   # [d, h, s]
        KTt = pers.tile([P, ST * P], P_DT)       # [d, s]
        Vt = pers.tile([P, ST, D], P_DT)         # [k(part), ktile, d]
        trit = pers.tile([P, P], f32)
        nc.sync.dma_start(out=trit, in_=tri)

        # mc=0 wo chunk is prefetched during the interleaved region (the only
        # chunk SBUF has room for before the phase-1 pools close)
        wopre = ctx.enter_context(tc.tile_pool(name="wopre", bufs=1))
        w0h = wopre.tile([P, 2 * SEQ // P, 512], f8, tag="wph")
        w0l = wopre.tile([P, 2 * SEQ // P, 512], f8, tag="wpl")
        wohr_ = woh.rearrange("(t p) m -> p t m", p=P)
        wolr_ = wol.rearrange("(t p) m -> p t m", p=P)

        def drip_w0(qq):
            q4 = (2 * SEQ // P) // 4
            nc.sync.dma_start(
                out=w0h[:, qq * q4 : (qq + 1) * q4, :],
                in_=wohr_[:, qq * q4 : (qq + 1) * q4, 0:512],
            )
            nc.sync.dma_start(
                out=w0l[:, qq * q4 : (qq + 1) * q4, :],
                in_=wolr_[:, qq * q4 : (qq + 1) * q4, 0:512],
            )

        apool = ctx.enter_context(tc.tile_pool(name="apool", bufs=1))
        Aall_h = [
            apool.tile([P, 2 * ST * D], f8, name=f"Aallh{i}") for i in range(NH // 2)
        ]
        Aall_l = [
            apool.tile([P, 2 * ST * D], f8, name=f"Aalll{i}") for i in range(NH // 2)
        ]
        # attention pools live through phase 1+2 and the tail
        ptsb = ctx.enter_context(tc.tile_pool(name="ptsb", bufs=2))
        ppool = ctx.enter_context(tc.tile_pool(name="ppool", bufs=8))
        stat = ctx.enter_context(tc.tile_pool(name="stat", bufs=12))
        atsb = ctx.enter_context(tc.tile_pool(name="atsb", bufs=3))
        s1ps = ctx.enter_context(tc.tile_pool(name="s1ps", bufs=3, space="PSUM"))
        atps = ctx.enter_context(tc.tile_pool(name="atps", bufs=1, space="PSUM"))
        aps = ctx.enter_context(tc.tile_pool(name="aps", bufs=1, space="PSUM"))

        blocks = [(qs, h) for qs in range(QS) for h in range(NH)]  # j = 4qs+h
        state = {}

        def rowA(j, qi, PTt, recips):
            qs, h = blocks[j]
            i = 4 * qs + qi
            f = i // 4      # fully-allowed 512-chunks
            dsub = i % 4    # full 128-subtiles in the diagonal chunk
            sums = stat.tile([P, 4], f32, tag="sums")
            ncol = 0
            lhsQ = QTt[:, h, i * P : (i + 1) * P]
            for c in range(f):
                S1t = s1ps.tile([P, 512], f32, tag="S1")
                nc.tensor.matmul(
                    S1t,
                    lhsQ,
                    KTt[:, c * 512 : (c + 1) * 512],
                    start=True,
                    stop=True,
                )
                Pt = ppool.tile([P, 512], P_DT, tag="P1")
                nc.scalar.activation(
                    out=Pt,
                    in_=S1t,
                    func=AF.Exp,
                    accum_out=sums[:, ncol : ncol + 1],
                )
                ncol += 1
                nc.sync.dma_start_transpose(
                    out=PTt[:, 4 * c : 4 * c + 4, qi * P : (qi + 1) * P],
                    in_=Pt,
                )
            # diagonal chunk, truncated to (dsub+1)*128 columns
            w = (dsub + 1) * P
            S1t = s1ps.tile([P, 512], f32, tag="S1")
            nc.tensor.matmul(
                S1t[:, 0:w],
                lhsQ,
                KTt[:, f * 512 : f * 512 + w],
                start=True,
                stop=True,
            )
            nc.vector.tensor_add(S1t[:, dsub * P : w], S1t[:, dsub * P : w], trit)
            Pt = ppool.tile([P, 512], P_DT, tag="P1")
            nc.scalar.activation(
                out=Pt[:, 0:w],
                in_=S1t[:, 0:w],
                func=AF.Exp,
                accum_out=sums[:, ncol : ncol + 1],
            )
            ncol += 1
            nc.sync.dma_start_transpose(
                out=PTt[:, 4 * f : 4 * f + dsub + 1, qi * P : (qi + 1) * P],
                in_=Pt[:, 0:w],
            )
            # masked-out subtiles of the diagonal chunk are never read: the
            # PV matmuls for diagonal key-tiles are narrowed to the live
            # query columns instead
            denom = stat.tile([P, 1], f32, tag="denom")
            nc.vector.tensor_reduce(
                out=denom, in_=sums[:, 0:ncol], axis=mybir.AxisListType.X, op=A_.add
            )
            recip = stat.tile([P, 1], f32, tag="recip")
            nc.vector.reciprocal(recip, denom)
            recips.append(recip)

        def open_A(j):
            PTt = ptsb.tile([P, ST, 512], P_DT, tag="PT")
            recips = []
            state[j] = (PTt, recips)
            return PTt, recips

        def stage_B(j):
            qs, h = blocks[j]
            PTt, recips = state.pop(j)
            nkt = 4 * qs + 4
            At = atps.tile([P, 512], f32, tag="At")
            for n in range(nkt):
                # diagonal key-tiles only reach query columns >= off
                off = max(0, n - 4 * qs) * P
                nc.tensor.matmul(
                    At[:, off:512],
                    Vt[:, n, :],
                    PTt[:, n, off:512],
                    start=(n == 0),
                    stop=(n == nkt - 1),
                )
            Atsb = atsb.tile([P, 512], P_DT, tag="Atsb")
            nc.vector.tensor_copy(out=Atsb, in_=At)
            Ap = aps.tile([P, 512], P_DT, tag="Ap")
            for qi in range(4):
                nc.tensor.transpose(
                    Ap[:, qi * P : (qi + 1) * P],
                    Atsb[:, qi * P : (qi + 1) * P],
                    idP,
                )
            hb = h % 2

            def dv(Aarr):
                return Aarr[h // 2][:, i * 2 * P : (i + 1) * 2 * P].rearrange(
                    "a (dd j) -> a dd j", dd=2
                )[:, :, hb * 64 : hb * 64 + 64].rearrange("a dd p -> a p dd")

            for qi in range(4):
                i = 4 * qs + qi
                # normalize on DVE, then split into fp8 hi + residual lo for
                # the DoubleRow output projection
                th = atsb.tile([P, P], f32, tag="th")
                nc.vector.tensor_scalar_mul(
                    th, Ap[:, qi * P : (qi + 1) * P], recips[qi]
                )
                thv = th.rearrange("a (p two) -> a p two", two=2)
                dh, dl = dv(Aall_h), dv(Aall_l)
                nc.vector.tensor_copy(out=dh, in_=thv)
                nc.vector.tensor_tensor(out=dl, in0=thv, in1=dh, op=A_.subtract)

        # ------------- phase 1 with interleaved attention blocks -------------
        with (
            tc.tile_pool(name="wpool", bufs=1) as wpool,
            tc.tile_pool(name="xpool", bufs=8) as xpool,
            tc.tile_pool(name="cspool", bufs=2) as cspool,
            tc.tile_pool(name="rpool", bufs=2) as rpool,
            tc.tile_pool(name="qps", bufs=1, space="PSUM") as qps,
            tc.tile_pool(name="kvsh", bufs=1, space="PSUM") as kvsh,
        ):
            wTth = wpool.tile([P, DD, EW + 2 * D], f8)
            wTtl = wpool.tile([P, DD, EW + 2 * D], f8)
            wTrh = wTh.rearrange("(t p) e -> p t e", p=P)
            wTrl = wTl.rearrange("(t p) e -> p t e", p=P)

            XG = min(8, DD)  # dd-tiles per streamed x chunk
            NG = DD // XG
            # interleave s-tile-0 x chunks with weight loads (weights in 8
            # sub-loads per array so the first matmuls start early)
            st0_x = []
            for g in range(NG):
                xc = xpool.tile([P, XG, 2, P], f8, tag="xc")
                nc.sync.dma_start(out=xc, in_=xTc[:, 0, g * XG : (g + 1) * XG, :, :])
                st0_x.append(xc)
                for half in range(2):
                    gw = 2 * g + half
                    nc.sync.dma_start(
                        out=wTth[:, gw * 4 : (gw + 1) * 4, :],
                        in_=wTrh[:, gw * 4 : (gw + 1) * 4, :],
                    )
                    nc.sync.dma_start(
                        out=wTtl[:, gw * 4 : (gw + 1) * 4, :],
                        in_=wTrl[:, gw * 4 : (gw + 1) * 4, :],
                    )
            def qkv_terms(Qp, KVp, xc, g, first_flag=True):
                for tp in range(XG // 2):
                    t = g * XG + 2 * tp
                    first = t == 0 and first_flag
                    last = t == DD - 2
                    lh = xc[:, 2 * tp : 2 * tp + 2, 0, :]
                    ll = xc[:, 2 * tp : 2 * tp + 2, 1, :]
                    terms = ((lh, wTth), (lh, wTtl), (ll, wTth))
                    for k, (lhsT, wt) in enumerate(terms):
                        nc.tensor.matmul(
                            Qp,
                            lhsT,
                            wt[:, t : t + 2, 0:EW],
                            start=(first and k == 0),
                            stop=(last and k == 2),
                            perf_mode=DR,
                        )
                    for k, (lhsT, wt) in enumerate(terms):
                        nc.tensor.matmul(
                            KVp,
                            lhsT,
                            wt[:, t : t + 2, EW : EW + 2 * D],
                            start=(first and k == 0),
                            stop=(last and k == 2),
                            perf_mode=DR,
                        )

            # s-tiles 0-2 are tripled: the weight stream is the DMA
            # bottleneck at kernel start (w hi+lo ~21us vs 7.7us PE per
            # s-tile), so each weight granule feeds three s-tiles' matmuls
            # (s1/s2 borrow PSUM from the still-idle attention pools)
            cst0 = cspool.tile([P, EW], P_DT, tag="cs")
            nc.sync.dma_start(out=cst0, in_=cs[0:P, :])
            cst1 = cspool.tile([P, EW], P_DT, tag="cs")
            nc.sync.dma_start(out=cst1, in_=cs[P : 2 * P, :])
            Qp0 = qps.tile([P, EW], f32, tag="Qp")
            KVp0 = kvsh.tile([P, 2 * D], f32, tag="KVp")
            Qp1 = s1ps.tile([P, 512], f32, tag="S1")
            KVt1 = atps.tile([P, 512], f32, tag="At")
            KVp1 = KVt1[:, 0 : 2 * D]
            Qp2 = s1ps.tile([P, 512], f32, tag="S1")
            KVp2 = kvsh.tile([P, 2 * D], f32, tag="KVx")
            st1_x = []
            st2_x = []
            for g in range(NG):
                xc1 = xpool.tile([P, XG, 2, P], f8, tag="xc")
                nc.sync.dma_start(out=xc1, in_=xTc[:, 1, g * XG : (g + 1) * XG, :, :])
                st1_x.append(xc1)
                xc2 = xpool.tile([P, XG, 2, P], f8, tag="xc")
                nc.sync.dma_start(out=xc2, in_=xTc[:, 2, g * XG : (g + 1) * XG, :, :])
                st2_x.append(xc2)
                qkv_terms(Qp0, KVp0, st0_x[g], g)
                qkv_terms(Qp1, KVp1, xc1, g)
                qkv_terms(Qp2, KVp2, xc2, g)

            for st in range(ST):
                j = st - 4  # attention block woven into this s-tile
                if j >= 0:
                    PTt, recips = open_A(j)
                if st == 0:
                    Qp, KVp, cst = Qp0, KVp0, cst0
                elif st == 1:
                    Qp, KVp, cst = Qp1, KVp1, cst1
                elif st == 2:
                    cst = cspool.tile([P, EW], P_DT, tag="cs")
                    nc.sync.dma_start(out=cst, in_=cs[st * P : (st + 1) * P, :])
                    Qp, KVp = Qp2, KVp2
                else:
                    cst = cspool.tile([P, EW], P_DT, tag="cs")
                    nc.sync.dma_start(out=cst, in_=cs[st * P : (st + 1) * P, :])
                    Qp = qps.tile([P, EW], f32, tag="Qp")
                    KVp = kvsh.tile([P, 2 * D], f32, tag="KVp")
                for g in range(DD // XG if st >= 3 else 0):
                    xc = xpool.tile([P, XG, 2, P], f8, tag="xc")
                    nc.sync.dma_start(
                        out=xc, in_=xTc[:, st, g * XG : (g + 1) * XG, :, :]
                    )
                    qkv_terms(Qp, KVp, xc, g)
                    # one attention row between x-chunk groups keeps ACT fed
                    # while PE grinds the projection matmuls
                    if j >= 0:
                        rowA(j, g, PTt, recips)

                # free the Q/KV PSUM banks fast: one copy each, rope reads SBUF
                qsb = rpool.tile([P, EW], f32, tag="qsb")
                nc.vector.tensor_copy(out=qsb, in_=Qp)
                kvsb = rpool.tile([P, 2 * D], f32, tag="kvsb")
                nc.vector.tensor_copy(out=kvsb, in_=KVp)

                def ttr_ew(out_, in0, in1, op):
                    nc.vector.tensor_tensor(out=out_, in0=in0, in1=in1, op=op)

                HF = EW // 2
                rq = rpool.tile([P, EW], P_DT, tag="rq")
                t1 = rpool.tile([P, HF], f32, tag="t1")
                t2 = rpool.tile([P, HF], f32, tag="t2")
                q_ev, q_od = qsb[:, 0:EW:2], qsb[:, 1:EW:2]
                cosr, sinr = cst[:, 0:HF], cst[:, HF : 2 * HF]
                ttr_ew(t1, q_ev, cosr, A_.mult)
                ttr_ew(t2, q_od, sinr, A_.mult)
                ttr_ew(rq[:, 0:EW:2], t1, t2, A_.subtract)
                ttr_ew(t1, q_ev, sinr, A_.mult)
                ttr_ew(t2, q_od, cosr, A_.mult)
                ttr_ew(rq[:, 1:EW:2], t1, t2, A_.add)

                rk = rpool.tile([P, D], P_DT, tag="rk")
                k_ev, k_od = kvsb[:, 0:D:2], kvsb[:, 1:D:2]
                cosk, sink = cst[:, 0 : D // 2], cst[:, HF : HF + D // 2]
                ttr_ew(t1[:, 0 : D // 2], k_ev, cosk, A_.mult)
                ttr_ew(t2[:, 0 : D // 2], k_od, sink, A_.mult)
                ttr_ew(rk[:, 0:D:2], t1[:, 0 : D // 2], t2[:, 0 : D // 2], A_.subtract)
                ttr_ew(t1[:, 0 : D // 2], k_ev, sink, A_.mult)
                ttr_ew(t2[:, 0 : D // 2], k_od, cosk, A_.mult)
                ttr_ew(rk[:, 1:D:2], t1[:, 0 : D // 2], t2[:, 0 : D // 2], A_.add)

                nc.scalar.activation(
                    out=Vt[:, st, :],
                    in_=kvsb[:, D : 2 * D],
                    func=AF.Copy,
                    scale=float(1.0 / WS1),
                )

                # transposes into [d, s] layouts via the DMA xbar: frees the
                # PE/DVE cycles and the TT PSUM bank
                for h in range(NH):
                    nc.sync.dma_start_transpose(
                        out=QTt[:, h, st * P : (st + 1) * P],
                        in_=rq[:, h * P : (h + 1) * P],
                    )
                nc.sync.dma_start_transpose(
                    out=KTt[:, st * P : (st + 1) * P], in_=rk
                )

                if j >= 1:
                    stage_B(j - 1)
                if st >= 12:
                    drip_w0(st - 12)

        # ---------------- tail: qs=3 blocks + output projection ----------------
        with (
            tc.tile_pool(name="wopool", bufs=2) as wopool,
            tc.tile_pool(name="osb", bufs=2) as osb,
            tc.tile_pool(name="ops", bufs=3, space="PSUM") as ops,
        ):
            wot_tiles = {}
            wqueue = []
            units = {}
            wohr = woh.rearrange("(t p) m -> p t m", p=P)
            wolr = wol.rearrange("(t p) m -> p t m", p=P)

            def alloc_wot(mc):
                wth = wopool.tile([P, JT, 512], f8, tag="woh")
                wtl = wopool.tile([P, JT, 512], f8, tag="wol")
                wot_tiles[mc] = (wth, wtl)
                # quarter q covers t-pairs [4q, 4q+4); hi then lo
                for qq in range(4):
                    wqueue.append((mc, qq, 0))
                    wqueue.append((mc, qq, 1))

            def drip(n):
                for _ in range(min(n, len(wqueue))):
                    mc, qq, lo = wqueue.pop(0)
                    wt = wot_tiles[mc][lo]
                    src = wolr if lo else wohr
                    nc.sync.dma_start(
                        out=wt[:, qq * (JT // 4) : (qq + 1) * (JT // 4), :],
                        in_=src[
                            :,
                            qq * (JT // 4) : (qq + 1) * (JT // 4),
                            mc * 512 : (mc + 1) * 512,
                        ],
                    )

            def load_wot(mc):
                alloc_wot(mc)
                drip(8)

            def unit_mms(mc, it, tps, start, stop):
                wth, wtl = wot_tiles[mc]
                if (mc, it) in units:
                    O = units[(mc, it)]
                else:
                    O = ops.tile([P, 512], f32, tag="O")
                    units[(mc, it)] = O
                for n, tp in enumerate(tps):
                    lh = Aall_h[it][:, 2 * tp * P : (2 * tp + 2) * P].rearrange(
                        "a (two s) -> a two s", two=2
                    )
                    ll = Aall_l[it][:, 2 * tp * P : (2 * tp + 2) * P].rearrange(
                        "a (two s) -> a two s", two=2
                    )
                    rh = wth[:, 2 * tp : 2 * tp + 2, :]
                    rl = wtl[:, 2 * tp : 2 * tp + 2, :]
                    for k, (lhsT, rhs) in enumerate(((lh, rh), (lh, rl), (ll, rh))):
                        nc.tensor.matmul(
                            O,
                            lhsT,
                            rhs,
                            start=(start and n == 0 and k == 0),
                            stop=(stop and n == len(tps) - 1 and k == 2),
                            perf_mode=DR,
                        )

            def unit_fin(mc, it):
                O = units.pop((mc, it))
                Ot = osb.tile([P, 512], P_DT, tag="Ot")
                nc.scalar.activation(
                    out=Ot, in_=O, func=AF.Copy, scale=float(1.0 / WS3)
                )
                nc.sync.dma_start(
                    out=out[it * P : (it + 1) * P, mc * 512 : (mc + 1) * 512],
                    in_=Ot,
                )

            def unit(mc, it):
                unit_mms(mc, it, range(ST), True, True)
                unit_fin(mc, it)

            wot_tiles[0] = (w0h, w0l)  # prefetched during the interleave
            alloc_wot(1)
            alloc_wot(2)
            # qs=3 attention blocks (need all 16 s-tiles), pipelined; wo
            # chunk loads drip between rows so they never block the
            # latency-critical P transposes on the DMA engines. Phase-3
            # units split: t0-7 accumulation only needs qs<=1 heads (final
            # long before the tail), t8-15 needs the qs=3 heads.
            HALF1, HALF2 = range(0, 8), range(8, 16)
            # these two first-half units depend only on qs<=1 heads (done
            # mid-interleave) and the prefetched mc0 chunk: they fill the
            # PE idle at tail start
            unit_mms(0, 0, HALF1, True, False)
            unit_mms(0, 1, HALF1, True, False)
            PTt, recips = open_A(12)
            for qi in range(4):
                rowA(12, qi, PTt, recips)
                drip(2)
            stage_B(11)
            for j in (13, 14, 15):
                PTt, recips = open_A(j)
                for qi in range(4):
                    rowA(j, qi, PTt, recips)
                    drip(2)
                stage_B(j - 1)
                if j == 13:
                    unit_mms(1, 0, HALF1, True, False)
                elif j == 14:
                    # Aall[0] complete after B(13)
                    unit_mms(0, 0, HALF2, False, True)
                    unit_fin(0, 0)
                    unit_mms(2, 0, HALF1, True, False)
                elif j == 15:
                    unit_mms(1, 0, HALF2, False, True)
                    unit_fin(1, 0)
            stage_B(15)
            drip(len(wqueue))
            unit_mms(2, 0, HALF2, False, True)
            unit_fin(2, 0)
            unit_mms(0, 1, HALF2, False, True)
            unit_fin(0, 1)
            # preloaded chunks' it=1 work covers the in-flight loads of the
            # later chunks (slot for mc+2 frees as soon as mc's last unit
            # is emitted)
            def load_wot_pre(mc):
                # rotate the wopre slot (mc0's chunk is consumed by now)
                wth = wopre.tile([P, JT, 512], f8, tag="wph")
                wtl = wopre.tile([P, JT, 512], f8, tag="wpl")
                wot_tiles[mc] = (wth, wtl)
                for qq in range(4):
                    wqueue.append((mc, qq, 0))
                    wqueue.append((mc, qq, 1))
                drip(8)

            def unit_split_cols(mc, it):
                # last unit: two column-halves so the closing copy/store
                # overlaps the second half's matmuls
                wth, wtl = wot_tiles[mc]
                for half in range(2):
                    O = ops.tile([P, 512], f32, tag="O")
                    cl, ch = half * 256, (half + 1) * 256
                    for n, tp in enumerate(range(ST)):
                        lh = Aall_h[it][:, 2 * tp * P : (2 * tp + 2) * P].rearrange(
                            "a (two s) -> a two s", two=2
                        )
                        ll = Aall_l[it][:, 2 * tp * P : (2 * tp + 2) * P].rearrange(
                            "a (two s) -> a two s", two=2
                        )
                        rh = wth[:, 2 * tp : 2 * tp + 2, cl:ch]
                        rl = wtl[:, 2 * tp : 2 * tp + 2, cl:ch]
                        for k, (lhsT, rhs) in enumerate(
                            ((lh, rh), (lh, rl), (ll, rh))
                        ):
                            nc.tensor.matmul(
                                O[:, 0:256],
                                lhsT,
                                rhs,
                                start=(n == 0 and k == 0),
                                stop=(n == ST - 1 and k == 2),
                                perf_mode=DR,
                            )
                    Ot = osb.tile([P, 512], P_DT, tag="Ot")
                    nc.scalar.activation(
                        out=Ot[:, 0:256],
                        in_=O[:, 0:256],
                        func=AF.Copy,
                        scale=float(1.0 / WS3),
                    )
                    nc.sync.dma_start(
                        out=out[
                            it * P : (it + 1) * P,
                            mc * 512 + cl : mc * 512 + ch,
                        ],
                        in_=Ot[:, 0:256],
                    )

            for mc in (1, 2, 3, 4, 5, 6, 7):
                if mc >= 3:
                    unit(mc, 0)
                if mc == 7:
                    unit_split_cols(mc, 1)
                else:
                    unit(mc, 1)
                wot_tiles.pop(mc)
                nxt = mc + 2 if mc >= 3 else {1: 3, 2: 4}.get(mc)
                if nxt is not None and nxt < MC and nxt not in wot_tiles:
                    load_wot(nxt)
                if mc == 2:
                    load_wot(5)

    nc.compile()
    return nc


def is_pure_causal(mask, SEQ):
    """True iff mask[i,j] == 0 for j<=i and <= NEG_THRESH for j>i."""
    m = np.asarray(mask, np.float32)
    if m.shape != (SEQ, SEQ):
        return False
    j = np.arange(SEQ)
    allowed = j[None, :] <= j[:, None]
    return bool((m[allowed] == 0).all() and (m[~allowed] <= NEG_THRESH).all())


def make_rope_tables(cos_freq, sin_freq, SEQ, scale_quarter):
    cos_t = np.tile(np.asarray(cos_freq, np.float32) * scale_quarter, (1, NH))
    sin_t = np.tile(np.asarray(sin_freq, np.float32) * scale_quarter, (1, NH))
    return np.ascontiguousarray(
        np.concatenate([cos_t, sin_t], axis=1).astype(np.float32)
    )




_BUILD_CACHE = {}


def kernel(
    x,
    cos_freq,
    sin_freq,
    positions,
    mask,
    wq,
    wk,
    wv,
    wo,
    _trace=False,
):
    import sys

    if "/opt/trn_rl_repo" not in sys.path:
        sys.path.insert(0, "/opt/trn_rl_repo")
    from concourse.bass_utils import run_bass_kernel_spmd
    import ml_dtypes

    x = np.asarray(x, np.float32)
    mask = np.asarray(mask, np.float32)
    wq = np.asarray(wq, np.float32)
    wk = np.asarray(wk, np.float32)
    wv = np.asarray(wv, np.float32)
    wo = np.asarray(wo, np.float32)
    SEQ, DIM = x.shape
    assert wq.shape[0] == CORES * NH * D and wk.shape[0] == CORES * D
    assert 2 * SEQ == wq.shape[0], "flatten structure requires H*D == 2*SEQ"

    bf16 = ml_dtypes.bfloat16
    f8 = ml_dtypes.float8_e4m3
    ST_, DD_ = SEQ // P, DIM // P

    if is_pure_causal(mask, SEQ):
        key = (SEQ, DIM, "causal")
        if key not in _BUILD_CACHE:
            _BUILD_CACHE[key] = build_attention_v7(SEQ, DIM)
        nc = _BUILD_CACHE[key]

        def hilo(a):
            hi = np.ascontiguousarray(a).astype(f8)
            lo = np.ascontiguousarray(a - hi.astype(np.float32)).astype(f8)
            return hi, lo

        # fold sqrt(scale) and the 1/WS1 weight pre-scale into rope tables
        scale_quarter = np.float32(D ** -0.25 / WS1)
        cs = make_rope_tables(cos_freq, sin_freq, SEQ, scale_quarter).astype(bf16)
        xT = np.ascontiguousarray(x.reshape(ST_, P, DD_, P).transpose(3, 0, 2, 1))
        xh_, xl_ = hilo(xT)
        xTc = np.ascontiguousarray(np.stack([xh_, xl_], axis=3))
        # wo row-blocks permuted so DoubleRow contraction pairs are adjacent
        JT_ = 2 * SEQ // P
        perm = [(jt % 2) * (JT_ // 2) + jt // 2 for jt in range(JT_)]
        woP = np.ascontiguousarray(
            (wo.T * np.float32(WS3)).reshape(JT_, P, DIM)[perm].reshape(2 * SEQ, DIM)
        )
        woh, wol = hilo(woP)
        tri = np.ascontiguousarray(mask[0:P, 0:P])

        in_maps = []
        for c in range(CORES):
            w_c = np.concatenate(
                [
                    wq[c * NH * D : (c + 1) * NH * D],
                    wk[c * D : (c + 1) * D],
                    wv[c * D : (c + 1) * D],
                ],
                axis=0,
            )
            wTh_, wTl_ = hilo(w_c.T * np.float32(WS1))
            in_maps.append(
                {
                    "xTc": xTc,
                    "wTh": wTh_,
                    "wTl": wTl_,
                    "cs": cs,
                    "tri": tri,
                    "woh": woh,
                    "wol": wol,
                }
            )
        res = run_bass_kernel_spmd(nc, in_maps, list(range(CORES)), trace=_trace)
        outp = np.concatenate(
            [np.asarray(res.results[c]["out"]) for c in range(CORES)], axis=0
        ).astype(np.float32)
        if _trace:
            return outp, res
        return outp

    # ---------------- general-mask fallback (v1 kernel) ----------------
    plan, blocks = analyze_mask(mask, SEQ)
    n_uniq = len(blocks)
    key = (SEQ, DIM, tuple(tuple(r) for r in plan))
    if key not in _BUILD_CACHE:
        _BUILD_CACHE[key] = build_attention_nc(SEQ, DIM, plan, n_uniq)
    nc = _BUILD_CACHE[key]

    scale_quarter = np.float32(D ** -0.25)
    csf = make_rope_tables(cos_freq, sin_freq, SEQ, scale_quarter)
    xT = np.ascontiguousarray(
        x.reshape(ST_, P, DD_, P).transpose(3, 0, 2, 1)
    ).astype(bf16)
    woT = np.ascontiguousarray(wo.T).astype(bf16)
    if n_uniq:
        mbs = np.ascontiguousarray(np.stack(blocks, axis=0))
    else:
        mbs = np.zeros((1, P, 512), np.float32)

    in_maps = []
    for c in range(CORES):
        w_c = np.concatenate(
            [
                wq[c * NH * D : (c + 1) * NH * D],
                wk[c * D : (c + 1) * D],
                wv[c * D : (c + 1) * D],
            ],
            axis=0,
        )
        in_maps.append(
            {
                "xT": xT,
                "wT": np.ascontiguousarray(w_c.T).astype(bf16),
                "cs": csf,
                "maskb": mbs,
                "woT": woT,
            }
        )
    res = run_bass_kernel_spmd(nc, in_maps, list(range(CORES)), trace=_trace)
    outp = np.concatenate(
        [np.asarray(res.results[c]["out"]) for c in range(CORES)], axis=0
    ).astype(np.float32)
    if _trace:
        return outp, res
    return outp


# revision 11
# speedup vs baseline: 1.0247x; 1.0049x over previous
"""Trainium2 Bass kernel for nn_Attention (GQA + RoPE + sliding-window mask).

Sharding: tensor-parallel over heads across 8 cores (4 q heads + 1 kv head
per core). The reference's quirky output flatten ((H,S,D)->(H,D,S)->
reshape(S, H*D)) makes the final projection row-shard by head block: core c
produces rows [256c, 256c+256) of the (2048, 4096) result with no collective.

Fast path (pure causal mask, the shape this problem produces):
  * phase 1 (QKV projections) and phase 3 (output projection) run as
    fp8-e4m3 hi/lo pairs in DoubleRow perf mode: X @ W ~= Xh@Wh + Xh@Wl +
    Xl@Wh with Xh = fp8(X), Xl = fp8(X - Xh) - 0.75x the PE time of one
    bf16 pass and more accurate than bf16 (~9-10 effective mantissa bits).
  * attention (phase 2) is interleaved INTO phase 1: block (qs, h) only
    needs s-tiles <= 4qs+3, so blocks weave between projection s-tiles with
    score rows emitted between x-chunk groups - softmax (ACT) latency hides
    under the projection matmuls.
  * no-max softmax (causal logits here are O(10), exp is safe in fp32),
    exp reads score PSUM directly with accum_out row sums; only the
    128-wide triangular diagonal block gets a mask add; diagonal PV
    matmuls are narrowed to the live query columns.
  * P transposed via DMA xbar; wo chunks prefetched/dripped so the big
    loads never head-of-line block the latency-critical transposes.

Fallback (any other mask): the v1 kernel (per-chunk mask add + 2-pass
max/exp softmax), correct for arbitrary additive masks.
"""

import numpy as np
from contextlib import ExitStack

P = 128
D = 128  # head dim
NH = 4   # q heads per core
CORES = 8
NEG_THRESH = -1e8


def build_attention_nc(
    SEQ,
    DIM,
    plan,
    n_uniq,
    p_dt_name="bfloat16",
    wo_dt_name="bfloat16",
    proj_dt_name="bfloat16",
    proj_f32r=True,
    score_f32r=True,
    use_dma_t=True,
):
    """Build the per-core Bass program.

    plan: list over q-tiles i (SEQ//128 entries) of lists of (chunk_idx, uid)
          where uid == -1 means the 512-wide chunk needs no mask add, else the
          index into the maskb tensor. Chunks absent from the list are fully
          masked (skipped).
    """
    import concourse.bass as bass
    import concourse.bacc as bacc
    import concourse.mybir as mybir
    import concourse.tile as tile
    from concourse.masks import make_identity

    f32 = mybir.dt.float32
    f32r = mybir.dt.float32r
    P_DT = getattr(mybir.dt, p_dt_name)
    WO_DT = getattr(mybir.dt, wo_dt_name)
    PJ_DT = getattr(mybir.dt, proj_dt_name)
    pj_f32r = proj_f32r and proj_dt_name == "float32"

    ST = SEQ // P          # 16 s-tiles
    DD = DIM // P          # 32 contraction tiles
    KC = SEQ // 512        # 4 key chunks
    QS = SEQ // 512        # 4 query supers
    EW = NH * D            # 512 q-projection width
    JT = 2 * SEQ // P      # 32 j-tiles for final matmul
    MC = DIM // 512        # 8 output chunks
    ITILES = (NH * 64) // P  # 2 output row tiles
    assert NH == 4 and SEQ % 512 == 0 and DIM % 512 == 0

    def mm_cast(ap, use_r):
        return ap.bitcast(f32r) if use_r else ap

    nc = bacc.Bacc(trn_type="TRN2", debug=False, num_devices=CORES)

    # x pre-tiled on host: xT[p, st, t, si] = x[st*128+si, t*128+p] so each
    # streamed chunk is one DMA with 2KB contiguous per-partition runs
    xT = nc.dram_tensor("xT", [P, ST, DD, P], PJ_DT, kind="ExternalInput").ap()
    wT = nc.dram_tensor("wT", [DIM, EW + 2 * D], PJ_DT, kind="ExternalInput").ap()
    cs = nc.dram_tensor("cs", [SEQ, EW], f32, kind="ExternalInput").ap()
    mb = nc.dram_tensor(
        "maskb", [max(n_uniq, 1), P, 512], f32, kind="ExternalInput"
    ).ap()
    woT = nc.dram_tensor("woT", [2 * SEQ, DIM], WO_DT, kind="ExternalInput").ap()
    out = nc.dram_tensor("out", [NH * 64, DIM], f32, kind="ExternalOutput").ap()

    with tile.TileContext(nc) as tc, ExitStack() as ctx:
        const = ctx.enter_context(tc.tile_pool(name="const", bufs=1))
        idF = const.tile([P, P], f32)
        make_identity(nc, idF)
        idP = const.tile([P, P], P_DT)
        make_identity(nc, idP)
        zeros = const.tile([P, 512], f32)
        nc.vector.memset(zeros, 0.0)

        pers = ctx.enter_context(tc.tile_pool(name="pers", bufs=1))
        QTt = pers.tile([P, NH, ST * P], f32)   # [d, h, s]
        KTt = pers.tile([P, ST * P], f32)       # [d, s]
        Vt = pers.tile([P, ST, D], P_DT)        # [k(part), ktile, d]
        if n_uniq > 0:
            mbt = pers.tile([P, n_uniq, 512], f32)

        # ---------------- phase 1: projections + rope + layout ----------------
        with (
            tc.tile_pool(name="wpool", bufs=1) as wpool,
            tc.tile_pool(name="xpool", bufs=6) as xpool,
            tc.tile_pool(name="cspool", bufs=2) as cspool,
            tc.tile_pool(name="rpool", bufs=2) as rpool,
            tc.tile_pool(name="qps", bufs=2, space="PSUM") as qps,
            tc.tile_pool(name="kvps", bufs=2, space="PSUM") as kvps,
            tc.tile_pool(name="tps", bufs=2, space="PSUM") as tps,
            tc.tile_pool(name="t2ps", bufs=2, space="PSUM") as t2ps,
        ):
            XGW = min(8, DD)
            wTt = wpool.tile([P, DD, EW + 2 * D], PJ_DT)
            wTr = wT.rearrange("(t p) e -> p t e", p=P)

            XG = min(8, DD)  # dd-tiles per streamed x chunk
            NG = DD // XG
            xTr = xT
            # Interleave the weight-chunk loads with s-tile 0's x chunks so
            # the first matmuls start as soon as chunk 0 of each lands.
            st0_x = []
            for g in range(NG):
                xTt = xpool.tile([P, XG, P], PJ_DT, tag="xT")
                nc.sync.dma_start(
                    out=xTt, in_=xTr[:, 0, g * XG : (g + 1) * XG, :]
                )
                st0_x.append(xTt)
                gw = g % (DD // XGW)
                nc.sync.dma_start(
                    out=wTt[:, gw * XGW : (gw + 1) * XGW, :],
                    in_=wTr[:, gw * XGW : (gw + 1) * XGW, :],
                )
            for st in range(ST):
                cst = cspool.tile([P, EW], f32, tag="cs")
                nc.sync.dma_start(out=cst, in_=cs[st * P : (st + 1) * P, :])

                Qp = qps.tile([P, EW], f32, tag="Qp")
                KVp = kvps.tile([P, 2 * D], f32, tag="KVp")
                for g in range(DD // XG):
                    if st == 0:
                        xTt = st0_x[g]
                    else:
                        xTt = xpool.tile([P, XG, P], PJ_DT, tag="xT")
                        nc.sync.dma_start(
                            out=xTt,
                            in_=xTr[:, st, g * XG : (g + 1) * XG, :],
                        )
                    for tt in range(XG):
                        t = g * XG + tt
                        lhsT = mm_cast(xTt[:, tt, :], pj_f32r)
                        nc.tensor.matmul(
                            Qp,
                            lhsT,
                            mm_cast(wTt[:, t, 0:EW], pj_f32r),
                            start=(t == 0),
                            stop=(t == DD - 1),
                        )
                        nc.tensor.matmul(
                            KVp,
                            lhsT,
                            mm_cast(wTt[:, t, EW : EW + 2 * D], pj_f32r),
                            start=(t == 0),
                            stop=(t == DD - 1),
                        )

                # rope via strided even/odd halves (2-level APs only — 3-level
                # APs overflow the fixed ISA instruction encoding).
                # tensor_tensor_reduce instead of tensor_tensor: the plain TT
                # ISA struct has a single sync-wait slot and walrus codegen
                # rejects the PE+DMA double wait Tile emits here; the TTR/ISA
                # struct carries up to 8. accum outputs are dummies.
                def ttr_ew(out, in0, in1, op):
                    nc.vector.tensor_tensor(out=out, in0=in0, in1=in1, op=op)

                A_ = mybir.AluOpType
                HF = EW // 2  # 256: cos table width for q
                rq = rpool.tile([P, EW], f32, tag="rq")
                t1 = rpool.tile([P, HF], f32, tag="t1")
                t2 = rpool.tile([P, HF], f32, tag="t2")
                q_ev, q_od = Qp[:, 0:EW:2], Qp[:, 1:EW:2]
                cosr, sinr = cst[:, 0:HF], cst[:, HF : 2 * HF]
                ttr_ew(t1, q_ev, cosr, A_.mult)
                ttr_ew(t2, q_od, sinr, A_.mult)
                ttr_ew(rq[:, 0:EW:2], t1, t2, A_.subtract)
                ttr_ew(t1, q_ev, sinr, A_.mult)
                ttr_ew(t2, q_od, cosr, A_.mult)
                ttr_ew(rq[:, 1:EW:2], t1, t2, A_.add)

                rk = rpool.tile([P, D], f32, tag="rk")
                k_ev, k_od = KVp[:, 0:D:2], KVp[:, 1:D:2]
                cosk, sink = cst[:, 0 : D // 2], cst[:, HF : HF + D // 2]
                ttr_ew(t1[:, 0 : D // 2], k_ev, cosk, A_.mult)
                ttr_ew(t2[:, 0 : D // 2], k_od, sink, A_.mult)
                ttr_ew(rk[:, 0:D:2], t1[:, 0 : D // 2], t2[:, 0 : D // 2], A_.subtract)
                ttr_ew(t1[:, 0 : D // 2], k_ev, sink, A_.mult)
                ttr_ew(t2[:, 0 : D // 2], k_od, cosk, A_.mult)
                ttr_ew(rk[:, 1:D:2], t1[:, 0 : D // 2], t2[:, 0 : D // 2], A_.add)

                # V -> bf16 [k, d] layout (ACT copy, cast)
                nc.scalar.activation(
                    out=Vt[:, st, :],
                    in_=KVp[:, D : 2 * D],
                    func=mybir.ActivationFunctionType.Copy,
                )

                # transpose rq (per head) and rk into [d, s] layouts
                T1 = tps.tile([P, EW], f32, tag="T1")
                for h in range(NH):
                    nc.tensor.transpose(
                        T1[:, h * P : (h + 1) * P], rq[:, h * P : (h + 1) * P], idF
                    )
                # write as f32r so walrus accepts them as f32r matmul operands
                nc.vector.tensor_copy(
                    out=mm_cast(QTt[:, :, st * P : (st + 1) * P], score_f32r),
                    in_=T1.rearrange("p (h s) -> p h s", h=NH),
                )
                T2 = t2ps.tile([P, P], f32, tag="T2")
                nc.tensor.transpose(T2, rk, idF)
                nc.vector.tensor_copy(
                    out=mm_cast(KTt[:, st * P : (st + 1) * P], score_f32r), in_=T2
                )

        # ---------------- phase 2: attention ----------------
        if n_uniq > 0:
            nc.sync.dma_start(out=mbt, in_=mb.rearrange("u p m -> p u m"))
        apool = ctx.enter_context(tc.tile_pool(name="apool", bufs=1))
        # split by head-pair so phase 3's first row-tile can start once
        # heads 0-1 finish, overlapping the rest of phase 2
        Aall = [
            apool.tile([P, 2 * ST * D], P_DT, name=f"Aall{i}")
            for i in range(NH // 2)
        ]
        with (
            tc.tile_pool(name="ptsb", bufs=2) as ptsb,
            tc.tile_pool(name="spool", bufs=6) as spool,
            tc.tile_pool(name="ppool", bufs=4) as ppool,
            tc.tile_pool(name="stat", bufs=12) as stat,
            tc.tile_pool(name="atsb", bufs=3) as atsb,
            tc.tile_pool(name="sps", bufs=2, space="PSUM") as sps,
            tc.tile_pool(name="ptps", bufs=2, space="PSUM") as ptps,
            tc.tile_pool(name="atps", bufs=1, space="PSUM") as atps,
            tc.tile_pool(name="aps", bufs=1, space="PSUM") as aps,
            tc.tile_pool(name="wopool", bufs=2 if n_uniq <= 4 else 1) as wopool,
            tc.tile_pool(name="osb", bufs=3) as osb,
            tc.tile_pool(name="ops", bufs=3, space="PSUM") as ops,
        ):
            for h in range(NH):
                for qs in range(QS):
                    PTt = ptsb.tile([P, ST, 512], P_DT, tag="PT")
                    kts_used = set()
                    recips = []
                    pt_written = set()
                    for qi in range(4):
                        i = 4 * qs + qi
                        row = plan[i]
                        if not row:
                            recips.append(None)
                            continue
                        pairs = [row[k : k + 2] for k in range(0, len(row), 2)]
                        stats = stat.tile([P, KC], f32, tag="stats")
                        ncols = 0
                        S_tiles = []
                        for pr in pairs:
                            W = 512 * len(pr)
                            S = sps.tile([P, 1024], f32, tag="S")
                            Ssb = spool.tile([P, 1024], f32, tag="Ssb")
                            masked_any = any(uid >= 0 for (_, uid) in pr)
                            for k, (c, uid) in enumerate(pr):
                                sl = S[:, k * 512 : (k + 1) * 512]
                                nc.tensor.matmul(
                                    sl,
                                    mm_cast(
                                        QTt[:, h, i * P : (i + 1) * P], score_f32r
                                    ),
                                    mm_cast(
                                        KTt[:, c * 512 : (c + 1) * 512], score_f32r
                                    ),
                                    start=True,
                                    stop=True,
                                )
                                if uid >= 0:
                                    nc.vector.tensor_add(sl, sl, mbt[:, uid, :])
                                # copy PSUM->SBUF to free the score bank early;
                                # alternate DVE/ACT to balance engine load
                                dst = Ssb[:, k * 512 : (k + 1) * 512]
                                if (i + k) % 2 == 0:
                                    nc.vector.tensor_copy(out=dst, in_=sl)
                                else:
                                    nc.scalar.activation(
                                        out=dst,
                                        in_=sl,
                                        func=mybir.ActivationFunctionType.Copy,
                                    )
                                if masked_any or len(pr) == 1:
                                    nc.vector.tensor_reduce(
                                        out=stats[:, ncols : ncols + 1],
                                        in_=dst,
                                        axis=mybir.AxisListType.X,
                                        op=mybir.AluOpType.max,
                                    )
                                    ncols += 1
                            if not masked_any and len(pr) == 2:
                                # one pair-wide max over both chunks (SBUF 2x)
                                nc.vector.tensor_reduce(
                                    out=stats[:, ncols : ncols + 1],
                                    in_=Ssb,
                                    axis=mybir.AxisListType.X,
                                    op=mybir.AluOpType.max,
                                )
                                ncols += 1
                            S_tiles.append((Ssb, pr))
                        negm = stat.tile([P, 1], f32, tag="negm")
                        nc.vector.tensor_reduce(
                            out=negm,
                            in_=stats[:, 0:ncols],
                            axis=mybir.AxisListType.X,
                            op=mybir.AluOpType.max,
                            negate=True,
                        )
                        sums = stat.tile([P, KC], f32, tag="sums")
                        for k, (Sk, pr) in enumerate(S_tiles):
                            W = 512 * len(pr)
                            Pt = ppool.tile([P, 1024], P_DT, tag="P")
                            nc.scalar.activation(
                                out=Pt[:, 0:W],
                                in_=Sk[:, 0:W],
                                func=mybir.ActivationFunctionType.Exp,
                                bias=negm,
                                accum_out=sums[:, k : k + 1],
                            )
                            # transpose P [q, k] -> PT [k, q]
                            for j, (c, uid) in enumerate(pr):
                                if use_dma_t:
                                    nc.sync.dma_start_transpose(
                                        out=PTt[
                                            :, 4 * c : 4 * c + 4, qi * P : (qi + 1) * P
                                        ],
                                        in_=Pt[:, j * 512 : (j + 1) * 512],
                                    )
                                else:
                                    PTp = ptps.tile([P, 512], P_DT, tag="PTp")
                                    for jj in range(4):
                                        nc.tensor.transpose(
                                            PTp[:, jj * P : (jj + 1) * P],
                                            Pt[:, j * 512 + jj * P : j * 512 + (jj + 1) * P],
                                            idP,
                                        )
                                    nc.vector.tensor_copy(
                                        out=PTt[:, 4 * c : 4 * c + 4, qi * P : (qi + 1) * P],
                                        in_=PTp.rearrange("p (kt q) -> p kt q", kt=4),
                                    )
                                for jj in range(4):
                                    kts_used.add(4 * c + jj)
                                    pt_written.add((4 * c + jj, qi))
                        denom = stat.tile([P, 1], f32, tag="denom")
                        nc.vector.tensor_reduce(
                            out=denom,
                            in_=sums[:, 0 : len(S_tiles)],
                            axis=mybir.AxisListType.X,
                            op=mybir.AluOpType.add,
                        )
                        recip = stat.tile([P, 1], f32, tag="recip")
                        nc.vector.reciprocal(recip, denom)
                        recips.append(recip)

                    # zero-fill PT holes (only for non-causal masks)
                    kts = sorted(kts_used)
                    for kt in kts:
                        for qi in range(4):
                            if (kt, qi) not in pt_written and recips[qi] is not None:
                                nc.vector.memset(
                                    PTt[:, kt, qi * P : (qi + 1) * P], 0.0
                                )
                            elif recips[qi] is None:
                                nc.vector.memset(
                                    PTt[:, kt, qi * P : (qi + 1) * P], 0.0
                                )

                    if not kts:
                        continue
                    # PV: A^T[d, q] accumulated over key tiles
                    At = atps.tile([P, 512], f32, tag="At")
                    for n, kt in enumerate(kts):
                        nc.tensor.matmul(
                            At,
                            Vt[:, kt, :],
                            PTt[:, kt, :],
                            start=(n == 0),
                            stop=(n == len(kts) - 1),
                        )
                    Atsb = atsb.tile([P, 512], P_DT, tag="Atsb")
                    nc.vector.tensor_copy(out=Atsb, in_=At)
                    Ap = aps.tile([P, 512], P_DT, tag="Ap")
                    for qi in range(4):
                        nc.tensor.transpose(
                            Ap[:, qi * P : (qi + 1) * P],
                            Atsb[:, qi * P : (qi + 1) * P],
                            idP,
                        )
                    # Aall layout: [sp, (t*2 + dd)*128 + hb*64 + p] so the final
                    # matmul's stationary slices are contiguous (walrus requires
                    # a single free dim on weight APs)
                    Ah = Aall[h // 2]
                    hb = h % 2
                    for qi in range(4):
                        i = 4 * qs + qi
                        # dview[sp, p, dd] == Ah[:, i*256 + dd*128 + hb*64 + p]
                        dview = Ah[:, i * 2 * P : (i + 1) * 2 * P].rearrange(
                            "a (dd j) -> a dd j", dd=2
                        )[:, :, hb * 64 : hb * 64 + 64].rearrange(
                            "a dd p -> a p dd"
                        )
                        if recips[qi] is None:
                            nc.vector.memset(dview, 0.0)
                            continue
                        nc.scalar.activation(
                            out=dview,
                            in_=Ap[:, qi * P : (qi + 1) * P].rearrange(
                                "a (p two) -> a p two", two=2
                            ),
                            func=mybir.ActivationFunctionType.Copy,
                            scale=recips[qi],
                        )

            # ---------------- phase 3: output projection ----------------
            for mc in range(MC):
                wot = wopool.tile([P, JT, 512], WO_DT, tag="wo")
                nc.sync.dma_start(
                    out=wot,
                    in_=woT[:, mc * 512 : (mc + 1) * 512].rearrange(
                        "(t p) m -> p t m", p=P
                    ),
                )
                for it in range(ITILES):
                    O = ops.tile([P, 512], f32, tag="O")
                    Av = Aall[it]
                    for jt in range(JT):
                        ddj, t = jt // ST, jt % ST
                        lhsT = Av[:, (t * 2 + ddj) * P : (t * 2 + ddj + 1) * P]
                        nc.tensor.matmul(
                            O,
                            lhsT,
                            wot[:, jt, :],
                            start=(jt == 0),
                            stop=(jt == JT - 1),
                        )
                    Ot = osb.tile([P, 512], f32, tag="Ot")
                    nc.scalar.activation(
                        out=Ot, in_=O, func=mybir.ActivationFunctionType.Copy
                    )
                    nc.sync.dma_start(
                        out=out[it * P : (it + 1) * P, mc * 512 : (mc + 1) * 512],
                        in_=Ot,
                    )

    # Bacc.compile() legalizes sync (>=2 waits split into EventSemaphore
    # instructions — this walrus caps every instruction at ONE sync wait)
    nc.compile()
    return nc


def analyze_mask(mask, SEQ):
    """Classify 128x512 mask blocks: skip / free / masked(dedup uid)."""
    ST = SEQ // P
    KC = SEQ // 512
    uniq = {}
    blocks = []
    plan = []
    for i in range(ST):
        row = []
        for c in range(KC):
            blk = mask[i * P : (i + 1) * P, c * 512 : (c + 1) * 512]
            if (blk <= NEG_THRESH).all():
                continue
            if not blk.any():
                row.append((c, -1))
            else:
                key = blk.tobytes()
                if key not in uniq:
                    uniq[key] = len(blocks)
                    blocks.append(np.ascontiguousarray(blk))
                row.append((c, uniq[key]))
        if not row:
            # fully masked query rows: keep all chunks so softmax matches
            # the reference's uniform distribution over -1e9 logits
            for c in range(KC):
                blk = mask[i * P : (i + 1) * P, c * 512 : (c + 1) * 512]
                key = blk.tobytes()
                if key not in uniq:
                    uniq[key] = len(blocks)
                    blocks.append(np.ascontiguousarray(blk))
                row.append((c, uniq[key]))
        plan.append(row)
    return plan, blocks


WS1 = 1024.0  # host pre-scale on wq/wk/wv before fp8 (values ~0.02*N(0,1))
WS3 = 256.0   # host pre-scale on wo before fp8


def build_attention_v7(
    SEQ,
    DIM,
    p_dt_name="bfloat16",
):
    import concourse.bass as bass
    import concourse.bacc as bacc
    import concourse.mybir as mybir
    import concourse.tile as tile
    from concourse.masks import make_identity

    f32 = mybir.dt.float32
    f8 = mybir.dt.float8e4
    P_DT = getattr(mybir.dt, p_dt_name)
    A_ = mybir.AluOpType
    AF = mybir.ActivationFunctionType
    DR = mybir.MatmulPerfMode.DoubleRow

    ST = SEQ // P          # 16 s-tiles
    DD = DIM // P          # 32 contraction tiles
    QS = SEQ // 512        # 4 query supers
    EW = NH * D            # 512 q-projection width
    JT = 2 * SEQ // P      # 32 j-tiles for final matmul
    MC = DIM // 512        # 8 output chunks
    ITILES = (NH * 64) // P  # 2 output row tiles
    assert NH == 4 and SEQ % 512 == 0 and DIM % 512 == 0

    nc = bacc.Bacc(trn_type="TRN2", debug=False, num_devices=CORES)

    # x hi/lo packed per dd-tile so each DMA moves 2KB contiguous runs
    xTc = nc.dram_tensor("xTc", [P, ST, DD, 2, P], f8, kind="ExternalInput").ap()
    wTh = nc.dram_tensor("wTh", [DIM, EW + 2 * D], f8, kind="ExternalInput").ap()
    wTl = nc.dram_tensor("wTl", [DIM, EW + 2 * D], f8, kind="ExternalInput").ap()
    cs = nc.dram_tensor("cs", [SEQ, EW], P_DT, kind="ExternalInput").ap()
    tri = nc.dram_tensor("tri", [P, P], f32, kind="ExternalInput").ap()
    # wo row blocks permuted host-side: block jt' = 2t+dd <- original dd*16+t
    woh = nc.dram_tensor("woh", [2 * SEQ, DIM], f8, kind="ExternalInput").ap()
    wol = nc.dram_tensor("wol", [2 * SEQ, DIM], f8, kind="ExternalInput").ap()
    out = nc.dram_tensor("out", [NH * 64, DIM], P_DT, kind="ExternalOutput").ap()

    with tile.TileContext(nc) as tc, ExitStack() as ctx:
        const = ctx.enter_context(tc.tile_pool(name="const", bufs=1))
        idP = const.tile([P, P], P_DT)
        make_identity(nc, idP)

        pers = ctx.enter_context(tc.tile_pool(name="pers", bufs=1))
        QTt = pers.tile([P, NH, ST * P], P_DT)   # [d, h, s]
        KTt = pers.tile([P, ST * P], P_DT)       # [d, s]
        Vt = pers.tile([P, ST, D], P_DT)         # [k(part), ktile, d]
        trit = pers.tile([P, P], f32)
        nc.sync.dma_start(out=trit, in_=tri)

        # mc=0 wo chunk is prefetched during the interleaved region (the only
        # chunk SBUF has room for before the phase-1 pools close)
        wopre = ctx.enter_context(tc.tile_pool(name="wopre", bufs=1))
        w0h = wopre.tile([P, 2 * SEQ // P, 512], f8, tag="wph")
        w0l = wopre.tile([P, 2 * SEQ // P, 512], f8, tag="wpl")
        wohr_ = woh.rearrange("(t p) m -> p t m", p=P)
        wolr_ = wol.rearrange("(t p) m -> p t m", p=P)

        def drip_w0(qq):
            q4 = (2 * SEQ // P) // 4
            nc.sync.dma_start(
                out=w0h[:, qq * q4 : (qq + 1) * q4, :],
                in_=wohr_[:, qq * q4 : (qq + 1) * q4, 0:512],
            )
            nc.sync.dma_start(
                out=w0l[:, qq * q4 : (qq + 1) * q4, :],
                in_=wolr_[:, qq * q4 : (qq + 1) * q4, 0:512],
            )

        apool = ctx.enter_context(tc.tile_pool(name="apool", bufs=1))
        Aall_h = [
            apool.tile([P, 2 * ST * D], f8, name=f"Aallh{i}") for i in range(NH // 2)
        ]
        Aall_l = [
            apool.tile([P, 2 * ST * D], f8, name=f"Aalll{i}") for i in range(NH // 2)
        ]
        # attention pools live through phase 1+2 and the tail
        ptsb = ctx.enter_context(tc.tile_pool(name="ptsb", bufs=2))
        ppool = ctx.enter_context(tc.tile_pool(name="ppool", bufs=8))
        stat = ctx.enter_context(tc.tile_pool(name="stat", bufs=12))
        atsb = ctx.enter_context(tc.tile_pool(name="atsb", bufs=3))
        s1ps = ctx.enter_context(tc.tile_pool(name="s1ps", bufs=3, space="PSUM"))
        atps = ctx.enter_context(tc.tile_pool(name="atps", bufs=1, space="PSUM"))
        aps = ctx.enter_context(tc.tile_pool(name="aps", bufs=1, space="PSUM"))

        blocks = [(qs, h) for qs in range(QS) for h in range(NH)]  # j = 4qs+h
        state = {}

        def rowA(j, qi, PTt, recips):
            qs, h = blocks[j]
            i = 4 * qs + qi
            f = i // 4      # fully-allowed 512-chunks
            dsub = i % 4    # full 128-subtiles in the diagonal chunk
            sums = stat.tile([P, 4], f32, tag="sums")
            ncol = 0
            lhsQ = QTt[:, h, i * P : (i + 1) * P]
            for c in range(f):
                S1t = s1ps.tile([P, 512], f32, tag="S1")
                nc.tensor.matmul(
                    S1t,
                    lhsQ,
                    KTt[:, c * 512 : (c + 1) * 512],
                    start=True,
                    stop=True,
                )
                Pt = ppool.tile([P, 512], P_DT, tag="P1")
                nc.scalar.activation(
                    out=Pt,
                    in_=S1t,
                    func=AF.Exp,
                    accum_out=sums[:, ncol : ncol + 1],
                )
                ncol += 1
                nc.sync.dma_start_transpose(
                    out=PTt[:, 4 * c : 4 * c + 4, qi * P : (qi + 1) * P],
                    in_=Pt,
                )
            # diagonal chunk, truncated to (dsub+1)*128 columns
            w = (dsub + 1) * P
            S1t = s1ps.tile([P, 512], f32, tag="S1")
            nc.tensor.matmul(
                S1t[:, 0:w],
                lhsQ,
                KTt[:, f * 512 : f * 512 + w],
                start=True,
                stop=True,
            )
            nc.vector.tensor_add(S1t[:, dsub * P : w], S1t[:, dsub * P : w], trit)
            Pt = ppool.tile([P, 512], P_DT, tag="P1")
            nc.scalar.activation(
                out=Pt[:, 0:w],
                in_=S1t[:, 0:w],
                func=AF.Exp,
                accum_out=sums[:, ncol : ncol + 1],
            )
            ncol += 1
            nc.sync.dma_start_transpose(
                out=PTt[:, 4 * f : 4 * f + dsub + 1, qi * P : (qi + 1) * P],
                in_=Pt[:, 0:w],
            )
            # masked-out subtiles of the diagonal chunk are never read: the
            # PV matmuls for diagonal key-tiles are narrowed to the live
            # query columns instead
            denom = stat.tile([P, 1], f32, tag="denom")
            nc.vector.tensor_reduce(
                out=denom, in_=sums[:, 0:ncol], axis=mybir.AxisListType.X, op=A_.add
            )
            recip = stat.tile([P, 1], f32, tag="recip")
            nc.vector.reciprocal(recip, denom)
            recips.append(recip)

        def open_A(j):
            PTt = ptsb.tile([P, ST, 512], P_DT, tag="PT")
            recips = []
            state[j] = (PTt, recips)
            return PTt, recips

        def stage_B(j):
            qs, h = blocks[j]
            PTt, recips = state.pop(j)
            nkt = 4 * qs + 4
            At = atps.tile([P, 512], f32, tag="At")
            for n in range(nkt):
                # diagonal key-tiles only reach query columns >= off
                off = max(0, n - 4 * qs) * P
                nc.tensor.matmul(
                    At[:, off:512],
                    Vt[:, n, :],
                    PTt[:, n, off:512],
                    start=(n == 0),
                    stop=(n == nkt - 1),
                )
            Atsb = atsb.tile([P, 512], P_DT, tag="Atsb")
            nc.vector.tensor_copy(out=Atsb, in_=At)
            Ap = aps.tile([P, 512], P_DT, tag="Ap")
            for qi in range(4):
                nc.tensor.transpose(
                    Ap[:, qi * P : (qi + 1) * P],
                    Atsb[:, qi * P : (qi + 1) * P],
                    idP,
                )
            hb = h % 2

            def dv(Aarr):
                return Aarr[h // 2][:, i * 2 * P : (i + 1) * 2 * P].rearrange(
                    "a (dd j) -> a dd j", dd=2
                )[:, :, hb * 64 : hb * 64 + 64].rearrange("a dd p -> a p dd")

            for qi in range(4):
                i = 4 * qs + qi
                # normalize on DVE, then split into fp8 hi + residual lo for
                # the DoubleRow output projection
                th = atsb.tile([P, P], f32, tag="th")
                nc.vector.tensor_scalar_mul(
                    th, Ap[:, qi * P : (qi + 1) * P], recips[qi]
                )
                thv = th.rearrange("a (p two) -> a p two", two=2)
                dh, dl = dv(Aall_h), dv(Aall_l)
                nc.vector.tensor_copy(out=dh, in_=thv)
                nc.vector.tensor_tensor(out=dl, in0=thv, in1=dh, op=A_.subtract)

        # ------------- phase 1 with interleaved attention blocks -------------
        with (
            tc.tile_pool(name="wpool", bufs=1) as wpool,
            tc.tile_pool(name="xpool", bufs=8) as xpool,
            tc.tile_pool(name="cspool", bufs=2) as cspool,
            tc.tile_pool(name="rpool", bufs=2) as rpool,
            tc.tile_pool(name="qps", bufs=1, space="PSUM") as qps,
            tc.tile_pool(name="kvsh", bufs=1, space="PSUM") as kvsh,
        ):
            wTth = wpool.tile([P, DD, EW + 2 * D], f8)
            wTtl = wpool.tile([P, DD, EW + 2 * D], f8)
            wTrh = wTh.rearrange("(t p) e -> p t e", p=P)
            wTrl = wTl.rearrange("(t p) e -> p t e", p=P)

            XG = min(8, DD)  # dd-tiles per streamed x chunk
            NG = DD // XG
            # interleave s-tile-0 x chunks with weight loads (weights in 8
            # sub-loads per array so the first matmuls start early)
            st0_x = []
            for g in range(NG):
                xc = xpool.tile([P, XG, 2, P], f8, tag="xc")
                nc.sync.dma_start(out=xc, in_=xTc[:, 0, g * XG : (g + 1) * XG, :, :])
                st0_x.append(xc)
                for half in range(2):
                    gw = 2 * g + half
                    nc.sync.dma_start(
                        out=wTth[:, gw * 4 : (gw + 1) * 4, :],
                        in_=wTrh[:, gw * 4 : (gw + 1) * 4, :],
                    )
                    nc.sync.dma_start(
                        out=wTtl[:, gw * 4 : (gw + 1) * 4, :],
                        in_=wTrl[:, gw * 4 : (gw + 1) * 4, :],
                    )
            def qkv_terms(Qp, KVp, xc, g, first_flag=True):
                for tp in range(XG // 2):
                    t = g * XG + 2 * tp
                    first = t == 0 and first_flag
                    last = t == DD - 2
                    lh = xc[:, 2 * tp : 2 * tp + 2, 0, :]
                    ll = xc[:, 2 * tp : 2 * tp + 2, 1, :]
                    terms = ((lh, wTth), (lh, wTtl), (ll, wTth))
                    for k, (lhsT, wt) in enumerate(terms):
                        nc.tensor.matmul(
                            Qp,
                            lhsT,
                            wt[:, t : t + 2, 0:EW],
                            start=(first and k == 0),
                            stop=(last and k == 2),
                            perf_mode=DR,
                        )
                    for k, (lhsT, wt) in enumerate(terms):
                        nc.tensor.matmul(
                            KVp,
                            lhsT,
                            wt[:, t : t + 2, EW : EW + 2 * D],
                            start=(first and k == 0),
                            stop=(last and k == 2),
                            perf_mode=DR,
                        )

            # s-tiles 0-2 are tripled: the weight stream is the DMA
            # bottleneck at kernel start (w hi+lo ~21us vs 7.7us PE per
            # s-tile), so each weight granule feeds three s-tiles' matmuls
            # (s1/s2 borrow PSUM from the still-idle attention pools)
            cst0 = cspool.tile([P, EW], P_DT, tag="cs")
            nc.sync.dma_start(out=cst0, in_=cs[0:P, :])
            cst1 = cspool.tile([P, EW], P_DT, tag="cs")
            nc.sync.dma_start(out=cst1, in_=cs[P : 2 * P, :])
            Qp0 = qps.tile([P, EW], f32, tag="Qp")
            KVp0 = kvsh.tile([P, 2 * D], f32, tag="KVp")
            Qp1 = s1ps.tile([P, 512], f32, tag="S1")
            KVt1 = atps.tile([P, 512], f32, tag="At")
            KVp1 = KVt1[:, 0 : 2 * D]
            Qp2 = s1ps.tile([P, 512], f32, tag="S1")
            KVp2 = kvsh.tile([P, 2 * D], f32, tag="KVx")
            st1_x = []
            st2_x = []
            for g in range(NG):
                xc1 = xpool.tile([P, XG, 2, P], f8, tag="xc")
                nc.sync.dma_start(out=xc1, in_=xTc[:, 1, g * XG : (g + 1) * XG, :, :])
                st1_x.append(xc1)
                xc2 = xpool.tile([P, XG, 2, P], f8, tag="xc")
                nc.sync.dma_start(out=xc2, in_=xTc[:, 2, g * XG : (g + 1) * XG, :, :])
                st2_x.append(xc2)
                qkv_terms(Qp0, KVp0, st0_x[g], g)
                qkv_terms(Qp1, KVp1, xc1, g)
                qkv_terms(Qp2, KVp2, xc2, g)

            for st in range(ST):
                j = st - 4  # attention block woven into this s-tile
                if j >= 0:
                    PTt, recips = open_A(j)
                if st == 0:
                    Qp, KVp, cst = Qp0, KVp0, cst0
                elif st == 1:
                    Qp, KVp, cst = Qp1, KVp1, cst1
                elif st == 2:
                    cst = cspool.tile([P, EW], P_DT, tag="cs")
                    nc.sync.dma_start(out=cst, in_=cs[st * P : (st + 1) * P, :])
                    Qp, KVp = Qp2, KVp2
                else:
                    cst = cspool.tile([P, EW], P_DT, tag="cs")
                    nc.sync.dma_start(out=cst, in_=cs[st * P : (st + 1) * P, :])
                    Qp = qps.tile([P, EW], f32, tag="Qp")
                    KVp = kvsh.tile([P, 2 * D], f32, tag="KVp")
                for g in range(DD // XG if st >= 3 else 0):
                    xc = xpool.tile([P, XG, 2, P], f8, tag="xc")
                    nc.sync.dma_start(
                        out=xc, in_=xTc[:, st, g * XG : (g + 1) * XG, :, :]
                    )
                    qkv_terms(Qp, KVp, xc, g)
                    # one attention row between x-chunk groups keeps ACT fed
                    # while PE grinds the projection matmuls
                    if j >= 0:
                        rowA(j, g, PTt, recips)

                # free the Q/KV PSUM banks fast: one copy each, rope reads SBUF
                qsb = rpool.tile([P, EW], f32, tag="qsb")
                nc.vector.tensor_copy(out=qsb, in_=Qp)
                kvsb = rpool.tile([P, 2 * D], f32, tag="kvsb")
                nc.vector.tensor_copy(out=kvsb, in_=KVp)

                def ttr_ew(out_, in0, in1, op):
                    nc.vector.tensor_tensor(out=out_, in0=in0, in1=in1, op=op)

                HF = EW // 2
                rq = rpool.tile([P, EW], P_DT, tag="rq")
                t1 = rpool.tile([P, HF], f32, tag="t1")
                t2 = rpool.tile([P, HF], f32, tag="t2")
                q_ev, q_od = qsb[:, 0:EW:2], qsb[:, 1:EW:2]
                cosr, sinr = cst[:, 0:HF], cst[:, HF : 2 * HF]
                ttr_ew(t1, q_ev, cosr, A_.mult)
                ttr_ew(t2, q_od, sinr, A_.mult)
                ttr_ew(rq[:, 0:EW:2], t1, t2, A_.subtract)
                ttr_ew(t1, q_ev, sinr, A_.mult)
                ttr_ew(t2, q_od, cosr, A_.mult)
                ttr_ew(rq[:, 1:EW:2], t1, t2, A_.add)

                rk = rpool.tile([P, D], P_DT, tag="rk")
                k_ev, k_od = kvsb[:, 0:D:2], kvsb[:, 1:D:2]
                cosk, sink = cst[:, 0 : D // 2], cst[:, HF : HF + D // 2]
                ttr_ew(t1[:, 0 : D // 2], k_ev, cosk, A_.mult)
                ttr_ew(t2[:, 0 : D // 2], k_od, sink, A_.mult)
                ttr_ew(rk[:, 0:D:2], t1[:, 0 : D // 2], t2[:, 0 : D // 2], A_.subtract)
                ttr_ew(t1[:, 0 : D // 2], k_ev, sink, A_.mult)
                ttr_ew(t2[:, 0 : D // 2], k_od, cosk, A_.mult)
                ttr_ew(rk[:, 1:D:2], t1[:, 0 : D // 2], t2[:, 0 : D // 2], A_.add)

                nc.scalar.activation(
                    out=Vt[:, st, :],
                    in_=kvsb[:, D : 2 * D],
                    func=AF.Copy,
                    scale=float(1.0 / WS1),
                )

                # transposes into [d, s] layouts via the DMA xbar: frees the
                # PE/DVE cycles and the TT PSUM bank
                for h in range(NH):
                    nc.sync.dma_start_transpose(
                        out=QTt[:, h, st * P : (st + 1) * P],
                        in_=rq[:, h * P : (h + 1) * P],
                    )
                nc.sync.dma_start_transpose(
                    out=KTt[:, st * P : (st + 1) * P], in_=rk
                )

                if j >= 1:
                    stage_B(j - 1)
                if st >= 12:
                    drip_w0(st - 12)

        # ---------------- tail: qs=3 blocks + output projection ----------------
        with (
            tc.tile_pool(name="wopool", bufs=2) as wopool,
            tc.tile_pool(name="osb", bufs=3) as osb,
            tc.tile_pool(name="ops", bufs=3, space="PSUM") as ops,
        ):
            wot_tiles = {}
            wqueue = []
            units = {}
            wohr = woh.rearrange("(t p) m -> p t m", p=P)
            wolr = wol.rearrange("(t p) m -> p t m", p=P)

            def alloc_wot(mc):
                wth = wopool.tile([P, JT, 512], f8, tag="woh")
                wtl = wopool.tile([P, JT, 512], f8, tag="wol")
                wot_tiles[mc] = (wth, wtl)
                # quarter q covers t-pairs [4q, 4q+4); hi then lo
                for qq in range(4):
                    wqueue.append((mc, qq, 0))
                    wqueue.append((mc, qq, 1))

            def drip(n):
                for _ in range(min(n, len(wqueue))):
                    mc, qq, lo = wqueue.pop(0)
                    wt = wot_tiles[mc][lo]
                    src = wolr if lo else wohr
                    nc.sync.dma_start(
                        out=wt[:, qq * (JT // 4) : (qq + 1) * (JT // 4), :],
                        in_=src[
                            :,
                            qq * (JT // 4) : (qq + 1) * (JT // 4),
                            mc * 512 : (mc + 1) * 512,
                        ],
                    )

            def load_wot(mc):
                alloc_wot(mc)
                drip(8)

            def unit_mms(mc, it, tps, start, stop):
                wth, wtl = wot_tiles[mc]
                if (mc, it) in units:
                    O = units[(mc, it)]
                else:
                    O = ops.tile([P, 512], f32, tag="O")
                    units[(mc, it)] = O
                for n, tp in enumerate(tps):
                    lh = Aall_h[it][:, 2 * tp * P : (2 * tp + 2) * P].rearrange(
                        "a (two s) -> a two s", two=2
                    )
                    ll = Aall_l[it][:, 2 * tp * P : (2 * tp + 2) * P].rearrange(
                        "a (two s) -> a two s", two=2
                    )
                    rh = wth[:, 2 * tp : 2 * tp + 2, :]
                    rl = wtl[:, 2 * tp : 2 * tp + 2, :]
                    for k, (lhsT, rhs) in enumerate(((lh, rh), (lh, rl), (ll, rh))):
                        nc.tensor.matmul(
                            O,
                            lhsT,
                            rhs,
                            start=(start and n == 0 and k == 0),
                            stop=(stop and n == len(tps) - 1 and k == 2),
                            perf_mode=DR,
                        )

            def unit_fin(mc, it):
                O = units.pop((mc, it))
                Ot = osb.tile([P, 512], P_DT, tag="Ot")
                nc.scalar.activation(
                    out=Ot, in_=O, func=AF.Copy, scale=float(1.0 / WS3)
                )
                nc.sync.dma_start(
                    out=out[it * P : (it + 1) * P, mc * 512 : (mc + 1) * 512],
                    in_=Ot,
                )

            def unit(mc, it):
                unit_mms(mc, it, range(ST), True, True)
                unit_fin(mc, it)

            wot_tiles[0] = (w0h, w0l)  # prefetched during the interleave
            alloc_wot(1)
            alloc_wot(2)
            # qs=3 attention blocks (need all 16 s-tiles), pipelined; wo
            # chunk loads drip between rows so they never block the
            # latency-critical P transposes on the DMA engines. Phase-3
            # units split: t0-7 accumulation only needs qs<=1 heads (final
            # long before the tail), t8-15 needs the qs=3 heads.
            HALF1, HALF2 = range(0, 8), range(8, 16)
            # these two first-half units depend only on qs<=1 heads (done
            # mid-interleave) and the prefetched mc0 chunk: they fill the
            # PE idle at tail start
            unit_mms(0, 0, HALF1, True, False)
            unit_mms(0, 1, HALF1, True, False)
            PTt, recips = open_A(12)
            for qi in range(4):
                rowA(12, qi, PTt, recips)
                drip(2)
            stage_B(11)
            for j in (13, 14, 15):
                PTt, recips = open_A(j)
                for qi in range(4):
                    rowA(j, qi, PTt, recips)
                    drip(2)
                stage_B(j - 1)
                if j == 13:
                    unit_mms(1, 0, HALF1, True, False)
                elif j == 14:
                    # Aall[0] complete after B(13)
                    unit_mms(0, 0, HALF2, False, True)
                    unit_fin(0, 0)
                    unit_mms(2, 0, HALF1, True, False)
                elif j == 15:
                    unit_mms(1, 0, HALF2, False, True)
                    unit_fin(1, 0)
            stage_B(15)
            drip(len(wqueue))
            unit_mms(2, 0, HALF2, False, True)
            unit_fin(2, 0)
            unit_mms(0, 1, HALF2, False, True)
            unit_fin(0, 1)
            # preloaded chunks' it=1 work covers the in-flight loads of the
            # later chunks (slot for mc+2 frees as soon as mc's last unit
            # is emitted)
            def load_wot_pre(mc):
                # rotate the wopre slot (mc0's chunk is consumed by now)
                wth = wopre.tile([P, JT, 512], f8, tag="wph")
                wtl = wopre.tile([P, JT, 512], f8, tag="wpl")
                wot_tiles[mc] = (wth, wtl)
                for qq in range(4):
                    wqueue.append((mc, qq, 0))
                    wqueue.append((mc, qq, 1))
                drip(8)

            def unit_split_cols(mc, it):
                # last unit: two column-halves so the closing copy/store
                # overlaps the second half's matmuls
                wth, wtl = wot_tiles[mc]
                for half in range(2):
                    O = ops.tile([P, 512], f32, tag="O")
                    cl, ch = half * 256, (half + 1) * 256
                    for n, tp in enumerate(range(ST)):
                        lh = Aall_h[it][:, 2 * tp * P : (2 * tp + 2) * P].rearrange(
                            "a (two s) -> a two s", two=2
                        )
                        ll = Aall_l[it][:, 2 * tp * P : (2 * tp + 2) * P].rearrange(
                            "a (two s) -> a two s", two=2
                        )
                        rh = wth[:, 2 * tp : 2 * tp + 2, cl:ch]
                        rl = wtl[:, 2 * tp : 2 * tp + 2, cl:ch]
                        for k, (lhsT, rhs) in enumerate(
                            ((lh, rh), (lh, rl), (ll, rh))
                        ):
                            nc.tensor.matmul(
                                O[:, 0:256],
                                lhsT,
                                rhs,
                                start=(n == 0 and k == 0),
                                stop=(n == ST - 1 and k == 2),
                                perf_mode=DR,
                            )
                    Ot = osb.tile([P, 512], P_DT, tag="Ot")
                    nc.scalar.activation(
                        out=Ot[:, 0:256],
                        in_=O[:, 0:256],
                        func=AF.Copy,
                        scale=float(1.0 / WS3),
                    )
                    nc.sync.dma_start(
                        out=out[
                            it * P : (it + 1) * P,
                            mc * 512 + cl : mc * 512 + ch,
                        ],
                        in_=Ot[:, 0:256],
                    )

            for mc in (1, 2, 3, 4, 5, 6, 7):
                if mc >= 3:
                    unit(mc, 0)
                if mc == 7:
                    unit_split_cols(mc, 1)
                else:
                    unit(mc, 1)
                wot_tiles.pop(mc)
                nxt = mc + 2 if mc >= 3 else {1: 3, 2: 4}.get(mc)
                if nxt is not None and nxt < MC and nxt not in wot_tiles:
                    load_wot(nxt)
                if mc == 2:
                    load_wot(5)

    nc.compile()
    return nc


def is_pure_causal(mask, SEQ):
    """True iff mask[i,j] == 0 for j<=i and <= NEG_THRESH for j>i."""
    m = np.asarray(mask, np.float32)
    if m.shape != (SEQ, SEQ):
        return False
    j = np.arange(SEQ)
    allowed = j[None, :] <= j[:, None]
    return bool((m[allowed] == 0).all() and (m[~allowed] <= NEG_THRESH).all())


def make_rope_tables(cos_freq, sin_freq, SEQ, scale_quarter):
    cos_t = np.tile(np.asarray(cos_freq, np.float32) * scale_quarter, (1, NH))
    sin_t = np.tile(np.asarray(sin_freq, np.float32) * scale_quarter, (1, NH))
    return np.ascontiguousarray(
        np.concatenate([cos_t, sin_t], axis=1).astype(np.float32)
    )




_BUILD_CACHE = {}


def kernel(
    x,
    cos_freq,
    sin_freq,
    positions,
    mask,
    wq,
    wk,
    wv,
    wo,
    _trace=False,
):
    import sys

    if "/opt/trn_rl_repo" not in sys.path:
        sys.path.insert(0, "/opt/trn_rl_repo")
    from concourse.bass_utils import run_bass_kernel_spmd
    import ml_dtypes

    x = np.asarray(x, np.float32)
    mask = np.asarray(mask, np.float32)
    wq = np.asarray(wq, np.float32)
    wk = np.asarray(wk, np.float32)
    wv = np.asarray(wv, np.float32)
    wo = np.asarray(wo, np.float32)
    SEQ, DIM = x.shape
    assert wq.shape[0] == CORES * NH * D and wk.shape[0] == CORES * D
    assert 2 * SEQ == wq.shape[0], "flatten structure requires H*D == 2*SEQ"

    bf16 = ml_dtypes.bfloat16
    f8 = ml_dtypes.float8_e4m3
    ST_, DD_ = SEQ // P, DIM // P

    if is_pure_causal(mask, SEQ):
        key = (SEQ, DIM, "causal")
        if key not in _BUILD_CACHE:
            _BUILD_CACHE[key] = build_attention_v7(SEQ, DIM)
        nc = _BUILD_CACHE[key]

        def hilo(a):
            hi = np.ascontiguousarray(a).astype(f8)
            lo = np.ascontiguousarray(a - hi.astype(np.float32)).astype(f8)
            return hi, lo

        # fold sqrt(scale) and the 1/WS1 weight pre-scale into rope tables
        scale_quarter = np.float32(D ** -0.25 / WS1)
        cs = make_rope_tables(cos_freq, sin_freq, SEQ, scale_quarter).astype(bf16)
        xT = np.ascontiguousarray(x.reshape(ST_, P, DD_, P).transpose(3, 0, 2, 1))
        xh_, xl_ = hilo(xT)
        xTc = np.ascontiguousarray(np.stack([xh_, xl_], axis=3))
        # wo row-blocks permuted so DoubleRow contraction pairs are adjacent
        JT_ = 2 * SEQ // P
        perm = [(jt % 2) * (JT_ // 2) + jt // 2 for jt in range(JT_)]
        woP = np.ascontiguousarray(
            (wo.T * np.float32(WS3)).reshape(JT_, P, DIM)[perm].reshape(2 * SEQ, DIM)
        )
        woh, wol = hilo(woP)
        tri = np.ascontiguousarray(mask[0:P, 0:P])

        in_maps = []
        for c in range(CORES):
            w_c = np.concatenate(
                [
                    wq[c * NH * D : (c + 1) * NH * D],
                    wk[c * D : (c + 1) * D],
                    wv[c * D : (c + 1) * D],
                ],
                axis=0,
            )
            wTh_, wTl_ = hilo(w_c.T * np.float32(WS1))
            in_maps.append(
                {
                    "xTc": xTc,
                    "wTh": wTh_,
                    "wTl": wTl_,
                    "cs": cs,
                    "tri": tri,
                    "woh": woh,
                    "wol": wol,
                }
            )
        res = run_bass_kernel_spmd(nc, in_maps, list(range(CORES)), trace=_trace)
        outp = np.concatenate(
            [np.asarray(res.results[c]["out"]) for c in range(CORES)], axis=0
        ).astype(np.float32)
        if _trace:
            return outp, res
        return outp

    # ---------------- general-mask fallback (v1 kernel) ----------------
    plan, blocks = analyze_mask(mask, SEQ)
    n_uniq = len(blocks)
    key = (SEQ, DIM, tuple(tuple(r) for r in plan))
    if key not in _BUILD_CACHE:
        _BUILD_CACHE[key] = build_attention_nc(SEQ, DIM, plan, n_uniq)
    nc = _BUILD_CACHE[key]

    scale_quarter = np.float32(D ** -0.25)
    csf = make_rope_tables(cos_freq, sin_freq, SEQ, scale_quarter)
    xT = np.ascontiguousarray(
        x.reshape(ST_, P, DD_, P).transpose(3, 0, 2, 1)
    ).astype(bf16)
    woT = np.ascontiguousarray(wo.T).astype(bf16)
    if n_uniq:
        mbs = np.ascontiguousarray(np.stack(blocks, axis=0))
    else:
        mbs = np.zeros((1, P, 512), np.float32)

    in_maps = []
    for c in range(CORES):
        w_c = np.concatenate(
            [
                wq[c * NH * D : (c + 1) * NH * D],
                wk[c * D : (c + 1) * D],
                wv[c * D : (c + 1) * D],
            ],
            axis=0,
        )
        in_maps.append(
            {
                "xT": xT,
                "wT": np.ascontiguousarray(w_c.T).astype(bf16),
                "cs": csf,
                "maskb": mbs,
                "woT": woT,
            }
        )
    res = run_bass_kernel_spmd(nc, in_maps, list(range(CORES)), trace=_trace)
    outp = np.concatenate(
        [np.asarray(res.results[c]["out"]) for c in range(CORES)], axis=0
    ).astype(np.float32)
    if _trace:
        return outp, res
    return outp
